# revision 1
# baseline (speedup 1.0000x reference)
"""Trainium2 Bass kernel for a 3-layer ResGatedGraphConv GNN (ClinicalGatedGCN).

Strategy (8 NeuronCores, SPMD), v2:
  - Nodes partitioned into 8 contiguous ranges (rank-blocked, padded to 128).
    Edges assigned to the rank owning their dst node, sorted by
    (rank, src-epoch, dst-group, dst) on the host.
  - Each rank computes the full [q|v] node table into local HBM (split at row
    32768 so int16 gather indices reach every row in two epochs). One
    dma_gather per (epoch, dst-group) fetches the src rows of [q|v] — this is
    the ONLY GpSimd gather; k[dst] is expanded on the PE via a host-shipped
    0/1 dst-selector S_T (matmul S_T.T @ k_group), with attr*We folded into
    the same PSUM accumulation via a K=1 rank-1 matmul.
  - The segment-sum over dst is a PE matmul against the host-shipped selector
    S (edge-major). S/S_T depend only on the edge structure, so they are
    built once on the host and streamed from DRAM each layer.
  - Gather sizes are exact per (epoch, group): nt = ceil(max-over-ranks
    count / 128) tiles, carried in meta (shapes shared across the SPMD
    program).
  - The qv table is built hi-rows-first each layer and epoch-1 gathers are
    issued before the lo rows are written, so Q7 descriptor generation
    overlaps table construction.
  - h stays feature-major; per layer the updated h slice is AllGather'd.
    Mean-pool per graph is a matmul against a host-built indicator with
    1/cnt folded in; partial pools are AllGather'd and summed; the tiny
    classifier runs on every core.
"""

import numpy as np
import ml_dtypes

import concourse.bacc as bacc
import concourse.bass as bass
import concourse.mybir as mybir
import concourse.tile as tile
from concourse.bass_utils import run_bass_kernel_spmd
from concourse.masks import make_identity

F32 = mybir.dt.float32
BF16 = mybir.dt.bfloat16
I16 = mybir.dt.int16
AF = mybir.ActivationFunctionType
OP = mybir.AluOpType

# ---------------- problem constants (hardcoded per spec) ----------------
N, E, H, G, NCLIN, NCLS = 50000, 800000, 128, 64, 16, 2
NLAYER = 3
EPS = 1e-5
SLOPE = 0.01
R = 8                      # ranks / NeuronCores
SPLIT = 32768              # int16 gather index limit -> 2 epochs

USE_BF16 = True            # table/h/gate dtype

NPR = (N + R - 1) // R     # real nodes per rank
NGRP = (NPR + 127) // 128  # 128-node groups per rank
NPAD = NGRP * 128          # padded nodes per rank
NTOT = R * NPAD            # rank-blocked total rows


def _np_dtab(use_bf16):
    return ml_dtypes.bfloat16 if use_bf16 else np.float32


def wrap_idxs_block(idx):
    """Wrap one gather call's indices: idx j -> [j%16, j//16], tiled to 128 parts."""
    n = len(idx)
    assert n % 16 == 0
    w = np.asarray(idx, np.int16).reshape(n // 16, 16).T
    return np.tile(w, (8, 1))


# ---------------------------------------------------------------------------
# host-side preprocessing
# ---------------------------------------------------------------------------

def prep(inputs, use_bf16=None):
    if use_bf16 is None:
        use_bf16 = USE_BF16
    dtab = _np_dtab(use_bf16)
    x = np.asarray(inputs["x"], np.float32)
    edge_index = np.asarray(inputs["edge_index"])
    edge_attr = np.asarray(inputs["edge_attr"], np.float32)[:, 0]
    batch = np.asarray(inputs["batch"]).astype(np.int64)
    clinical = np.asarray(inputs["clinical"], np.float32)
    Wk, bk = np.asarray(inputs["Wk"], np.float32), np.asarray(inputs["bk"], np.float32)
    Wq, bq = np.asarray(inputs["Wq"], np.float32), np.asarray(inputs["bq"], np.float32)
    Wv, bv = np.asarray(inputs["Wv"], np.float32), np.asarray(inputs["bv"], np.float32)
    Ws, bs = np.asarray(inputs["Ws"], np.float32), np.asarray(inputs["bs"], np.float32)
    We, be = np.asarray(inputs["We"], np.float32), np.asarray(inputs["be"], np.float32)
    gamma = np.asarray(inputs["gamma"], np.float32)
    beta = np.asarray(inputs["beta"], np.float32)
    rmean = np.asarray(inputs["rmean"], np.float32)
    rvar = np.asarray(inputs["rvar"], np.float32)
    Wc, bc = np.asarray(inputs["Wc"], np.float32), np.asarray(inputs["bc"], np.float32)

    src = edge_index[0].astype(np.int64)
    dst = edge_index[1].astype(np.int64)

    # BN folded: A*x + B
    A = gamma / np.sqrt(rvar + EPS)
    B = beta - rmean * A
    bgate = bk + bq + be          # folded into k table bias
    rb_row = (src // NPR) * NPAD + (src % NPR)

    e_rank = dst // NPR
    epoch = (rb_row >= SPLIT).astype(np.int64)
    dst_local = dst - e_rank * NPR
    group = dst_local // 128
    dst_rel = dst_local % 128

    # per (ep, rank, group) counts -> exact per-(ep,group) tile counts
    counts = np.zeros((2, R, NGRP), np.int64)
    np.add.at(counts, (epoch, e_rank, group), 1)
    nt_l = [np.ceil(counts[ep].max(axis=0) / 128).astype(int) for ep in (0, 1)]
    off_l = [np.concatenate([[0], np.cumsum(nt)]).astype(int) for nt in nt_l]

    # graph counts for mean pooling
    cntg = np.bincount(batch, minlength=G).astype(np.float32)
    inv_cnt = 1.0 / np.maximum(cntg, 1.0)

    order = np.lexsort((dst, group, epoch, e_rank))
    src_s, dst_rel_s, attr_s = rb_row[order], dst_rel[order], edge_attr[order]
    ep_s, rank_s, grp_s = epoch[order], e_rank[order], group[order]
    key = ((rank_s * 2 + ep_s) * NGRP + grp_s)
    starts = np.searchsorted(key, np.arange(R * 2 * NGRP + 1))

    bias_k = np.zeros((NLAYER, 128, H), np.float32)
    for l in range(NLAYER):
        bias_k[l, :, :] = bgate[l][None, :]
    bias_v = np.zeros((NLAYER, 128, 2 * H), np.float32)
    for l in range(NLAYER):
        bias_v[l, :, H:2 * H] = bv[l][None, :]
    has_bias_k = bool(np.any(bias_k != 0))
    has_bias_qv = bool(np.any(bias_v != 0))
    has_bs = bool(np.any(bs != 0))
    has_bc = bool(np.any(bc != 0))
    We_row = We[:, 0, :].reshape(NLAYER, 1, H)

    x_rb = np.zeros((R * 128, NPAD), np.float32)
    for r in range(R):
        lo, hi = r * NPR, min((r + 1) * NPR, N)
        x_rb[r * 128:(r + 1) * 128, 0:hi - lo] = x[lo:hi].T

    in_maps = []
    for r in range(R):
        ep_arrs = {}
        for ep in (0, 1):
            nt = nt_l[ep]
            off = off_l[ep]
            tot_tiles = int(off[-1])
            slots = tot_tiles * 128
            gidx = np.zeros((128, tot_tiles * 8), np.int16)
            S = np.zeros((128, slots), dtab)
            ST = np.zeros((128, slots), dtab)
            attr_row = np.zeros((1, slots), dtab)
            for g in range(NGRP):
                ntg = int(nt[g])
                if ntg == 0:
                    continue
                k = (r * 2 + ep) * NGRP + g
                s0 = int(starts[k])
                n = int(counts[ep, r, g])
                o = int(off[g])
                idx = np.zeros((ntg * 128,), np.int64)
                idx[:n] = src_s[s0:s0 + n] - ep * SPLIT
                gidx[:, o * 8:(o + ntg) * 8] = wrap_idxs_block(idx)
                j = np.arange(n)
                t = j // 128
                p = j % 128
                drel = dst_rel_s[s0:s0 + n].astype(np.int64)
                col = (o + t) * 128
                S[p, col + drel] = 1
                ST[drel, col + p] = 1
                attr_row[0, o * 128 + j] = attr_s[s0:s0 + n].astype(dtab)
            ep_arrs[ep] = (gidx, S, ST, attr_row)
        IndT = np.zeros((NPAD, G), np.float32)
        lo, hi = r * NPR, min((r + 1) * NPR, N)
        IndT[np.arange(hi - lo), batch[lo:hi]] = inv_cnt[batch[lo:hi]]
        im = {
            "x_rb": x_rb.astype(dtab),
            "xT_loc": x_rb[r * 128:(r + 1) * 128].astype(dtab),
            "Wk": Wk.astype(dtab), "Wq": Wq.astype(dtab), "Wv": Wv.astype(dtab),
            "Ws": Ws.astype(dtab),
            "We_row": We_row.astype(dtab),
            "bias_k": bias_k,
            "bias_qv": bias_v,
            "bs_col": bs.reshape(NLAYER, H, 1),
            "A_col": A.reshape(NLAYER, H, 1),
            "B_col": B.reshape(NLAYER, H, 1),
            "gidx0": ep_arrs[0][0], "S0": ep_arrs[0][1], "ST0": ep_arrs[0][2],
            "attr0": ep_arrs[0][3],
            "gidx1": ep_arrs[1][0], "S1": ep_arrs[1][1], "ST1": ep_arrs[1][2],
            "attr1": ep_arrs[1][3],
            "IndT": IndT.astype(dtab),
            "clinT": clinical.T.copy(),
            "Wc_h": Wc[0:H], "Wc_c": Wc[H:H + NCLIN],
            "bc_rep": np.tile(bc, (G, 1)),
        }
        in_maps.append(im)
    meta = dict(NT0=tuple(int(v) for v in nt_l[0]),
                NT1=tuple(int(v) for v in nt_l[1]),
                has_bias_k=has_bias_k, has_bias_qv=has_bias_qv,
                has_bs=has_bs, has_bc=has_bc, use_bf16=use_bf16)
    return in_maps, meta


# ---------------------------------------------------------------------------
# device program
# ---------------------------------------------------------------------------

def build(meta):
    use_bf16 = meta["use_bf16"]
    parts = meta.get("parts", 4)
    DT = BF16 if use_bf16 else F32
    NT_L = [list(meta["NT0"]), list(meta["NT1"])]
    OFF_L = [np.concatenate([[0], np.cumsum(nt)]).astype(int) for nt in NT_L]
    TOT = [int(o[-1]) for o in OFF_L]
    TMAX = max(max(NT_L[0]), max(NT_L[1]))
    KQC = 4                           # kq psum chunk (tiles, 1 PSUM bank)

    nc = bacc.Bacc("TRN2", target_bir_lowering=False, debug=False, num_devices=R)

    def din(name, shape, dt):
        return nc.dram_tensor(name, shape, dt, kind="ExternalInput").ap()

    t_x_rb = din("x_rb", [R * 128, NPAD], DT)
    t_xT_loc = din("xT_loc", [128, NPAD], DT)
    t_Wk = din("Wk", [NLAYER, H, H], DT)
    t_Wq = din("Wq", [NLAYER, H, H], DT)
    t_Wv = din("Wv", [NLAYER, H, H], DT)
    t_Ws = din("Ws", [NLAYER, H, H], DT)
    t_We_row = din("We_row", [NLAYER, 1, H], DT)
    t_bias_k = din("bias_k", [NLAYER, 128, H], F32)
    t_bias_qv = din("bias_qv", [NLAYER, 128, 2 * H], F32)
    t_bs = din("bs_col", [NLAYER, H, 1], F32)
    t_A = din("A_col", [NLAYER, H, 1], F32)
    t_B = din("B_col", [NLAYER, H, 1], F32)
    t_gidx = [din("gidx0", [128, TOT[0] * 8], I16),
              din("gidx1", [128, TOT[1] * 8], I16)]
    t_S = [din("S0", [128, TOT[0] * 128], DT),
           din("S1", [128, TOT[1] * 128], DT)]
    t_ST = [din("ST0", [128, TOT[0] * 128], DT),
            din("ST1", [128, TOT[1] * 128], DT)]
    t_attr = [din("attr0", [1, TOT[0] * 128], DT),
              din("attr1", [1, TOT[1] * 128], DT)]
    t_IndT = din("IndT", [NPAD, G], DT)
    t_clinT = din("clinT", [NCLIN, G], F32)
    t_Wc_h = din("Wc_h", [H, NCLS], F32)
    t_Wc_c = din("Wc_c", [NCLIN, NCLS], F32)
    t_bc = din("bc_rep", [G, NCLS], F32)

    t_out = nc.dram_tensor("out", [G, NCLS], F32, kind="ExternalOutput").ap()

    qv_lo = nc.dram_tensor("qv_lo", [SPLIT, 2 * H], DT).ap()
    qv_hi = nc.dram_tensor("qv_hi", [NTOT - SPLIT, 2 * H], DT).ap()
    h_loc = [nc.dram_tensor(f"h_loc{l}", [128, NPAD], DT).ap() for l in range(2)]
    ag_out = [nc.dram_tensor(f"ag_out{l}", [R * 128, NPAD], DT,
                             addr_space="Shared").ap() for l in range(2)]
    pool_in = nc.dram_tensor("pool_in", [G, H], F32).ap()
    pool_out = nc.dram_tensor("pool_out", [R * G, H], F32, addr_space="Shared").ap()

    chunks = []
    c0 = 0
    while c0 < NPAD:
        csz = min(512, NPAD - c0)
        chunks.append((c0, csz))
        c0 += csz

    with tile.TileContext(nc) as tc:
        import contextlib
        with contextlib.ExitStack() as ctx:
            consts = ctx.enter_context(tc.tile_pool(name="consts", bufs=1))
            hsb = ctx.enter_context(tc.tile_pool(name="hsb", bufs=1))
            h3p = ctx.enter_context(tc.tile_pool(name="h3p", bufs=1))
            ksb = ctx.enter_context(tc.tile_pool(name="ksb", bufs=1))
            lhp = ctx.enter_context(tc.tile_pool(name="lhp", bufs=4))
            stg = ctx.enter_context(tc.tile_pool(name="stg", bufs=4))
            edg = ctx.enter_context(tc.tile_pool(name="edg", bufs=3))
            sel = ctx.enter_context(tc.tile_pool(name="sel", bufs=3))
            edm = ctx.enter_context(tc.tile_pool(name="edm", bufs=3))
            pnode = ctx.enter_context(tc.tile_pool(name="pnode", bufs=2, space="PSUM"))
            pkq = ctx.enter_context(tc.tile_pool(name="pkq", bufs=2, space="PSUM"))
            pedge = ctx.enter_context(tc.tile_pool(name="pedge", bufs=2, space="PSUM"))
            ppool = ctx.enter_context(tc.tile_pool(name="ppool", bufs=1, space="PSUM"))

            _cid = [0]

            def load_const(src_ap, shape, dt):
                _cid[0] += 1
                t = consts.tile(shape, dt, tag=f"c{_cid[0]}_{src_ap.tensor.name}")
                nc.sync.dma_start(t[:], src_ap)
                return t

            W_t = {}
            for nm, tt in (("k", t_Wk), ("q", t_Wq), ("v", t_Wv), ("s", t_Ws)):
                for l in range(NLAYER):
                    W_t[nm, l] = load_const(tt[l], [H, H], DT)
            We_t = [load_const(t_We_row[l], [1, H], DT) for l in range(NLAYER)]
            bias_k_t = [load_const(t_bias_k[l], [128, H], F32)
                        for l in range(NLAYER)] if meta["has_bias_k"] else None
            bias_qv_t = [load_const(t_bias_qv[l], [128, 2 * H], F32)
                         for l in range(NLAYER)] if meta["has_bias_qv"] else None
            bs_t = [load_const(t_bs[l], [H, 1], F32) for l in range(NLAYER)]
            A_t = [load_const(t_A[l], [H, 1], F32) for l in range(NLAYER)]
            B_t = [load_const(t_B[l], [H, 1], F32) for l in range(NLAYER)]
            ident = consts.tile([128, 128], DT)
            make_identity(nc, ident[:])
            if use_bf16:
                identf = consts.tile([128, 128], F32)
                make_identity(nc, identf[:])
            else:
                identf = ident
            gidx_t = [load_const(t_gidx[0], [128, TOT[0] * 8], I16),
                      load_const(t_gidx[1], [128, TOT[1] * 8], I16)]
            clin_t = load_const(t_clinT, [NCLIN, G], F32)
            Wch_t = load_const(t_Wc_h, [H, NCLS], F32)
            Wcc_t = load_const(t_Wc_c, [NCLIN, NCLS], F32)
            bc_t = load_const(t_bc, [G, NCLS], F32) if meta["has_bc"] else None

            hs = hsb.tile([128, NPAD], F32)        # s + agg accumulator
            k_sb = ksb.tile([128, NGRP, H], DT)    # local k table (node-major)
            h3_prev = None
            h3f = None

            # qv sub-block order: hi rows (>= SPLIT) first, then lo rows
            qv_blocks_hi, qv_blocks_lo = [], []
            for rb in range(R):
                for (c0, csz) in chunks:
                    for s in range(csz // 128):
                        row = rb * NPAD + c0 + s * 128
                        (qv_blocks_hi if row >= SPLIT else qv_blocks_lo).append(
                            (rb, c0 + s * 128))

            def make_runs(blocks):
                # group runs of 4 blocks sharing one lh load (512 cols)
                runs = []
                i = 0
                while i < len(blocks):
                    rb0, r0 = blocks[i]
                    run = [(rb0, r0)]
                    while (len(run) < 4 and i + len(run) < len(blocks)):
                        rbn, rn = blocks[i + len(run)]
                        if rbn == rb0 and rn == run[-1][1] + 128:
                            run.append((rbn, rn))
                        else:
                            break
                    i += len(run)
                    runs.append((rb0, r0, len(run)))
                return runs

            def emit_run(rb0, r0, nrun, l, hsrc):
                    csz = 128 * nrun
                    lh = lhp.tile([128, 512], DT, tag="lh")
                    nc.sync.dma_start(
                        lh[:, 0:csz], hsrc[rb0 * 128:(rb0 + 1) * 128,
                                           r0:r0 + csz])
                    for s in range(nrun):
                        row = rb0 * NPAD + r0 + s * 128
                        ps_full = pnode.tile([128, 512], F32, tag="pn")
                        ps = ps_full[:, 0:2 * H]
                        for jj, nm in enumerate(("q", "v")):
                            nc.tensor.matmul(
                                out=ps[:, jj * H:(jj + 1) * H],
                                lhsT=lh[:, s * 128:(s + 1) * 128],
                                rhs=W_t[nm, l][:], start=True, stop=True)
                        st = stg.tile([128, 2 * H], DT, tag="st")
                        if meta["has_bias_qv"]:
                            nc.vector.tensor_tensor(
                                out=st[:], in0=ps[:], in1=bias_qv_t[l][:],
                                op=OP.add)
                        else:
                            nc.scalar.activation(st[:], ps[:], AF.Copy)
                        if row < SPLIT:
                            nc.sync.dma_start(qv_lo[row:row + 128, :], st[:])
                        else:
                            nc.sync.dma_start(
                                qv_hi[row - SPLIT:row - SPLIT + 128, :], st[:])

            def emit_qv(blocks, l, hsrc):
                for (rb0, r0, nrun) in make_runs(blocks):
                    emit_run(rb0, r0, nrun, l, hsrc)

            def emit_edges(ep, l, filler=None):
                nt_list = NT_L[ep]
                off = OFF_L[ep]
                qv_tab = qv_lo if ep == 0 else qv_hi
                for g in range(NGRP):
                    if filler is not None:
                        filler(g)
                    nt = int(nt_list[g])
                    if nt == 0:
                        continue
                    ne = nt * 128
                    o = int(off[g])
                    gt = edg.tile([128, TMAX, 2 * H], DT, tag="g")
                    nc.gpsimd.dma_gather(
                        gt[:, 0:nt, :], qv_tab[:], gidx_t[ep][:, o * 8:(o + nt) * 8],
                        ne, ne, 2 * H, single_packet=(ne <= 512))
                    St = sel.tile([128, TMAX, 128], DT, tag="S")
                    nc.sync.dma_start(
                        St[:, 0:nt, :],
                        t_S[ep][:, o * 128:(o + nt) * 128].rearrange(
                            "p (t d) -> p t d", t=nt))
                    STt = sel.tile([128, TMAX, 128], DT, tag="ST")
                    nc.sync.dma_start(
                        STt[:, 0:nt, :],
                        t_ST[ep][:, o * 128:(o + nt) * 128].rearrange(
                            "p (t d) -> p t d", t=nt))
                    at = sel.tile([1, TMAX * 128], DT, tag="attr")
                    nc.sync.dma_start(at[:, 0:ne],
                                      t_attr[ep][:, o * 128:o * 128 + ne])
                    ktb = edm.tile([128, TMAX, 128], DT, tag="kt")
                    for q0 in range(0, nt, KQC):
                        qn = min(KQC, nt - q0)
                        pk = pkq.tile([128, KQC, 128], F32, tag="kq")
                        for t in range(q0, q0 + qn):
                            nc.tensor.matmul(
                                out=pk[:, t - q0, :], lhsT=STt[:, t, :],
                                rhs=k_sb[:, g, :], start=True, stop=False)
                            nc.tensor.matmul(
                                out=pk[:, t - q0, :],
                                lhsT=at[0:1, t * 128:(t + 1) * 128],
                                rhs=We_t[l][:], start=False, stop=True)
                        nc.vector.tensor_tensor(
                            out=ktb[:, q0:q0 + qn, :], in0=pk[:, 0:qn, :],
                            in1=gt[:, q0:q0 + qn, 0:H], op=OP.add)
                    nc.scalar.activation(ktb[:, 0:nt, :], ktb[:, 0:nt, :],
                                         AF.Sigmoid)
                    msg = edm.tile([128, TMAX, 128], DT, tag="msg")
                    nc.vector.tensor_tensor(out=msg[:, 0:nt, :],
                                            in0=ktb[:, 0:nt, :],
                                            in1=gt[:, 0:nt, H:2 * H], op=OP.mult)
                    pa = pedge.tile([128, 128], F32, tag="pa")
                    for t in range(nt):
                        nc.tensor.matmul(out=pa[:], lhsT=msg[:, t, :],
                                         rhs=St[:, t, :], start=(t == 0),
                                         stop=(t == nt - 1))
                    nc.vector.tensor_tensor(
                        out=hs[:, g * 128:(g + 1) * 128],
                        in0=hs[:, g * 128:(g + 1) * 128], in1=pa[:], op=OP.add)

            for l in range(NLAYER):
                hsrc = t_x_rb if l == 0 else ag_out[l - 1]

                # ---- qv node tables: hi rows first; lo rows interleaved
                # into the ep1 edge phase so PE/DMA build them while Q7 gathers
                emit_qv(qv_blocks_hi, l, hsrc)
                # ---- s-table (feature-major) into hs + local k table in SBUF
                for (c0, csz) in chunks:
                    if l == 0:
                        rhs_t = lhp.tile([128, 512], DT, tag="lh")
                        nc.sync.dma_start(rhs_t[:, 0:csz], t_xT_loc[:, c0:c0 + csz])
                        rhs_ap = rhs_t[:, 0:csz]
                    else:
                        rhs_ap = h3_prev[:, c0:c0 + csz]
                    pss = pnode.tile([128, 512], F32, tag="pn")
                    nc.tensor.matmul(out=pss[:, 0:csz], lhsT=W_t["s", l][:],
                                     rhs=rhs_ap, start=True, stop=True)
                    if meta["has_bs"]:
                        nc.scalar.activation(hs[:, c0:c0 + csz], pss[:, 0:csz],
                                             AF.Identity, bias=bs_t[l][:], scale=1.0)
                    else:
                        nc.scalar.activation(hs[:, c0:c0 + csz], pss[:, 0:csz],
                                             AF.Copy)
                    for s in range(csz // 128):
                        psk_full = pnode.tile([128, 512], F32, tag="pn")
                        psk = psk_full[:, 0:2 * H]
                        nc.tensor.matmul(out=psk[:, 0:H],
                                         lhsT=rhs_ap[:, s * 128:(s + 1) * 128],
                                         rhs=W_t["k", l][:], start=True, stop=True)
                        gi = c0 // 128 + s
                        if meta["has_bias_k"]:
                            nc.vector.tensor_tensor(out=k_sb[:, gi, :],
                                                    in0=psk[:, 0:H],
                                                    in1=bias_k_t[l][:], op=OP.add)
                        else:
                            nc.scalar.activation(k_sb[:, gi, :], psk[:, 0:H],
                                                 AF.Copy)

                lo_runs = make_runs(qv_blocks_lo)
                pos = [0]
                _l, _hsrc = l, hsrc

                def filler(g, _l=l, _hsrc=hsrc, pos=pos, lo_runs=lo_runs):
                    tgt = (g + 1) * len(lo_runs) // NGRP
                    while pos[0] < min(tgt, len(lo_runs)):
                        rb0, r0, nrun = lo_runs[pos[0]]
                        emit_run(rb0, r0, nrun, _l, _hsrc)
                        pos[0] += 1

                if parts >= 2:
                    emit_edges(1, l, filler)
                while pos[0] < len(lo_runs):
                    rb0, r0, nrun = lo_runs[pos[0]]
                    emit_run(rb0, r0, nrun, l, hsrc)
                    pos[0] += 1
                if parts >= 2:
                    emit_edges(0, l)

                # ---- h update: leaky + BN
                if parts < 3:
                    break
                LCH = 896
                for lc0 in range(0, NPAD, LCH):
                    lsz = min(LCH, NPAD - lc0)
                    sl = slice(lc0, lc0 + lsz)
                    tmp = stg.tile([128, LCH], F32, tag="lrelu")
                    nc.vector.tensor_scalar_mul(tmp[:, 0:lsz], hs[:, sl], SLOPE)
                    nc.vector.tensor_tensor(out=hs[:, sl], in0=hs[:, sl],
                                            in1=tmp[:, 0:lsz], op=OP.max)
                if l < 2:
                    h3 = h3p.tile([128, NPAD], DT)
                    nc.scalar.activation(h3[:], hs[:], AF.Identity,
                                         bias=B_t[l][:], scale=A_t[l][:])
                    nc.sync.dma_start(h_loc[l][:], h3[:])
                    nc.gpsimd.collective_compute(
                        "AllGather", OP.bypass,
                        replica_groups=[list(range(R))],
                        ins=[h_loc[l][:]], outs=[ag_out[l][:]])
                    h3_prev = h3
                else:
                    h3f = hsb.tile([128, NPAD], DT)
                    nc.scalar.activation(h3f[:], hs[:], AF.Identity,
                                         bias=B_t[l][:], scale=A_t[l][:])

            if parts < 4:
                z_dbg = stg.tile([G, NCLS], F32, tag="zsb")
                nc.vector.tensor_copy(z_dbg[:], hs[0:G, 0:NCLS])
                nc.sync.dma_start(t_out[:], z_dbg[:])
            else:
                # ---- pooling
                pp = ppool.tile([G, H], F32)
                for c in range(NGRP):
                    trp = pedge.tile([128, 128], DT, tag="pa")
                    nc.tensor.transpose(out=trp[:], in_=h3f[:, c * 128:(c + 1) * 128],
                                        identity=ident[:])
                    hnode = stg.tile([128, 128], DT, tag="hnode")
                    nc.vector.tensor_copy(hnode[:], trp[:])
                    ind_t = stg.tile([128, G], DT, tag="ind")
                    nc.sync.dma_start(ind_t[:], t_IndT[c * 128:(c + 1) * 128, :])
                    nc.tensor.matmul(out=pp[:], lhsT=ind_t[:], rhs=hnode[:],
                                     start=(c == 0), stop=(c == NGRP - 1))
                pool_sb = stg.tile([G, H], F32, tag="poolsb")
                nc.vector.tensor_copy(pool_sb[:], pp[:])
                nc.sync.dma_start(pool_in[:], pool_sb[:])
                nc.gpsimd.collective_compute(
                    "AllGather", OP.bypass, replica_groups=[list(range(R))],
                    ins=[pool_in[:]], outs=[pool_out[:]])
                pr = stg.tile([G, R, H], F32, tag="pr")
                nc.sync.dma_start(pr[:], pool_out[:].rearrange("(r g) h -> g r h", r=R))
                pooled = stg.tile([G, H], F32, tag="pooled")
                nc.vector.tensor_tensor(out=pooled[:], in0=pr[:, 0, :], in1=pr[:, 1, :],
                                        op=OP.add)
                for r in range(2, R):
                    nc.vector.tensor_tensor(out=pooled[:], in0=pooled[:],
                                            in1=pr[:, r, :], op=OP.add)
                ptp = pedge.tile([H, G], F32, tag="pa")
                nc.tensor.transpose(out=ptp[:], in_=pooled[:], identity=identf[0:G, 0:G])
                pooledT = stg.tile([H, G], F32, tag="pooledT")
                nc.vector.tensor_copy(pooledT[:], ptp[:])
                zp = pedge.tile([G, NCLS], F32, tag="pa")
                nc.tensor.matmul(out=zp[:], lhsT=pooledT[:], rhs=Wch_t[:],
                                 start=True, stop=False)
                nc.tensor.matmul(out=zp[:], lhsT=clin_t[:], rhs=Wcc_t[:],
                                 start=False, stop=True)
                z_sb = stg.tile([G, NCLS], F32, tag="zsb")
                if meta["has_bc"]:
                    nc.vector.tensor_tensor(out=z_sb[:], in0=zp[:], in1=bc_t[:],
                                            op=OP.add)
                else:
                    nc.vector.tensor_copy(z_sb[:], zp[:])
                nc.sync.dma_start(t_out[:], z_sb[:])

    nc.compile()
    return nc


# ---------------------------------------------------------------------------

_CACHE = {}


def kernel(**inputs):
    in_maps, meta = prep(inputs)
    key = tuple(sorted((k, v) for k, v in meta.items()))
    if key not in _CACHE:
        _CACHE[key] = build(meta)
    nc = _CACHE[key]
    res = run_bass_kernel_spmd(nc, in_maps, list(range(R)))
    return np.asarray(res.results[0]["out"], np.float32)


def kernel_profiled(**inputs):
    """Like kernel() but also returns (exec_time_ns, trace_path)."""
    in_maps, meta = prep(inputs)
    key = tuple(sorted((k, v) for k, v in meta.items()))
    if key not in _CACHE:
        _CACHE[key] = build(meta)
    nc = _CACHE[key]
    res = run_bass_kernel_spmd(nc, in_maps, list(range(R)), trace=True)
    out = np.asarray(res.results[0]["out"], np.float32)
    trace_path = None
    if res.instructions_and_trace is not None:
        trace_path = res.instructions_and_trace[1]
    return out, res.exec_time_ns, trace_path


if __name__ == "__main__":
    pass



# revision 4
# speedup vs baseline: 1.0417x; 1.0417x over previous
"""Trainium2 Bass kernel for a 3-layer ResGatedGraphConv GNN (ClinicalGatedGCN).

Strategy (8 NeuronCores, SPMD), v3:
  - Nodes partitioned into 8 contiguous ranges of 6250 (padded to 6272 = 49
    groups of 128). Edges assigned to the rank owning their dst node, sorted
    by (src-chunk epoch, dst-group, dst) on the host.
  - The [q|v] node table is built DISTRIBUTED: each rank computes q,v (and k)
    only for its OWN nodes from its SBUF-resident h (one fused matmul per
    128-node group against [Wq|Wv|Wk]), stores the [q|v] rows to local DRAM,
    and AllGathers the table in two chunks (groups 0:24 -> qv_t0, 24:49 ->
    qv_t1; both tables stay under the int16 gather-index limit). This removes
    the per-layer h AllGather, all h re-reads, and ~7/8 of the table matmuls
    and PSUM->SBUF copies of the replicated design.
  - One dma_gather per (epoch, dst-group) fetches the src rows of [q|v].
    k[dst] is expanded on the PE via a host-shipped 0/1 selector ST; the
    segment-sum over dst is a PE matmul against selector S. S and ST are
    packed in ONE fp8 blob per (epoch, group) (0/1 is exact in fp8; matmul
    operands may mix fp8 with bf16), halving selector DMA traffic.
  - The edge-embedding rank-1 term (attr x We + gate bias) is batched: one
    K=5 matmul per 4-tile PSUM chunk against a block-diagonal [5, 4H]
    constant, with per-chunk attr columns shipped once as an fp8 constant.
  - BatchNorm folding: A = gamma/sqrt(var+eps) is folded into the NEXT
    layer's weights (and the classifier); B is folded into effective biases.
    The on-device h update is a single fused leaky-ReLU
    (scalar_tensor_tensor max(x, slope*x)).
  - Mean-pool per graph is a matmul against a host-built indicator with
    1/cnt folded in; partial pools are AllGather'd and summed; the tiny
    classifier runs on every core.
"""

import numpy as np
import ml_dtypes

import concourse.bacc as bacc
import concourse.bass as bass
import concourse.mybir as mybir
import concourse.tile as tile
from concourse.bass_utils import run_bass_kernel_spmd
from concourse.masks import make_identity

F32 = mybir.dt.float32
BF16 = mybir.dt.bfloat16
FP8 = mybir.dt.float8e4
I16 = mybir.dt.int16
AF = mybir.ActivationFunctionType
OP = mybir.AluOpType

# ---------------- problem constants (hardcoded per spec) ----------------
N, E, H, G, NCLIN, NCLS = 50000, 800000, 128, 64, 16, 2
NLAYER = 3
EPS = 1e-5
SLOPE = 0.01
R = 8                      # ranks / NeuronCores

NPR = (N + R - 1) // R     # real nodes per rank (6250)
NGRP = (NPR + 127) // 128  # 128-node groups per rank (49)
NPAD = NGRP * 128          # padded nodes per rank (6272)
C0G = 24                   # groups in AG chunk 0
C1G = NGRP - C0G           # groups in AG chunk 1 (25)
C0N = C0G * 128            # 3072 rows
C1N = C1G * 128            # 3200 rows
KQC = 4                    # psum chunk size in edge tiles (1 PSUM bank)


def wrap_idxs_block(idx):
    """Wrap one gather call's indices: idx j -> [j%16, j//16], tiled to 128 parts."""
    n = len(idx)
    assert n % 16 == 0
    w = np.asarray(idx, np.int16).reshape(n // 16, 16).T
    return np.tile(w, (8, 1))


# ---------------------------------------------------------------------------
# host-side preprocessing
# ---------------------------------------------------------------------------

def prep(inputs):
    x = np.asarray(inputs["x"], np.float32)
    edge_index = np.asarray(inputs["edge_index"])
    edge_attr = np.asarray(inputs["edge_attr"], np.float32)[:, 0]
    batch = np.asarray(inputs["batch"]).astype(np.int64)
    clinical = np.asarray(inputs["clinical"], np.float32)
    Wk, bk = np.asarray(inputs["Wk"], np.float32), np.asarray(inputs["bk"], np.float32)
    Wq, bq = np.asarray(inputs["Wq"], np.float32), np.asarray(inputs["bq"], np.float32)
    Wv, bv = np.asarray(inputs["Wv"], np.float32), np.asarray(inputs["bv"], np.float32)
    Ws, bs = np.asarray(inputs["Ws"], np.float32), np.asarray(inputs["bs"], np.float32)
    We, be = np.asarray(inputs["We"], np.float32), np.asarray(inputs["be"], np.float32)
    gamma = np.asarray(inputs["gamma"], np.float32)
    beta = np.asarray(inputs["beta"], np.float32)
    rmean = np.asarray(inputs["rmean"], np.float32)
    rvar = np.asarray(inputs["rvar"], np.float32)
    Wc, bc = np.asarray(inputs["Wc"], np.float32), np.asarray(inputs["bc"], np.float32)

    src = edge_index[0].astype(np.int64)
    dst = edge_index[1].astype(np.int64)

    # BN folded: true h_out = A*leaky(pre) + B; device h3 = leaky(pre).
    A = gamma / np.sqrt(rvar + EPS)          # [3, H]
    B = beta - rmean * A                     # [3, H]

    # effective weights: fold diag(A[l-1]) into layer-l input maps, and the
    # B[l-1] offset into layer-l biases.
    Wq_e = np.stack([Wq[l] * (A[l - 1][:, None] if l else 1.0) for l in range(NLAYER)])
    Wk_e = np.stack([Wk[l] * (A[l - 1][:, None] if l else 1.0) for l in range(NLAYER)])
    Wv_e = np.stack([Wv[l] * (A[l - 1][:, None] if l else 1.0) for l in range(NLAYER)])
    Ws_e = np.stack([Ws[l] * (A[l - 1][:, None] if l else 1.0) for l in range(NLAYER)])
    bgate = np.stack([bk[l] + bq[l] + be[l]
                      + (B[l - 1] @ (Wk[l] + Wq[l]) if l else 0.0)
                      for l in range(NLAYER)])
    bv_e = np.stack([bv[l] + (B[l - 1] @ Wv[l] if l else 0.0) for l in range(NLAYER)])
    bs_e = np.stack([bs[l] + (B[l - 1] @ Ws[l] if l else 0.0) for l in range(NLAYER)])
    Wc_h = Wc[0:H] * A[2][:, None]
    bc_e = bc + B[2] @ Wc[0:H]

    Wqvk = np.concatenate([Wq_e, Wv_e, Wk_e], axis=2)     # [3, H, 3H]

    # block-diagonal edge-term constant: rows 0..3 carry We on diag block,
    # row 4 carries the gate bias (broadcast to each tile block).
    BD = np.zeros((NLAYER, 5, KQC * H), np.float32)
    for l in range(NLAYER):
        for t in range(KQC):
            BD[l, t, t * H:(t + 1) * H] = We[l, 0, :]
            BD[l, 4, t * H:(t + 1) * H] = bgate[l]

    has_bv = bool(np.any(bv_e != 0))
    has_bs = bool(np.any(bs_e != 0))

    # ---- edge structure ----
    e_rank = dst // NPR
    dst_local = dst - e_rank * NPR
    grp = dst_local // 128
    drel = dst_local % 128
    rs = src // NPR
    lr = src - rs * NPR
    ep = (lr >= C0N).astype(np.int64)
    trow = np.where(ep == 0, rs * C0N + lr, rs * C1N + (lr - C0N))

    counts = np.zeros((2, R, NGRP), np.int64)
    np.add.at(counts, (ep, e_rank, grp), 1)
    nt_l = [np.ceil(counts[e].max(axis=0) / 128).astype(int) for e in (0, 1)]
    off_l = [np.concatenate([[0], np.cumsum(nt)]).astype(int) for nt in nt_l]
    nch_l = [np.ceil(nt / KQC).astype(int) for nt in nt_l]
    choff_l = [np.concatenate([[0], np.cumsum(nc)]).astype(int) for nc in nch_l]

    cntg = np.bincount(batch, minlength=G).astype(np.float32)
    inv_cnt = 1.0 / np.maximum(cntg, 1.0)

    order = np.lexsort((dst, grp, ep, e_rank))
    trow_s, drel_s, attr_s = trow[order], drel[order], edge_attr[order]
    key = (e_rank[order] * 2 + ep[order]) * NGRP + grp[order]
    starts = np.searchsorted(key, np.arange(R * 2 * NGRP + 1))

    in_maps = []
    for r in range(R):
        ep_arrs = {}
        for e in (0, 1):
            nt = nt_l[e]
            off = off_l[e]
            nch = nch_l[e]
            choff = choff_l[e]
            tot = int(off[-1])
            chtot = int(choff[-1])
            gidx = np.zeros((128, tot * 8), np.int16)
            blob = np.zeros((128, tot * 256), ml_dtypes.float8_e4m3)
            attr4 = np.zeros((5, chtot * 128), ml_dtypes.float8_e4m3)
            attr4[4, :] = 1.0
            for g in range(NGRP):
                ntg = int(nt[g])
                if ntg == 0:
                    continue
                k = (r * 2 + e) * NGRP + g
                s0 = int(starts[k])
                n = int(counts[e, r, g])
                o = int(off[g])
                co = int(choff[g])
                idx = np.zeros((ntg * 128,), np.int64)
                idx[:n] = trow_s[s0:s0 + n]
                gidx[:, o * 8:(o + ntg) * 8] = wrap_idxs_block(idx)
                j = np.arange(n)
                t = j // 128
                p = j % 128
                dr = drel_s[s0:s0 + n].astype(np.int64)
                # S: [p_edge, d] at cols (o+t)*256 + d
                blob[p, (o + t) * 256 + dr] = 1
                # ST: [drel, p_edge] at cols (o+t)*256 + 128 + p
                blob[dr, (o + t) * 256 + 128 + p] = 1
                attr4[t % KQC, (co + t // KQC) * 128 + p] = \
                    attr_s[s0:s0 + n].astype(ml_dtypes.float8_e4m3)
            ep_arrs[e] = (gidx, blob, attr4)
        IndT = np.zeros((NPAD, G), np.float32)
        lo, hi = r * NPR, min((r + 1) * NPR, N)
        IndT[np.arange(hi - lo), batch[lo:hi]] = inv_cnt[batch[lo:hi]]
        xT_loc = np.zeros((128, NPAD), np.float32)
        xT_loc[:, 0:hi - lo] = x[lo:hi].T
        im = {
            "xT_loc": xT_loc.astype(ml_dtypes.bfloat16),
            "Wqvk": Wqvk.astype(ml_dtypes.bfloat16),
            "Ws": Ws_e.astype(ml_dtypes.bfloat16),
            "BD": BD.astype(ml_dtypes.bfloat16),
            "bv_row": np.concatenate(
                [np.zeros((NLAYER, 1, H), np.float32),
                 bv_e.reshape(NLAYER, 1, H),
                 np.zeros((NLAYER, 1, H), np.float32)], axis=2
            ).astype(ml_dtypes.bfloat16),
            "bs_col": bs_e.reshape(NLAYER, H, 1),
            "gidx0": ep_arrs[0][0], "blob0": ep_arrs[0][1], "attr0": ep_arrs[0][2],
            "gidx1": ep_arrs[1][0], "blob1": ep_arrs[1][1], "attr1": ep_arrs[1][2],
            "IndT": IndT.astype(ml_dtypes.bfloat16),
            "clinT": clinical.T.copy(),
            "Wc_h": Wc_h, "Wc_c": Wc[H:H + NCLIN],
            "bc_rep": np.tile(bc_e, (G, 1)),
        }
        in_maps.append(im)
    meta = dict(NT0=tuple(int(v) for v in nt_l[0]),
                NT1=tuple(int(v) for v in nt_l[1]),
                has_bv=has_bv, has_bs=has_bs)
    return in_maps, meta


# ---------------------------------------------------------------------------
# device program
# ---------------------------------------------------------------------------

def build(meta):
    parts = meta.get("parts", 4)
    DT = BF16
    NT_L = [list(meta["NT0"]), list(meta["NT1"])]
    OFF_L = [np.concatenate([[0], np.cumsum(nt)]).astype(int) for nt in NT_L]
    NCH_L = [np.ceil(np.array(nt) / KQC).astype(int) for nt in NT_L]
    CHOFF_L = [np.concatenate([[0], np.cumsum(nc)]).astype(int) for nc in NCH_L]
    TOT = [int(o[-1]) for o in OFF_L]
    CHTOT = [int(c[-1]) for c in CHOFF_L]
    TMAX = max(max(NT_L[0]), max(NT_L[1]))

    nc = bacc.Bacc("TRN2", target_bir_lowering=False, debug=False, num_devices=R)

    def din(name, shape, dt):
        return nc.dram_tensor(name, shape, dt, kind="ExternalInput").ap()

    t_xT = din("xT_loc", [128, NPAD], DT)
    t_Wqvk = din("Wqvk", [NLAYER, H, 3 * H], DT)
    t_Ws = din("Ws", [NLAYER, H, H], DT)
    t_BD = din("BD", [NLAYER, 5, KQC * H], DT)
    t_bv = din("bv_row", [NLAYER, 1, 3 * H], DT)
    t_bs = din("bs_col", [NLAYER, H, 1], F32)
    t_gidx = [din("gidx0", [128, TOT[0] * 8], I16),
              din("gidx1", [128, TOT[1] * 8], I16)]
    t_blob = [din("blob0", [128, TOT[0] * 256], FP8),
              din("blob1", [128, TOT[1] * 256], FP8)]
    t_attr = [din("attr0", [5, CHTOT[0] * 128], FP8),
              din("attr1", [5, CHTOT[1] * 128], FP8)]
    t_IndT = din("IndT", [NPAD, G], DT)
    t_clinT = din("clinT", [NCLIN, G], F32)
    t_Wc_h = din("Wc_h", [H, NCLS], F32)
    t_Wc_c = din("Wc_c", [NCLIN, NCLS], F32)
    t_bc = din("bc_rep", [G, NCLS], F32)

    t_out = nc.dram_tensor("out", [G, NCLS], F32, kind="ExternalOutput").ap()

    qv_loc = nc.dram_tensor("qv_loc", [NPAD, 2 * H], DT).ap()
    qv_t = [nc.dram_tensor("qv_t0", [R * C0N, 2 * H], DT, addr_space="Shared").ap(),
            nc.dram_tensor("qv_t1", [R * C1N, 2 * H], DT, addr_space="Shared").ap()]
    pool_in = nc.dram_tensor("pool_in", [G, H], F32).ap()
    pool_out = nc.dram_tensor("pool_out", [R * G, H], F32, addr_space="Shared").ap()

    with tile.TileContext(nc) as tc:
        import contextlib
        with contextlib.ExitStack() as ctx:
            consts = ctx.enter_context(tc.tile_pool(name="consts", bufs=1))
            hsb = ctx.enter_context(tc.tile_pool(name="hsb", bufs=1))
            h3p = ctx.enter_context(tc.tile_pool(name="h3p", bufs=1))
            ksb = ctx.enter_context(tc.tile_pool(name="ksb", bufs=1))
            stg = ctx.enter_context(tc.tile_pool(name="stg", bufs=4))
            edg = ctx.enter_context(tc.tile_pool(name="edg", bufs=3))
            sel = ctx.enter_context(tc.tile_pool(name="sel", bufs=3))
            edm = ctx.enter_context(tc.tile_pool(name="edm", bufs=3))
            pnode = ctx.enter_context(tc.tile_pool(name="pnode", bufs=2, space="PSUM"))
            pkq = ctx.enter_context(tc.tile_pool(name="pkq", bufs=2, space="PSUM"))
            pedge = ctx.enter_context(tc.tile_pool(name="pedge", bufs=2, space="PSUM"))
            ppool = ctx.enter_context(tc.tile_pool(name="ppool", bufs=1, space="PSUM"))

            _cid = [0]

            def load_const(src_ap, shape, dt):
                _cid[0] += 1
                t = consts.tile(shape, dt, tag=f"c{_cid[0]}_{src_ap.tensor.name}")
                nc.sync.dma_start(t[:], src_ap)
                return t

            Wqvk_t = [load_const(t_Wqvk[l], [H, 3 * H], DT) for l in range(NLAYER)]
            Ws_t = [load_const(t_Ws[l], [H, H], DT) for l in range(NLAYER)]
            BD_t = [load_const(t_BD[l], [5, KQC * H], DT) for l in range(NLAYER)]
            bv_t = ([load_const(t_bv[l], [1, 3 * H], DT) for l in range(NLAYER)]
                    if meta["has_bv"] else None)
            bs_t = ([load_const(t_bs[l], [H, 1], F32) for l in range(NLAYER)]
                    if meta["has_bs"] else None)
            ones_t = consts.tile([1, 128], DT)
            nc.vector.memset(ones_t[:], 1.0)
            gidx_t = [load_const(t_gidx[0], [128, TOT[0] * 8], I16),
                      load_const(t_gidx[1], [128, TOT[1] * 8], I16)]
            attr_t = [load_const(t_attr[0], [5, CHTOT[0] * 128], FP8),
                      load_const(t_attr[1], [5, CHTOT[1] * 128], FP8)]
            xT_t = load_const(t_xT, [128, NPAD], DT)
            ident = consts.tile([128, 128], DT)
            make_identity(nc, ident[:])
            identf = consts.tile([128, 128], F32)
            make_identity(nc, identf[:])
            clin_t = load_const(t_clinT, [NCLIN, G], F32)
            Wch_t = load_const(t_Wc_h, [H, NCLS], F32)
            Wcc_t = load_const(t_Wc_c, [NCLIN, NCLS], F32)
            bc_t = load_const(t_bc, [G, NCLS], F32)

            hs = hsb.tile([128, NPAD], DT, tag="hs")   # s + agg accumulator
            k_sb = ksb.tile([128, NGRP, H], DT)        # local k table (node-major)
            h3a = h3p.tile([128, NPAD], DT, tag="h3a")
            h3b = h3p.tile([128, NPAD], DT, tag="h3b")
            h3f = h3a                                  # l=2 output reuses h3a

            def emit_edges(e, l):
                nt_list = NT_L[e]
                off = OFF_L[e]
                choff = CHOFF_L[e]
                for g in range(NGRP):
                    nt = int(nt_list[g])
                    if nt == 0:
                        continue
                    ne = nt * 128
                    o = int(off[g])
                    co = int(choff[g])
                    bt = sel.tile([128, TMAX, 256], FP8, tag="blob")
                    nc.sync.dma_start(
                        bt[:, 0:nt, :],
                        t_blob[e][:, o * 256:(o + nt) * 256].rearrange(
                            "p (t d) -> p t d", t=nt))
                    gt = edg.tile([128, TMAX, 2 * H], DT, tag="g")
                    nc.gpsimd.dma_gather(
                        gt[:, 0:nt, :], qv_t[e][:],
                        gidx_t[e][:, o * 8:(o + nt) * 8],
                        ne, ne, 2 * H, single_packet=(ne <= 512))
                    ktb = edm.tile([128, TMAX, 128], DT, tag="kt")
                    for ci in range((nt + KQC - 1) // KQC):
                        q0 = ci * KQC
                        qn = min(KQC, nt - q0)
                        pk = pkq.tile([128, KQC, 128], F32, tag="kq")
                        nc.tensor.matmul(
                            out=pk[:, 0:KQC, :],
                            lhsT=attr_t[e][0:5, (co + ci) * 128:(co + ci + 1) * 128],
                            rhs=BD_t[l][:], start=True, stop=False,
                            skip_group_check=True)
                        for t in range(q0, q0 + qn):
                            nc.tensor.matmul(
                                out=pk[:, t - q0, :],
                                lhsT=bt[:, t, 128:256],
                                rhs=k_sb[:, g, :], start=False,
                                stop=(t == q0 + qn - 1),
                                skip_group_check=True)
                        nc.vector.tensor_tensor(
                            out=ktb[:, q0:q0 + qn, :], in0=pk[:, 0:qn, :],
                            in1=gt[:, q0:q0 + qn, 0:H], op=OP.add)
                    nc.scalar.activation(ktb[:, 0:nt, :], ktb[:, 0:nt, :],
                                         AF.Sigmoid)
                    msg = edm.tile([128, TMAX, 128], DT, tag="msg")
                    nc.vector.tensor_tensor(out=msg[:, 0:nt, :],
                                            in0=ktb[:, 0:nt, :],
                                            in1=gt[:, 0:nt, H:2 * H], op=OP.mult)
                    pa = pedge.tile([128, 128], F32, tag="pa")
                    for t in range(nt):
                        nc.tensor.matmul(out=pa[:], lhsT=msg[:, t, :],
                                         rhs=bt[:, t, 0:128], start=(t == 0),
                                         stop=(t == nt - 1))
                    nc.vector.tensor_tensor(
                        out=hs[:, g * 128:(g + 1) * 128],
                        in0=hs[:, g * 128:(g + 1) * 128], in1=pa[:], op=OP.add)

            for l in range(NLAYER):
                hsrc = xT_t if l == 0 else (h3a if l == 1 else h3b)

                # ---- local qvk build + chunked table AllGather
                g = 0
                while g < NGRP:
                    ns = min(4, NGRP - g, (C0G - g) if g < C0G else (NGRP - g))
                    st = stg.tile([128, 4, 2 * H], DT, tag="st")
                    for s in range(ns):
                        ps = pnode.tile([128, 512], F32, tag="pn")
                        nc.tensor.matmul(
                            out=ps[:, 0:3 * H],
                            lhsT=hsrc[:, (g + s) * 128:(g + s + 1) * 128],
                            rhs=Wqvk_t[l][:], start=True,
                            stop=not meta["has_bv"],
                            skip_group_check=True)
                        if meta["has_bv"]:
                            nc.tensor.matmul(
                                out=ps[:, 0:3 * H], lhsT=ones_t[:],
                                rhs=bv_t[l][:], start=False, stop=True,
                                skip_group_check=True)
                        nc.scalar.activation(st[:, s, :], ps[:, 0:2 * H], AF.Copy)
                        nc.vector.tensor_copy(k_sb[:, g + s, :],
                                              ps[:, 2 * H:3 * H])
                    nc.sync.dma_start(
                        qv_loc[g * 128:(g + ns) * 128, :].rearrange(
                            "(s p) f -> p s f", p=128),
                        st[:, 0:ns, :])
                    g += ns
                    if g == C0G:
                        nc.gpsimd.collective_compute(
                            "AllGather", OP.bypass,
                            replica_groups=[list(range(R))],
                            ins=[qv_loc[0:C0N, :]], outs=[qv_t[0][:]])
                nc.gpsimd.collective_compute(
                    "AllGather", OP.bypass,
                    replica_groups=[list(range(R))],
                    ins=[qv_loc[C0N:NPAD, :]], outs=[qv_t[1][:]])

                # ---- s-part (feature-major) into hs
                for c0 in range(0, NPAD, 512):
                    csz = min(512, NPAD - c0)
                    pss = pnode.tile([128, 512], F32, tag="pn")
                    nc.tensor.matmul(out=pss[:, 0:csz], lhsT=Ws_t[l][:],
                                     rhs=hsrc[:, c0:c0 + csz],
                                     start=True, stop=True)
                    if meta["has_bs"]:
                        nc.scalar.activation(hs[:, c0:c0 + csz], pss[:, 0:csz],
                                             AF.Identity, bias=bs_t[l][:],
                                             scale=1.0)
                    else:
                        nc.scalar.activation(hs[:, c0:c0 + csz], pss[:, 0:csz],
                                             AF.Copy)

                # ---- edges: epoch 0 (chunk-0 table) then epoch 1
                if parts >= 2:
                    emit_edges(0, l)
                    emit_edges(1, l)

                # ---- h update: fused leaky relu (BN folded into weights)
                if parts < 3:
                    break
                h3_new = h3f if l == 2 else (h3a if l == 0 else h3b)
                nc.vector.scalar_tensor_tensor(
                    out=h3_new[:], in0=hs[:], scalar=float(SLOPE),
                    in1=hs[:], op0=OP.mult, op1=OP.max)

            if parts < 4:
                z_dbg = stg.tile([G, NCLS], F32, tag="zsb")
                nc.vector.tensor_copy(z_dbg[:], hs[0:G, 0:NCLS])
                nc.sync.dma_start(t_out[:], z_dbg[:])
            else:
                # ---- pooling
                pp = ppool.tile([G, H], F32)
                for c in range(NGRP):
                    trp = pedge.tile([128, 128], DT, tag="pa")
                    nc.tensor.transpose(out=trp[:], in_=h3f[:, c * 128:(c + 1) * 128],
                                        identity=ident[:])
                    hnode = stg.tile([128, 128], DT, tag="hnode")
                    nc.vector.tensor_copy(hnode[:], trp[:])
                    ind_t = stg.tile([128, G], DT, tag="ind")
                    nc.sync.dma_start(ind_t[:], t_IndT[c * 128:(c + 1) * 128, :])
                    nc.tensor.matmul(out=pp[:], lhsT=ind_t[:], rhs=hnode[:],
                                     start=(c == 0), stop=(c == NGRP - 1))
                pool_sb = stg.tile([G, H], F32, tag="poolsb")
                nc.vector.tensor_copy(pool_sb[:], pp[:])
                nc.sync.dma_start(pool_in[:], pool_sb[:])
                nc.gpsimd.collective_compute(
                    "AllGather", OP.bypass, replica_groups=[list(range(R))],
                    ins=[pool_in[:]], outs=[pool_out[:]])
                pr = stg.tile([G, R, H], F32, tag="pr")
                nc.sync.dma_start(pr[:], pool_out[:].rearrange("(r g) h -> g r h", r=R))
                pooled = stg.tile([G, H], F32, tag="pooled")
                nc.vector.tensor_tensor(out=pooled[:], in0=pr[:, 0, :], in1=pr[:, 1, :],
                                        op=OP.add)
                for r in range(2, R):
                    nc.vector.tensor_tensor(out=pooled[:], in0=pooled[:],
                                            in1=pr[:, r, :], op=OP.add)
                ptp = pedge.tile([H, G], F32, tag="pa")
                nc.tensor.transpose(out=ptp[:], in_=pooled[:], identity=identf[0:G, 0:G])
                pooledT = stg.tile([H, G], F32, tag="pooledT")
                nc.vector.tensor_copy(pooledT[:], ptp[:])
                zp = pedge.tile([G, NCLS], F32, tag="pa")
                nc.tensor.matmul(out=zp[:], lhsT=pooledT[:], rhs=Wch_t[:],
                                 start=True, stop=False)
                nc.tensor.matmul(out=zp[:], lhsT=clin_t[:], rhs=Wcc_t[:],
                                 start=False, stop=True)
                z_sb = stg.tile([G, NCLS], F32, tag="zsb")
                nc.vector.tensor_tensor(out=z_sb[:], in0=zp[:], in1=bc_t[:],
                                        op=OP.add)
                nc.sync.dma_start(t_out[:], z_sb[:])

    nc.compile()
    return nc


# ---------------------------------------------------------------------------

_CACHE = {}


def kernel(**inputs):
    in_maps, meta = prep(inputs)
    key = tuple(sorted((k, v) for k, v in meta.items()))
    if key not in _CACHE:
        _CACHE[key] = build(meta)
    nc = _CACHE[key]
    res = run_bass_kernel_spmd(nc, in_maps, list(range(R)))
    return np.asarray(res.results[0]["out"], np.float32)


def kernel_profiled(**inputs):
    """Like kernel() but also returns (exec_time_ns, trace_path)."""
    in_maps, meta = prep(inputs)
    key = tuple(sorted((k, v) for k, v in meta.items()))
    if key not in _CACHE:
        _CACHE[key] = build(meta)
    nc = _CACHE[key]
    res = run_bass_kernel_spmd(nc, in_maps, list(range(R)), trace=True)
    out = np.asarray(res.results[0]["out"], np.float32)
    trace_path = None
    if res.instructions_and_trace is not None:
        trace_path = res.instructions_and_trace[1]
    return out, res.exec_time_ns, trace_path


if __name__ == "__main__":
    pass


# revision 31
# speedup vs baseline: 1.5902x; 1.5265x over previous
"""Trainium2 Bass kernel for a 3-layer ResGatedGraphConv GNN (ClinicalGatedGCN).

Strategy (8 NeuronCores, SPMD), v3:
  - Nodes partitioned into 8 contiguous ranges of 6250 (padded to 6272 = 49
    groups of 128). Edges assigned to the rank owning their dst node, sorted
    by (src-chunk epoch, dst-group, dst) on the host.
  - The [q|v] node table is built DISTRIBUTED: each rank computes q,v (and k)
    only for its OWN nodes from its SBUF-resident h (one fused matmul per
    128-node group against [Wq|Wv|Wk]), stores the [q|v] rows to local DRAM,
    and AllGathers the table in two chunks (groups 0:24 -> qv_t0, 24:49 ->
    qv_t1; both tables stay under the int16 gather-index limit). This removes
    the per-layer h AllGather, all h re-reads, and ~7/8 of the table matmuls
    and PSUM->SBUF copies of the replicated design.
  - One dma_gather per (epoch, dst-group) fetches the src rows of [q|v].
    k[dst] is expanded on the PE via a host-shipped 0/1 selector ST; the
    segment-sum over dst is a PE matmul against selector S. S and ST are
    packed in ONE fp8 blob per (epoch, group) (0/1 is exact in fp8; matmul
    operands may mix fp8 with bf16), halving selector DMA traffic.
  - The edge-embedding rank-1 term (attr x We + gate bias) is batched: one
    K=5 matmul per 4-tile PSUM chunk against a block-diagonal [5, 4H]
    constant, with per-chunk attr columns shipped once as an fp8 constant.
  - BatchNorm folding: A = gamma/sqrt(var+eps) is folded into the NEXT
    layer's weights (and the classifier); B is folded into effective biases.
    The on-device h update is a single fused leaky-ReLU
    (scalar_tensor_tensor max(x, slope*x)).
  - Mean-pool per graph is a matmul against a host-built indicator with
    1/cnt folded in; partial pools are AllGather'd and summed; the tiny
    classifier runs on every core.
"""

import numpy as np
import ml_dtypes

import concourse.bacc as bacc
import concourse.bass as bass
import concourse.mybir as mybir
import concourse.tile as tile
from concourse.bass_utils import run_bass_kernel_spmd
from concourse.masks import make_identity

F32 = mybir.dt.float32
BF16 = mybir.dt.bfloat16
FP8 = mybir.dt.float8e4
I16 = mybir.dt.int16
AF = mybir.ActivationFunctionType
OP = mybir.AluOpType

# ---------------- problem constants (hardcoded per spec) ----------------
N, E, H, G, NCLIN, NCLS = 50000, 800000, 128, 64, 16, 2
NLAYER = 3
EPS = 1e-5
SLOPE = 0.01
R = 8                      # ranks / NeuronCores

NPR = (N + R - 1) // R     # real nodes per rank (6250)
NGRP = (NPR + 127) // 128  # 128-node groups per rank (49)
NPAD = NGRP * 128          # padded nodes per rank (6272)
C0G = 24                   # groups in AG chunk 0
C1G = NGRP - C0G           # groups in AG chunk 1 (25)
C0N = C0G * 128            # 3072 rows
C1N = C1G * 128            # 3200 rows
KQC = 4                    # psum chunk size in edge tiles (1 PSUM bank)


def wrap_idxs_block(idx):
    """Wrap one gather call's indices: idx j -> [j%16, j//16], tiled to 128 parts."""
    n = len(idx)
    assert n % 16 == 0
    w = np.asarray(idx, np.int16).reshape(n // 16, 16).T
    return np.tile(w, (8, 1))


# ---------------------------------------------------------------------------
# host-side preprocessing
# ---------------------------------------------------------------------------

def prep(inputs):
    x = np.asarray(inputs["x"], np.float32)
    edge_index = np.asarray(inputs["edge_index"])
    edge_attr = np.asarray(inputs["edge_attr"], np.float32)[:, 0]
    batch = np.asarray(inputs["batch"]).astype(np.int64)
    clinical = np.asarray(inputs["clinical"], np.float32)
    Wk, bk = np.asarray(inputs["Wk"], np.float32), np.asarray(inputs["bk"], np.float32)
    Wq, bq = np.asarray(inputs["Wq"], np.float32), np.asarray(inputs["bq"], np.float32)
    Wv, bv = np.asarray(inputs["Wv"], np.float32), np.asarray(inputs["bv"], np.float32)
    Ws, bs = np.asarray(inputs["Ws"], np.float32), np.asarray(inputs["bs"], np.float32)
    We, be = np.asarray(inputs["We"], np.float32), np.asarray(inputs["be"], np.float32)
    gamma = np.asarray(inputs["gamma"], np.float32)
    beta = np.asarray(inputs["beta"], np.float32)
    rmean = np.asarray(inputs["rmean"], np.float32)
    rvar = np.asarray(inputs["rvar"], np.float32)
    Wc, bc = np.asarray(inputs["Wc"], np.float32), np.asarray(inputs["bc"], np.float32)

    src = edge_index[0].astype(np.int64)
    dst = edge_index[1].astype(np.int64)

    # BN folded: true h_out = A*leaky(pre) + B; device h3 = leaky(pre).
    A = gamma / np.sqrt(rvar + EPS)          # [3, H]
    B = beta - rmean * A                     # [3, H]

    # effective weights: fold diag(A[l-1]) into layer-l input maps, and the
    # B[l-1] offset into layer-l biases.
    Wq_e = np.stack([Wq[l] * (A[l - 1][:, None] if l else 1.0) for l in range(NLAYER)])
    Wk_e = np.stack([Wk[l] * (A[l - 1][:, None] if l else 1.0) for l in range(NLAYER)])
    Wv_e = np.stack([Wv[l] * (A[l - 1][:, None] if l else 1.0) for l in range(NLAYER)])
    Ws_e = np.stack([Ws[l] * (A[l - 1][:, None] if l else 1.0) for l in range(NLAYER)])
    bgate = np.stack([bk[l] + bq[l] + be[l]
                      + (B[l - 1] @ (Wk[l] + Wq[l]) if l else 0.0)
                      for l in range(NLAYER)])
    bv_e = np.stack([bv[l] + (B[l - 1] @ Wv[l] if l else 0.0) for l in range(NLAYER)])
    bs_e = np.stack([bs[l] + (B[l - 1] @ Ws[l] if l else 0.0) for l in range(NLAYER)])
    Wc_h = Wc[0:H] * A[2][:, None]
    bc_e = bc + B[2] @ Wc[0:H]

    Wqvk = np.concatenate([Wq_e, Wv_e, Wk_e], axis=2)     # [3, H, 3H]

    # block-diagonal edge-term constant: rows 0..3 carry We on diag block,
    # row 4 carries the gate bias (broadcast to each tile block).
    BD = np.zeros((NLAYER, 5, KQC * H), np.float32)
    for l in range(NLAYER):
        for t in range(KQC):
            BD[l, t, t * H:(t + 1) * H] = We[l, 0, :]
            BD[l, 4, t * H:(t + 1) * H] = bgate[l]

    has_bv = bool(np.any(bv_e != 0))
    has_bs = bool(np.any(bs_e != 0))

    # ---- edge structure ----
    e_rank = dst // NPR
    dst_local = dst - e_rank * NPR
    grp = dst_local // 128
    drel = dst_local % 128
    rs = src // NPR
    lr = src - rs * NPR
    ep = (lr >= C0N).astype(np.int64)
    trow = np.where(ep == 0, rs * C0N + lr, rs * C1N + (lr - C0N))

    counts = np.zeros((2, R, NGRP), np.int64)
    np.add.at(counts, (ep, e_rank, grp), 1)
    nt_l = [np.ceil(counts[e].max(axis=0) / 128).astype(int) for e in (0, 1)]
    off_l = [np.concatenate([[0], np.cumsum(nt)]).astype(int) for nt in nt_l]
    nch_l = [np.ceil(nt / KQC).astype(int) for nt in nt_l]
    choff_l = [np.concatenate([[0], np.cumsum(nc)]).astype(int) for nc in nch_l]

    cntg = np.bincount(batch, minlength=G).astype(np.float32)
    inv_cnt = 1.0 / np.maximum(cntg, 1.0)

    order = np.lexsort((dst, grp, ep, e_rank))
    trow_s, drel_s, attr_s = trow[order], drel[order], edge_attr[order]
    key = (e_rank[order] * 2 + ep[order]) * NGRP + grp[order]
    starts = np.searchsorted(key, np.arange(R * 2 * NGRP + 1))

    in_maps = []
    for r in range(R):
        ep_arrs = {}
        for e in (0, 1):
            nt = nt_l[e]
            off = off_l[e]
            nch = nch_l[e]
            choff = choff_l[e]
            tot = int(off[-1])
            chtot = int(choff[-1])
            gidx = np.zeros((128, tot * 8), np.int16)
            blob = np.zeros((128, tot * 256), ml_dtypes.float8_e4m3)
            attr4 = np.zeros((5, chtot * 128), ml_dtypes.float8_e4m3)
            attr4[4, :] = 1.0
            for g in range(NGRP):
                ntg = int(nt[g])
                if ntg == 0:
                    continue
                k = (r * 2 + e) * NGRP + g
                s0 = int(starts[k])
                n = int(counts[e, r, g])
                o = int(off[g])
                co = int(choff[g])
                idx = np.zeros((ntg * 128,), np.int64)
                idx[:n] = trow_s[s0:s0 + n]
                gidx[:, o * 8:(o + ntg) * 8] = wrap_idxs_block(idx)
                j = np.arange(n)
                t = j // 128
                p = j % 128
                dr = drel_s[s0:s0 + n].astype(np.int64)
                # S: [p_edge, d] at cols (o+t)*256 + d
                blob[p, (o + t) * 256 + dr] = 1
                # ST: [drel, p_edge] at cols (o+t)*256 + 128 + p
                blob[dr, (o + t) * 256 + 128 + p] = 1
                attr4[t % KQC, (co + t // KQC) * 128 + p] = \
                    attr_s[s0:s0 + n].astype(ml_dtypes.float8_e4m3)
            ep_arrs[e] = (gidx, blob, attr4)
        IndT = np.zeros((NPAD, G), np.float32)
        lo, hi = r * NPR, min((r + 1) * NPR, N)
        IndT[np.arange(hi - lo), batch[lo:hi]] = inv_cnt[batch[lo:hi]]
        xT_loc = np.zeros((128, NPAD), np.float32)
        xT_loc[:, 0:hi - lo] = x[lo:hi].T
        x_rb = np.zeros((R * 128, NPAD), np.float32)
        for rr in range(R):
            rlo, rhi = rr * NPR, min((rr + 1) * NPR, N)
            x_rb[rr * 128:(rr + 1) * 128, 0:rhi - rlo] = x[rlo:rhi].T
        im = {
            "xT_loc": xT_loc.astype(ml_dtypes.bfloat16),
            "x_rb": x_rb.astype(ml_dtypes.bfloat16),
            "Wqvk": Wqvk.astype(ml_dtypes.bfloat16),
            "Ws": Ws_e.astype(ml_dtypes.bfloat16),
            "BD": BD.astype(ml_dtypes.bfloat16),
            "bv_row": np.concatenate(
                [np.zeros((NLAYER, 1, H), np.float32),
                 bv_e.reshape(NLAYER, 1, H),
                 np.zeros((NLAYER, 1, H), np.float32)], axis=2
            ).astype(ml_dtypes.bfloat16),
            "bs_col": bs_e.reshape(NLAYER, H, 1),
            "gidx0": ep_arrs[0][0], "blob0": ep_arrs[0][1], "attr0": ep_arrs[0][2],
            "gidx1": ep_arrs[1][0], "blob1": ep_arrs[1][1], "attr1": ep_arrs[1][2],
            "IndT": IndT.astype(ml_dtypes.bfloat16),
            "clinT": clinical.T.copy(),
            "Wc_h": Wc_h, "Wc_c": Wc[H:H + NCLIN],
            "bc_rep": np.tile(bc_e, (G, 1)),
        }
        in_maps.append(im)
    meta = dict(NT0=tuple(int(v) for v in nt_l[0]),
                NT1=tuple(int(v) for v in nt_l[1]),
                has_bv=has_bv, has_bs=has_bs, tab_fp8=True, dr_seg=True)
    return in_maps, meta


# ---------------------------------------------------------------------------
# device program
# ---------------------------------------------------------------------------

def build(meta):
    parts = meta.get("parts", 4)
    DT = BF16
    TDT = FP8 if meta.get("tab_fp8", True) else BF16
    NT_L = [list(meta["NT0"]), list(meta["NT1"])]
    OFF_L = [np.concatenate([[0], np.cumsum(nt)]).astype(int) for nt in NT_L]
    NCH_L = [np.ceil(np.array(nt) / KQC).astype(int) for nt in NT_L]
    CHOFF_L = [np.concatenate([[0], np.cumsum(nc)]).astype(int) for nc in NCH_L]
    TOT = [int(o[-1]) for o in OFF_L]
    CHTOT = [int(c[-1]) for c in CHOFF_L]
    TMAX = max(max(NT_L[0]), max(NT_L[1]))

    nc = bacc.Bacc("TRN2", target_bir_lowering=False, debug=False, num_devices=R)

    def din(name, shape, dt):
        return nc.dram_tensor(name, shape, dt, kind="ExternalInput").ap()

    t_xT = din("xT_loc", [128, NPAD], DT)
    t_xrb = din("x_rb", [R * 128, NPAD], DT)
    t_Wqvk = din("Wqvk", [NLAYER, H, 3 * H], DT)
    t_Ws = din("Ws", [NLAYER, H, H], DT)
    t_BD = din("BD", [NLAYER, 5, KQC * H], DT)
    t_bv = din("bv_row", [NLAYER, 1, 3 * H], DT)
    t_bs = din("bs_col", [NLAYER, H, 1], F32)
    t_gidx = [din("gidx0", [128, TOT[0] * 8], I16),
              din("gidx1", [128, TOT[1] * 8], I16)]
    t_blob = [din("blob0", [128, TOT[0] * 256], FP8),
              din("blob1", [128, TOT[1] * 256], FP8)]
    t_attr = [din("attr0", [5, CHTOT[0] * 128], FP8),
              din("attr1", [5, CHTOT[1] * 128], FP8)]
    t_IndT = din("IndT", [NPAD, G], DT)
    t_clinT = din("clinT", [NCLIN, G], F32)
    t_Wc_h = din("Wc_h", [H, NCLS], F32)
    t_Wc_c = din("Wc_c", [NCLIN, NCLS], F32)
    t_bc = din("bc_rep", [G, NCLS], F32)

    t_out = nc.dram_tensor("out", [G, NCLS], F32, kind="ExternalOutput").ap()

    qv_loc = nc.dram_tensor("qv_loc", [NPAD, 2 * H], TDT).ap()
    # per-layer AG'd table pairs (separate per layer so the next layer's
    # AllGather never overwrites a table the current layer still gathers from)
    qv_tl = [None,
             [nc.dram_tensor("qv1_t0", [R * C0N, 2 * H], TDT, addr_space="Shared").ap(),
              nc.dram_tensor("qv1_t1", [R * C1N, 2 * H], TDT, addr_space="Shared").ap()],
             [nc.dram_tensor("qv2_t0", [R * C0N, 2 * H], TDT, addr_space="Shared").ap(),
              nc.dram_tensor("qv2_t1", [R * C1N, 2 * H], TDT, addr_space="Shared").ap()]]
    # layer-0 tables are built locally (x is replicated), no collective
    qv_tl[0] = [nc.dram_tensor("qv0_t0", [R * C0N, 2 * H], TDT).ap(),
                nc.dram_tensor("qv0_t1", [R * C1N, 2 * H], TDT).ap()]
    pool_in = nc.dram_tensor("pool_in", [G, H], F32).ap()
    pool_out = nc.dram_tensor("pool_out", [R * G, H], F32, addr_space="Shared").ap()

    with tile.TileContext(nc) as tc:
        import contextlib
        with contextlib.ExitStack() as ctx:
            consts = ctx.enter_context(tc.tile_pool(name="consts", bufs=1))
            hsb = ctx.enter_context(tc.tile_pool(name="hsb", bufs=1))
            h3p = ctx.enter_context(tc.tile_pool(name="h3p", bufs=1))
            ksb = ctx.enter_context(tc.tile_pool(name="ksb", bufs=1))
            stg = ctx.enter_context(tc.tile_pool(name="stg", bufs=4))
            tail = ctx.enter_context(tc.tile_pool(name="tail", bufs=2))
            edg = ctx.enter_context(tc.tile_pool(name="edg", bufs=4))
            sel = ctx.enter_context(tc.tile_pool(name="sel", bufs=4))
            edm = ctx.enter_context(tc.tile_pool(name="edm", bufs=3))
            pnode = ctx.enter_context(tc.tile_pool(name="pnode", bufs=2, space="PSUM"))
            pkq = ctx.enter_context(tc.tile_pool(name="pkq", bufs=2, space="PSUM"))
            pedge = ctx.enter_context(tc.tile_pool(name="pedge", bufs=2, space="PSUM"))
            ppool = ctx.enter_context(tc.tile_pool(name="ppool", bufs=1, space="PSUM"))

            _cid = [0]

            def load_const(src_ap, shape, dt):
                _cid[0] += 1
                t = consts.tile(shape, dt, tag=f"c{_cid[0]}_{src_ap.tensor.name}")
                nc.sync.dma_start(t[:], src_ap)
                return t

            Wqvk_t = [load_const(t_Wqvk[l], [H, 3 * H], DT) for l in range(NLAYER)]
            Ws_t = [load_const(t_Ws[l], [H, H], DT) for l in range(NLAYER)]
            BD_t = [load_const(t_BD[l], [5, KQC * H], DT) for l in range(NLAYER)]
            bv_t = ([load_const(t_bv[l], [1, 3 * H], DT) for l in range(NLAYER)]
                    if meta["has_bv"] else None)
            bs_t = ([load_const(t_bs[l], [H, 1], F32) for l in range(NLAYER)]
                    if meta["has_bs"] else None)
            ones_t = consts.tile([1, 128], DT)
            nc.vector.memset(ones_t[:], 1.0)
            gidx_t = [load_const(t_gidx[0], [128, TOT[0] * 8], I16),
                      load_const(t_gidx[1], [128, TOT[1] * 8], I16)]
            attr_t = [load_const(t_attr[0], [5, CHTOT[0] * 128], FP8),
                      load_const(t_attr[1], [5, CHTOT[1] * 128], FP8)]
            xT_t = load_const(t_xT, [128, NPAD], DT)
            ident = consts.tile([128, 128], DT)
            make_identity(nc, ident[:])
            identf = consts.tile([128, 128], F32)
            make_identity(nc, identf[:])
            clin_t = load_const(t_clinT, [NCLIN, G], F32)
            Wch_t = load_const(t_Wc_h, [H, NCLS], F32)
            Wcc_t = load_const(t_Wc_c, [NCLIN, NCLS], F32)
            bc_t = load_const(t_bc, [G, NCLS], F32)

            # ping-pong state by layer parity
            hs_pp = [hsb.tile([128, NPAD], DT, tag="hs0", name="hs0"),
                     hsb.tile([128, NPAD], DT, tag="hs1", name="hs1")]
            k_pp = [ksb.tile([128, NGRP, H], DT, tag="k0", name="k0"),
                    ksb.tile([128, NGRP, H], DT, tag="k1", name="k1")]
            h3_pp = [h3p.tile([128, NPAD], DT, tag="h3a", name="h3a"),
                     h3p.tile([128, NPAD], DT, tag="h3b", name="h3b")]
            h3f = h3_pp[0]                             # layer-2 output parity

            def build_chunk(l, c, hsrc):
                """Local q|v|k for groups of AG-chunk c of layer l."""
                g0, g1 = (0, C0G) if c == 0 else (C0G, NGRP)
                k_sb = k_pp[l % 2]
                g = g0
                while g < g1:
                    ns = min(8, g1 - g)
                    st = stg.tile([128, 8, 2 * H], TDT, tag="st")
                    for s in range(ns):
                        gg = g + s
                        ps = pnode.tile([128, 512], F32, tag="pn")
                        nc.tensor.matmul(
                            out=ps[:, 0:3 * H],
                            lhsT=hsrc[:, gg * 128:(gg + 1) * 128],
                            rhs=Wqvk_t[l][:], start=True,
                            stop=not meta["has_bv"],
                            skip_group_check=True)
                        if meta["has_bv"]:
                            nc.tensor.matmul(
                                out=ps[:, 0:3 * H], lhsT=ones_t[:],
                                rhs=bv_t[l][:], start=False, stop=True,
                                skip_group_check=True)
                        nc.scalar.activation(st[:, s, :], ps[:, 0:2 * H],
                                             AF.Copy)
                        nc.scalar.activation(k_sb[:, gg, :],
                                             ps[:, 2 * H:3 * H], AF.Copy)
                    nc.sync.dma_start(
                        qv_loc[g * 128:(g + ns) * 128, :].rearrange(
                            "(s p) f -> p s f", p=128),
                        st[:, 0:ns, :])
                    g += ns

            def emit_ag(l, c):
                ins = qv_loc[0:C0N, :] if c == 0 else qv_loc[C0N:NPAD, :]
                nc.gpsimd.collective_compute(
                    "AllGather", OP.bypass, replica_groups=[list(range(R))],
                    ins=[ins], outs=[qv_tl[l][c][:]])

            def emit_spart(l, hsrc):
                hs = hs_pp[l % 2]
                for c0 in range(0, NPAD, 512):
                    csz = min(512, NPAD - c0)
                    pss = pnode.tile([128, 512], F32, tag="pn")
                    nc.tensor.matmul(out=pss[:, 0:csz], lhsT=Ws_t[l][:],
                                     rhs=hsrc[:, c0:c0 + csz],
                                     start=True, stop=True)
                    if meta["has_bs"]:
                        nc.scalar.activation(hs[:, c0:c0 + csz], pss[:, 0:csz],
                                             AF.Identity, bias=bs_t[l][:],
                                             scale=1.0)
                    else:
                        nc.scalar.activation(hs[:, c0:c0 + csz], pss[:, 0:csz],
                                             AF.Copy)

            def emit_leaky(l, c):
                sl = slice(0, C0N) if c == 0 else slice(C0N, NPAD)
                nc.vector.scalar_tensor_tensor(
                    out=h3_pp[l % 2][:, sl], in0=hs_pp[l % 2][:, sl],
                    scalar=float(SLOPE), in1=hs_pp[l % 2][:, sl],
                    op0=OP.mult, op1=OP.max)

            def emit_edges(e, l, g0, g1):
                nt_list = NT_L[e]
                off = OFF_L[e]
                choff = CHOFF_L[e]
                hs = hs_pp[l % 2]
                k_sb = k_pp[l % 2]
                tab = qv_tl[l][e]
                for g in range(g0, g1):
                    nt = int(nt_list[g])
                    if nt == 0:
                        continue
                    ne = nt * 128
                    o = int(off[g])
                    co = int(choff[g])
                    bt = sel.tile([128, TMAX, 256], FP8, tag="blob")
                    nc.sync.dma_start(
                        bt[:, 0:nt, :],
                        t_blob[e][:, o * 256:(o + nt) * 256].rearrange(
                            "p (t d) -> p t d", t=nt))
                    gt = edg.tile([128, TMAX, 2 * H], TDT, tag="g")
                    nc.gpsimd.dma_gather(
                        gt[:, 0:nt, :], tab[:],
                        gidx_t[e][:, o * 8:(o + nt) * 8],
                        ne, ne, 2 * H, single_packet=(ne <= 512))
                    ktb = edm.tile([128, TMAX, 128], DT, tag="kt")
                    for ci in range((nt + KQC - 1) // KQC):
                        q0 = ci * KQC
                        qn = min(KQC, nt - q0)
                        pk = pkq.tile([128, KQC, 128], F32, tag="kq")
                        nc.tensor.matmul(
                            out=pk[:, 0:KQC, :],
                            lhsT=attr_t[e][0:5, (co + ci) * 128:(co + ci + 1) * 128],
                            rhs=BD_t[l][:], start=True, stop=False,
                            skip_group_check=True)
                        for t in range(q0, q0 + qn):
                            nc.tensor.matmul(
                                out=pk[:, t - q0, :],
                                lhsT=bt[:, t, 128:256],
                                rhs=k_sb[:, g, :], start=False, stop=False,
                                skip_group_check=True)
                        # q[src] folded into the same PSUM via identity matmul
                        nc.tensor.matmul(
                            out=pk[:, 0:qn, :], lhsT=ident[:],
                            rhs=gt[:, q0:q0 + qn, 0:H], start=False, stop=True,
                            skip_group_check=True)
                        nc.scalar.activation(ktb[:, q0:q0 + qn, :],
                                             pk[:, 0:qn, :], AF.Sigmoid)
                    dr = meta.get("dr_seg", False)
                    msg = edm.tile([128, TMAX, 128], FP8 if dr else DT, tag="msg")
                    nc.vector.tensor_tensor(out=msg[:, 0:nt, :],
                                            in0=ktb[:, 0:nt, :],
                                            in1=gt[:, 0:nt, H:2 * H], op=OP.mult)
                    pa = pedge.tile([128, 128], F32, tag="pa")
                    if dr:
                        npair = nt // 2
                        for t2 in range(0, npair * 2, 2):
                            nc.tensor.matmul(
                                out=pa[:], lhsT=msg[:, t2:t2 + 2, :],
                                rhs=bt[:, t2:t2 + 2, 0:128], start=(t2 == 0),
                                stop=(t2 + 2 == nt),
                                perf_mode=mybir.MatmulPerfMode.DoubleRow,
                                skip_group_check=True)
                        if nt % 2:
                            nc.tensor.matmul(out=pa[:], lhsT=msg[:, nt - 1, :],
                                             rhs=bt[:, nt - 1, 0:128],
                                             start=(nt == 1), stop=True,
                                             skip_group_check=True)
                    else:
                        for t in range(nt):
                            nc.tensor.matmul(out=pa[:], lhsT=msg[:, t, :],
                                             rhs=bt[:, t, 0:128], start=(t == 0),
                                             stop=(t == nt - 1))
                    nc.vector.tensor_tensor(
                        out=hs[:, g * 128:(g + 1) * 128],
                        in0=hs[:, g * 128:(g + 1) * 128], in1=pa[:], op=OP.add)

            # ---- layer-0 table from x: replicated input, so each rank builds
            # the FULL table locally (no collective, no exposed startup AG).
            for c, (g0, g1, tab, cbase) in enumerate(
                    ((0, C0G, qv_tl[0][0], C0N), (C0G, NGRP, qv_tl[0][1], C1N))):
                for rb in range(R):
                    g = g0
                    while g < g1:
                        ns = min(8, g1 - g)
                        lh = stg.tile([128, 1024], DT, tag="lh")
                        nc.sync.dma_start(
                            lh[:, 0:ns * 128],
                            t_xrb[rb * 128:(rb + 1) * 128,
                                  g * 128:(g + ns) * 128])
                        st = stg.tile([128, 8, 2 * H], TDT, tag="st")
                        for s in range(0, ns, 2):
                            n2 = min(2, ns - s)
                            ps = pnode.tile([128, 512], F32, tag="pn")
                            for u in range(n2):
                                nc.tensor.matmul(
                                    out=ps[:, u * 256:u * 256 + 2 * H],
                                    lhsT=lh[:, (s + u) * 128:(s + u + 1) * 128],
                                    rhs=Wqvk_t[0][:, 0:2 * H], start=True,
                                    stop=not meta["has_bv"],
                                    skip_group_check=True)
                                if meta["has_bv"]:
                                    nc.tensor.matmul(
                                        out=ps[:, u * 256:u * 256 + 2 * H],
                                        lhsT=ones_t[:],
                                        rhs=bv_t[0][0:1, 0:2 * H],
                                        start=False, stop=True,
                                        skip_group_check=True)
                            if (s // 2) % 2 == 0:
                                nc.scalar.activation(st[:, s:s + n2, :],
                                                     ps[:, 0:n2 * 256], AF.Copy)
                            else:
                                nc.vector.tensor_copy(st[:, s:s + n2, :],
                                                      ps[:, 0:n2 * 256])
                        r0 = rb * cbase + (g - g0) * 128
                        nc.sync.dma_start(
                            tab[r0:r0 + ns * 128, :].rearrange(
                                "(s p) f -> p s f", p=128),
                            st[:, 0:ns, :])
                        g += ns
            # local k table + s-part for layer 0 from the local x slice
            for g in range(NGRP):
                psk = pnode.tile([128, 512], F32, tag="pn")
                nc.tensor.matmul(out=psk[:, 0:H],
                                 lhsT=xT_t[:, g * 128:(g + 1) * 128],
                                 rhs=Wqvk_t[0][:, 2 * H:3 * H],
                                 start=True, stop=True, skip_group_check=True)
                nc.scalar.activation(k_pp[0][:, g, :], psk[:, 0:H], AF.Copy)
            emit_spart(0, xT_t)

            pp = ppool.tile([G, H], F32)
            indc = consts.tile([128, NGRP, G], DT)
            nc.sync.dma_start(
                indc[:], t_IndT[:].rearrange("(c p) g -> p c g", p=128))

            def emit_pool_part(c0g, c1g):
                for c in range(c0g, c1g):
                    trp = pedge.tile([128, 128], DT, tag="pa")
                    nc.tensor.transpose(out=trp[:],
                                        in_=h3f[:, c * 128:(c + 1) * 128],
                                        identity=ident[:])
                    hnode = tail.tile([128, 128], DT, tag="hnode")
                    nc.vector.tensor_copy(hnode[:], trp[:])
                    nc.tensor.matmul(out=pp[:],
                                     lhsT=indc[:, c, :],
                                     rhs=hnode[:],
                                     start=(c == 0), stop=(c == NGRP - 1))

            for l in range(NLAYER):
                hsrc_next = h3_pp[l % 2]
                if parts >= 2:
                    # first half: both epochs over groups 0..C0G
                    emit_edges(0, l, 0, C0G)
                    emit_edges(1, l, 0, C0G)
                # h3 chunk-0 final: start next layer's table chunk 0 (or pool)
                emit_leaky(l, 0)
                if l < NLAYER - 1:
                    build_chunk(l + 1, 0, hsrc_next)
                    emit_ag(l + 1, 0)
                elif parts >= 4:
                    emit_pool_part(0, C0G)
                if parts >= 2:
                    emit_edges(0, l, C0G, NGRP)
                    emit_edges(1, l, C0G, NGRP)
                if parts < 3:
                    break
                emit_leaky(l, 1)
                if l < NLAYER - 1:
                    build_chunk(l + 1, 1, hsrc_next)
                    emit_ag(l + 1, 1)
                    emit_spart(l + 1, hsrc_next)
                elif parts >= 4:
                    emit_pool_part(C0G, NGRP)

            if parts < 4:
                z_dbg = tail.tile([G, NCLS], F32, tag="zsb")
                nc.vector.tensor_copy(z_dbg[:], hs_pp[0][0:G, 0:NCLS])
                nc.sync.dma_start(t_out[:], z_dbg[:])
            else:
                pool_sb = tail.tile([G, H], F32, tag="poolsb")
                nc.vector.tensor_copy(pool_sb[:], pp[:])
                nc.sync.dma_start(pool_in[:], pool_sb[:])
                nc.gpsimd.collective_compute(
                    "AllGather", OP.bypass, replica_groups=[list(range(R))],
                    ins=[pool_in[:]], outs=[pool_out[:]])
                pr = tail.tile([G, R, H], F32, tag="pr")
                nc.sync.dma_start(pr[:], pool_out[:].rearrange("(r g) h -> g r h", r=R))
                pooled = tail.tile([G, H], F32, tag="pooled")
                nc.vector.tensor_tensor(out=pooled[:], in0=pr[:, 0, :], in1=pr[:, 1, :],
                                        op=OP.add)
                for r in range(2, R):
                    nc.vector.tensor_tensor(out=pooled[:], in0=pooled[:],
                                            in1=pr[:, r, :], op=OP.add)
                ptp = pedge.tile([H, G], F32, tag="pa")
                nc.tensor.transpose(out=ptp[:], in_=pooled[:], identity=identf[0:G, 0:G])
                pooledT = tail.tile([H, G], F32, tag="pooledT")
                nc.vector.tensor_copy(pooledT[:], ptp[:])
                zp = pedge.tile([G, NCLS], F32, tag="pa")
                nc.tensor.matmul(out=zp[:], lhsT=pooledT[:], rhs=Wch_t[:],
                                 start=True, stop=False)
                nc.tensor.matmul(out=zp[:], lhsT=clin_t[:], rhs=Wcc_t[:],
                                 start=False, stop=True)
                z_sb = tail.tile([G, NCLS], F32, tag="zsb")
                nc.vector.tensor_tensor(out=z_sb[:], in0=zp[:], in1=bc_t[:],
                                        op=OP.add)
                nc.sync.dma_start(t_out[:], z_sb[:])

    nc.compile()
    return nc


# ---------------------------------------------------------------------------

_CACHE = {}


def kernel(**inputs):
    in_maps, meta = prep(inputs)
    key = tuple(sorted((k, v) for k, v in meta.items()))
    if key not in _CACHE:
        _CACHE[key] = build(meta)
    nc = _CACHE[key]
    res = run_bass_kernel_spmd(nc, in_maps, list(range(R)))
    return np.asarray(res.results[0]["out"], np.float32)


def kernel_profiled(**inputs):
    """Like kernel() but also returns (exec_time_ns, trace_path)."""
    in_maps, meta = prep(inputs)
    key = tuple(sorted((k, v) for k, v in meta.items()))
    if key not in _CACHE:
        _CACHE[key] = build(meta)
    nc = _CACHE[key]
    res = run_bass_kernel_spmd(nc, in_maps, list(range(R)), trace=True)
    out = np.asarray(res.results[0]["out"], np.float32)
    trace_path = None
    if res.instructions_and_trace is not None:
        trace_path = res.instructions_and_trace[1]
    return out, res.exec_time_ns, trace_path


if __name__ == "__main__":
    pass


# revision 36
# speedup vs baseline: 1.6065x; 1.0103x over previous
"""Trainium2 Bass kernel for a 3-layer ResGatedGraphConv GNN (ClinicalGatedGCN).

Strategy (8 NeuronCores, SPMD), v3:
  - Nodes partitioned into 8 contiguous ranges of 6250 (padded to 6272 = 49
    groups of 128). Edges assigned to the rank owning their dst node, sorted
    by (src-chunk epoch, dst-group, dst) on the host.
  - The [q|v] node table is built DISTRIBUTED: each rank computes q,v (and k)
    only for its OWN nodes from its SBUF-resident h (one fused matmul per
    128-node group against [Wq|Wv|Wk]), stores the [q|v] rows to local DRAM,
    and AllGathers the table in two chunks (groups 0:24 -> qv_t0, 24:49 ->
    qv_t1; both tables stay under the int16 gather-index limit). This removes
    the per-layer h AllGather, all h re-reads, and ~7/8 of the table matmuls
    and PSUM->SBUF copies of the replicated design.
  - One dma_gather per (epoch, dst-group) fetches the src rows of [q|v].
    k[dst] is expanded on the PE via a host-shipped 0/1 selector ST; the
    segment-sum over dst is a PE matmul against selector S. S and ST are
    packed in ONE fp8 blob per (epoch, group) (0/1 is exact in fp8; matmul
    operands may mix fp8 with bf16), halving selector DMA traffic.
  - The edge-embedding rank-1 term (attr x We + gate bias) is batched: one
    K=5 matmul per 4-tile PSUM chunk against a block-diagonal [5, 4H]
    constant, with per-chunk attr columns shipped once as an fp8 constant.
  - BatchNorm folding: A = gamma/sqrt(var+eps) is folded into the NEXT
    layer's weights (and the classifier); B is folded into effective biases.
    The on-device h update is a single fused leaky-ReLU
    (scalar_tensor_tensor max(x, slope*x)).
  - Mean-pool per graph is a matmul against a host-built indicator with
    1/cnt folded in; partial pools are AllGather'd and summed; the tiny
    classifier runs on every core.
"""

import numpy as np
import ml_dtypes

import concourse.bacc as bacc
import concourse.bass as bass
import concourse.mybir as mybir
import concourse.tile as tile
from concourse.bass_utils import run_bass_kernel_spmd
from concourse.masks import make_identity

F32 = mybir.dt.float32
BF16 = mybir.dt.bfloat16
FP8 = mybir.dt.float8e4
I16 = mybir.dt.int16
AF = mybir.ActivationFunctionType
OP = mybir.AluOpType

# ---------------- problem constants (hardcoded per spec) ----------------
N, E, H, G, NCLIN, NCLS = 50000, 800000, 128, 64, 16, 2
NLAYER = 3
EPS = 1e-5
SLOPE = 0.01
R = 8                      # ranks / NeuronCores

NPR = (N + R - 1) // R     # real nodes per rank (6250)
NGRP = (NPR + 127) // 128  # 128-node groups per rank (49)
NPAD = NGRP * 128          # padded nodes per rank (6272)
C0G = 24                   # groups in AG chunk 0
C1G = NGRP - C0G           # groups in AG chunk 1 (25)
C0N = C0G * 128            # 3072 rows
C1N = C1G * 128            # 3200 rows
KQC = 4                    # psum chunk size in edge tiles (1 PSUM bank)


def wrap_idxs_block(idx):
    """Wrap one gather call's indices: idx j -> [j%16, j//16], tiled to 128 parts."""
    n = len(idx)
    assert n % 16 == 0
    w = np.asarray(idx, np.int16).reshape(n // 16, 16).T
    return np.tile(w, (8, 1))


# ---------------------------------------------------------------------------
# host-side preprocessing
# ---------------------------------------------------------------------------

def prep(inputs):
    x = np.asarray(inputs["x"], np.float32)
    edge_index = np.asarray(inputs["edge_index"])
    edge_attr = np.asarray(inputs["edge_attr"], np.float32)[:, 0]
    batch = np.asarray(inputs["batch"]).astype(np.int64)
    clinical = np.asarray(inputs["clinical"], np.float32)
    Wk, bk = np.asarray(inputs["Wk"], np.float32), np.asarray(inputs["bk"], np.float32)
    Wq, bq = np.asarray(inputs["Wq"], np.float32), np.asarray(inputs["bq"], np.float32)
    Wv, bv = np.asarray(inputs["Wv"], np.float32), np.asarray(inputs["bv"], np.float32)
    Ws, bs = np.asarray(inputs["Ws"], np.float32), np.asarray(inputs["bs"], np.float32)
    We, be = np.asarray(inputs["We"], np.float32), np.asarray(inputs["be"], np.float32)
    gamma = np.asarray(inputs["gamma"], np.float32)
    beta = np.asarray(inputs["beta"], np.float32)
    rmean = np.asarray(inputs["rmean"], np.float32)
    rvar = np.asarray(inputs["rvar"], np.float32)
    Wc, bc = np.asarray(inputs["Wc"], np.float32), np.asarray(inputs["bc"], np.float32)

    src = edge_index[0].astype(np.int64)
    dst = edge_index[1].astype(np.int64)

    # BN folded: true h_out = A*leaky(pre) + B; device h3 = leaky(pre).
    A = gamma / np.sqrt(rvar + EPS)          # [3, H]
    B = beta - rmean * A                     # [3, H]

    # effective weights: fold diag(A[l-1]) into layer-l input maps, and the
    # B[l-1] offset into layer-l biases.
    Wq_e = np.stack([Wq[l] * (A[l - 1][:, None] if l else 1.0) for l in range(NLAYER)])
    Wk_e = np.stack([Wk[l] * (A[l - 1][:, None] if l else 1.0) for l in range(NLAYER)])
    Wv_e = np.stack([Wv[l] * (A[l - 1][:, None] if l else 1.0) for l in range(NLAYER)])
    Ws_e = np.stack([Ws[l] * (A[l - 1][:, None] if l else 1.0) for l in range(NLAYER)])
    bgate = np.stack([bk[l] + bq[l] + be[l]
                      + (B[l - 1] @ (Wk[l] + Wq[l]) if l else 0.0)
                      for l in range(NLAYER)])
    bv_e = np.stack([bv[l] + (B[l - 1] @ Wv[l] if l else 0.0) for l in range(NLAYER)])
    bs_e = np.stack([bs[l] + (B[l - 1] @ Ws[l] if l else 0.0) for l in range(NLAYER)])
    Wc_h = Wc[0:H] * A[2][:, None]
    bc_e = bc + B[2] @ Wc[0:H]

    Wqvk = np.concatenate([Wq_e, Wv_e, Wk_e], axis=2)     # [3, H, 3H]

    # block-diagonal edge-term constant: rows 0..3 carry We on diag block,
    # row 4 carries the gate bias (broadcast to each tile block).
    BD = np.zeros((NLAYER, 5, KQC * H), np.float32)
    for l in range(NLAYER):
        for t in range(KQC):
            BD[l, t, t * H:(t + 1) * H] = We[l, 0, :]
            BD[l, 4, t * H:(t + 1) * H] = bgate[l]

    has_bv = bool(np.any(bv_e != 0))
    has_bs = bool(np.any(bs_e != 0))

    # ---- edge structure ----
    e_rank = dst // NPR
    dst_local = dst - e_rank * NPR
    grp = dst_local // 128
    drel = dst_local % 128
    rs = src // NPR
    lr = src - rs * NPR
    ep = (lr >= C0N).astype(np.int64)
    trow = np.where(ep == 0, rs * C0N + lr, rs * C1N + (lr - C0N))

    counts = np.zeros((2, R, NGRP), np.int64)
    np.add.at(counts, (ep, e_rank, grp), 1)
    nt_l = [np.ceil(counts[e].max(axis=0) / 128).astype(int) for e in (0, 1)]
    off_l = [np.concatenate([[0], np.cumsum(nt)]).astype(int) for nt in nt_l]
    nch_l = [np.ceil(nt / KQC).astype(int) for nt in nt_l]
    choff_l = [np.concatenate([[0], np.cumsum(nc)]).astype(int) for nc in nch_l]

    cntg = np.bincount(batch, minlength=G).astype(np.float32)
    inv_cnt = 1.0 / np.maximum(cntg, 1.0)

    order = np.lexsort((dst, grp, ep, e_rank))
    trow_s, drel_s, attr_s = trow[order], drel[order], edge_attr[order]
    key = (e_rank[order] * 2 + ep[order]) * NGRP + grp[order]
    starts = np.searchsorted(key, np.arange(R * 2 * NGRP + 1))

    in_maps = []
    for r in range(R):
        ep_arrs = {}
        for e in (0, 1):
            nt = nt_l[e]
            off = off_l[e]
            nch = nch_l[e]
            choff = choff_l[e]
            tot = int(off[-1])
            chtot = int(choff[-1])
            gidx = np.zeros((128, tot * 8), np.int16)
            blob = np.zeros((128, tot * 256), ml_dtypes.float8_e4m3)
            attr4 = np.zeros((5, chtot * 128), ml_dtypes.float8_e4m3)
            attr4[4, :] = 1.0
            for g in range(NGRP):
                ntg = int(nt[g])
                if ntg == 0:
                    continue
                k = (r * 2 + e) * NGRP + g
                s0 = int(starts[k])
                n = int(counts[e, r, g])
                o = int(off[g])
                co = int(choff[g])
                idx = np.zeros((ntg * 128,), np.int64)
                idx[:n] = trow_s[s0:s0 + n]
                gidx[:, o * 8:(o + ntg) * 8] = wrap_idxs_block(idx)
                j = np.arange(n)
                t = j // 128
                p = j % 128
                dr = drel_s[s0:s0 + n].astype(np.int64)
                # S: [p_edge, d] at cols (o+t)*256 + d
                blob[p, (o + t) * 256 + dr] = 1
                # ST: [drel, p_edge] at cols (o+t)*256 + 128 + p
                blob[dr, (o + t) * 256 + 128 + p] = 1
                attr4[t % KQC, (co + t // KQC) * 128 + p] = \
                    attr_s[s0:s0 + n].astype(ml_dtypes.float8_e4m3)
            ep_arrs[e] = (gidx, blob, attr4)
        IndT = np.zeros((NPAD, G), np.float32)
        lo, hi = r * NPR, min((r + 1) * NPR, N)
        IndT[np.arange(hi - lo), batch[lo:hi]] = inv_cnt[batch[lo:hi]]
        xT_loc = np.zeros((128, NPAD), np.float32)
        xT_loc[:, 0:hi - lo] = x[lo:hi].T
        x_rb = np.zeros((R * 128, NPAD), np.float32)
        for rr in range(R):
            rlo, rhi = rr * NPR, min((rr + 1) * NPR, N)
            x_rb[rr * 128:(rr + 1) * 128, 0:rhi - rlo] = x[rlo:rhi].T
        im = {
            "xT_loc": xT_loc.astype(ml_dtypes.bfloat16),
            "x_rb": x_rb.astype(ml_dtypes.float8_e4m3),
            "Wqvk": Wqvk.astype(ml_dtypes.bfloat16),
            "Ws": Ws_e.astype(ml_dtypes.bfloat16),
            "BD": BD.astype(ml_dtypes.bfloat16),
            "bv_row": np.concatenate(
                [np.zeros((NLAYER, 1, H), np.float32),
                 bv_e.reshape(NLAYER, 1, H),
                 np.zeros((NLAYER, 1, H), np.float32)], axis=2
            ).astype(ml_dtypes.bfloat16),
            "bs_col": bs_e.reshape(NLAYER, H, 1),
            "gidx0": ep_arrs[0][0], "blob0": ep_arrs[0][1], "attr0": ep_arrs[0][2],
            "gidx1": ep_arrs[1][0], "blob1": ep_arrs[1][1], "attr1": ep_arrs[1][2],
            "IndT": IndT.astype(ml_dtypes.bfloat16),
            "clinT": clinical.T.copy(),
            "Wc_h": Wc_h, "Wc_c": Wc[H:H + NCLIN],
            "bc_rep": np.tile(bc_e, (G, 1)),
        }
        in_maps.append(im)
    meta = dict(NT0=tuple(int(v) for v in nt_l[0]),
                NT1=tuple(int(v) for v in nt_l[1]),
                has_bv=has_bv, has_bs=has_bs, tab_fp8=True, dr_seg=True)
    return in_maps, meta


# ---------------------------------------------------------------------------
# device program
# ---------------------------------------------------------------------------

def build(meta):
    parts = meta.get("parts", 4)
    DT = BF16
    TDT = FP8 if meta.get("tab_fp8", True) else BF16
    NT_L = [list(meta["NT0"]), list(meta["NT1"])]
    OFF_L = [np.concatenate([[0], np.cumsum(nt)]).astype(int) for nt in NT_L]
    NCH_L = [np.ceil(np.array(nt) / KQC).astype(int) for nt in NT_L]
    CHOFF_L = [np.concatenate([[0], np.cumsum(nc)]).astype(int) for nc in NCH_L]
    TOT = [int(o[-1]) for o in OFF_L]
    CHTOT = [int(c[-1]) for c in CHOFF_L]
    TMAX = max(max(NT_L[0]), max(NT_L[1]))

    nc = bacc.Bacc("TRN2", target_bir_lowering=False, debug=False, num_devices=R)

    def din(name, shape, dt):
        return nc.dram_tensor(name, shape, dt, kind="ExternalInput").ap()

    t_xT = din("xT_loc", [128, NPAD], DT)
    t_xrb = din("x_rb", [R * 128, NPAD], FP8)
    t_Wqvk = din("Wqvk", [NLAYER, H, 3 * H], DT)
    t_Ws = din("Ws", [NLAYER, H, H], DT)
    t_BD = din("BD", [NLAYER, 5, KQC * H], DT)
    t_bv = din("bv_row", [NLAYER, 1, 3 * H], DT)
    t_bs = din("bs_col", [NLAYER, H, 1], F32)
    t_gidx = [din("gidx0", [128, TOT[0] * 8], I16),
              din("gidx1", [128, TOT[1] * 8], I16)]
    t_blob = [din("blob0", [128, TOT[0] * 256], FP8),
              din("blob1", [128, TOT[1] * 256], FP8)]
    t_attr = [din("attr0", [5, CHTOT[0] * 128], FP8),
              din("attr1", [5, CHTOT[1] * 128], FP8)]
    t_IndT = din("IndT", [NPAD, G], DT)
    t_clinT = din("clinT", [NCLIN, G], F32)
    t_Wc_h = din("Wc_h", [H, NCLS], F32)
    t_Wc_c = din("Wc_c", [NCLIN, NCLS], F32)
    t_bc = din("bc_rep", [G, NCLS], F32)

    t_out = nc.dram_tensor("out", [G, NCLS], F32, kind="ExternalOutput").ap()

    qv_loc = nc.dram_tensor("qv_loc", [NPAD, 2 * H], TDT).ap()
    # per-layer AG'd table pairs (separate per layer so the next layer's
    # AllGather never overwrites a table the current layer still gathers from)
    qv_tl = [None,
             [nc.dram_tensor("qv1_t0", [R * C0N, 2 * H], TDT, addr_space="Shared").ap(),
              nc.dram_tensor("qv1_t1", [R * C1N, 2 * H], TDT, addr_space="Shared").ap()],
             [nc.dram_tensor("qv2_t0", [R * C0N, 2 * H], TDT, addr_space="Shared").ap(),
              nc.dram_tensor("qv2_t1", [R * C1N, 2 * H], TDT, addr_space="Shared").ap()]]
    # layer-0 tables are built locally (x is replicated), no collective
    qv_tl[0] = [nc.dram_tensor("qv0_t0", [R * C0N, 2 * H], TDT).ap(),
                nc.dram_tensor("qv0_t1", [R * C1N, 2 * H], TDT).ap()]
    pool_in = nc.dram_tensor("pool_in", [G, H], F32).ap()
    pool_out = nc.dram_tensor("pool_out", [R * G, H], F32, addr_space="Shared").ap()

    with tile.TileContext(nc) as tc:
        import contextlib
        with contextlib.ExitStack() as ctx:
            consts = ctx.enter_context(tc.tile_pool(name="consts", bufs=1))
            hsb = ctx.enter_context(tc.tile_pool(name="hsb", bufs=1))
            h3p = ctx.enter_context(tc.tile_pool(name="h3p", bufs=1))
            ksb = ctx.enter_context(tc.tile_pool(name="ksb", bufs=1))
            stg = ctx.enter_context(tc.tile_pool(name="stg", bufs=4))
            tail = ctx.enter_context(tc.tile_pool(name="tail", bufs=2))
            edg = ctx.enter_context(tc.tile_pool(name="edg", bufs=5))
            sel = ctx.enter_context(tc.tile_pool(name="sel", bufs=6))
            edm = ctx.enter_context(tc.tile_pool(name="edm", bufs=3))
            pnode = ctx.enter_context(tc.tile_pool(name="pnode", bufs=2, space="PSUM"))
            pkq = ctx.enter_context(tc.tile_pool(name="pkq", bufs=2, space="PSUM"))
            pedge = ctx.enter_context(tc.tile_pool(name="pedge", bufs=2, space="PSUM"))
            ppool = ctx.enter_context(tc.tile_pool(name="ppool", bufs=1, space="PSUM"))

            _cid = [0]

            def load_const(src_ap, shape, dt):
                _cid[0] += 1
                t = consts.tile(shape, dt, tag=f"c{_cid[0]}_{src_ap.tensor.name}")
                nc.sync.dma_start(t[:], src_ap)
                return t

            Wqvk_t = [load_const(t_Wqvk[l], [H, 3 * H], DT) for l in range(NLAYER)]
            Ws_t = [load_const(t_Ws[l], [H, H], DT) for l in range(NLAYER)]
            BD_t = [load_const(t_BD[l], [5, KQC * H], DT) for l in range(NLAYER)]
            bv_t = ([load_const(t_bv[l], [1, 3 * H], DT) for l in range(NLAYER)]
                    if meta["has_bv"] else None)
            bs_t = ([load_const(t_bs[l], [H, 1], F32) for l in range(NLAYER)]
                    if meta["has_bs"] else None)
            ones_t = consts.tile([1, 128], DT)
            nc.vector.memset(ones_t[:], 1.0)
            gidx_t = [load_const(t_gidx[0], [128, TOT[0] * 8], I16),
                      load_const(t_gidx[1], [128, TOT[1] * 8], I16)]
            attr_t = [load_const(t_attr[0], [5, CHTOT[0] * 128], FP8),
                      load_const(t_attr[1], [5, CHTOT[1] * 128], FP8)]
            xT_t = load_const(t_xT, [128, NPAD], DT)
            ident = consts.tile([128, 128], DT)
            make_identity(nc, ident[:])
            identf = consts.tile([128, 128], F32)
            make_identity(nc, identf[:])
            clin_t = load_const(t_clinT, [NCLIN, G], F32)
            Wch_t = load_const(t_Wc_h, [H, NCLS], F32)
            Wcc_t = load_const(t_Wc_c, [NCLIN, NCLS], F32)
            bc_t = load_const(t_bc, [G, NCLS], F32)

            # ping-pong state by layer parity
            hs_pp = [hsb.tile([128, NPAD], DT, tag="hs0", name="hs0"),
                     hsb.tile([128, NPAD], DT, tag="hs1", name="hs1")]
            k_pp = [ksb.tile([128, NGRP, H], DT, tag="k0", name="k0"),
                    ksb.tile([128, NGRP, H], DT, tag="k1", name="k1")]
            h3_pp = [h3p.tile([128, NPAD], DT, tag="h3a", name="h3a"),
                     h3p.tile([128, NPAD], DT, tag="h3b", name="h3b")]
            h3f = h3_pp[0]                             # layer-2 output parity

            def build_chunk(l, c, hsrc):
                """Local q|v|k for groups of AG-chunk c of layer l."""
                g0, g1 = (0, C0G) if c == 0 else (C0G, NGRP)
                k_sb = k_pp[l % 2]
                g = g0
                while g < g1:
                    ns = min(8, g1 - g)
                    st = stg.tile([128, 8, 2 * H], TDT, tag="st")
                    for s in range(ns):
                        gg = g + s
                        ps = pnode.tile([128, 512], F32, tag="pn")
                        nc.tensor.matmul(
                            out=ps[:, 0:3 * H],
                            lhsT=hsrc[:, gg * 128:(gg + 1) * 128],
                            rhs=Wqvk_t[l][:], start=True,
                            stop=not meta["has_bv"],
                            skip_group_check=True)
                        if meta["has_bv"]:
                            nc.tensor.matmul(
                                out=ps[:, 0:3 * H], lhsT=ones_t[:],
                                rhs=bv_t[l][:], start=False, stop=True,
                                skip_group_check=True)
                        nc.scalar.activation(st[:, s, :], ps[:, 0:2 * H],
                                             AF.Copy)
                        nc.scalar.activation(k_sb[:, gg, :],
                                             ps[:, 2 * H:3 * H], AF.Copy)
                    nc.sync.dma_start(
                        qv_loc[g * 128:(g + ns) * 128, :].rearrange(
                            "(s p) f -> p s f", p=128),
                        st[:, 0:ns, :])
                    g += ns

            def emit_ag(l, c):
                ins = qv_loc[0:C0N, :] if c == 0 else qv_loc[C0N:NPAD, :]
                nc.gpsimd.collective_compute(
                    "AllGather", OP.bypass, replica_groups=[list(range(R))],
                    ins=[ins], outs=[qv_tl[l][c][:]])

            def emit_spart(l, hsrc):
                hs = hs_pp[l % 2]
                for c0 in range(0, NPAD, 512):
                    csz = min(512, NPAD - c0)
                    pss = pnode.tile([128, 512], F32, tag="pn")
                    nc.tensor.matmul(out=pss[:, 0:csz], lhsT=Ws_t[l][:],
                                     rhs=hsrc[:, c0:c0 + csz],
                                     start=True, stop=True)
                    if meta["has_bs"]:
                        nc.scalar.activation(hs[:, c0:c0 + csz], pss[:, 0:csz],
                                             AF.Identity, bias=bs_t[l][:],
                                             scale=1.0)
                    else:
                        nc.scalar.activation(hs[:, c0:c0 + csz], pss[:, 0:csz],
                                             AF.Copy)

            def emit_leaky(l, c):
                sl = slice(0, C0N) if c == 0 else slice(C0N, NPAD)
                nc.vector.scalar_tensor_tensor(
                    out=h3_pp[l % 2][:, sl], in0=hs_pp[l % 2][:, sl],
                    scalar=float(SLOPE), in1=hs_pp[l % 2][:, sl],
                    op0=OP.mult, op1=OP.max)

            def emit_edges(e, l, g0, g1):
                nt_list = NT_L[e]
                off = OFF_L[e]
                choff = CHOFF_L[e]
                hs = hs_pp[l % 2]
                k_sb = k_pp[l % 2]
                tab = qv_tl[l][e]
                for g in range(g0, g1):
                    nt = int(nt_list[g])
                    if nt == 0:
                        continue
                    ne = nt * 128
                    o = int(off[g])
                    co = int(choff[g])
                    bt = sel.tile([128, TMAX, 256], FP8, tag="blob")
                    nc.sync.dma_start(
                        bt[:, 0:nt, :],
                        t_blob[e][:, o * 256:(o + nt) * 256].rearrange(
                            "p (t d) -> p t d", t=nt))
                    gt = edg.tile([128, TMAX, 2 * H], TDT, tag="g")
                    nc.gpsimd.dma_gather(
                        gt[:, 0:nt, :], tab[:],
                        gidx_t[e][:, o * 8:(o + nt) * 8],
                        ne, ne, 2 * H, single_packet=(ne <= 512))
                    ktb = edm.tile([128, TMAX, 128], DT, tag="kt")
                    for ci in range((nt + KQC - 1) // KQC):
                        q0 = ci * KQC
                        qn = min(KQC, nt - q0)
                        pk = pkq.tile([128, KQC, 128], F32, tag="kq")
                        nc.tensor.matmul(
                            out=pk[:, 0:KQC, :],
                            lhsT=attr_t[e][0:5, (co + ci) * 128:(co + ci + 1) * 128],
                            rhs=BD_t[l][:], start=True, stop=False,
                            skip_group_check=True)
                        for t in range(q0, q0 + qn):
                            nc.tensor.matmul(
                                out=pk[:, t - q0, :],
                                lhsT=bt[:, t, 128:256],
                                rhs=k_sb[:, g, :], start=False, stop=False,
                                skip_group_check=True)
                        # q[src] folded into the same PSUM via identity matmul
                        nc.tensor.matmul(
                            out=pk[:, 0:qn, :], lhsT=ident[:],
                            rhs=gt[:, q0:q0 + qn, 0:H], start=False, stop=True,
                            skip_group_check=True)
                        nc.scalar.activation(ktb[:, q0:q0 + qn, :],
                                             pk[:, 0:qn, :], AF.Sigmoid)
                    dr = meta.get("dr_seg", False)
                    msg = edm.tile([128, TMAX, 128], FP8 if dr else DT, tag="msg")
                    nc.vector.tensor_tensor(out=msg[:, 0:nt, :],
                                            in0=ktb[:, 0:nt, :],
                                            in1=gt[:, 0:nt, H:2 * H], op=OP.mult)
                    pa = pedge.tile([128, 128], F32, tag="pa")
                    if dr:
                        npair = nt // 2
                        for t2 in range(0, npair * 2, 2):
                            nc.tensor.matmul(
                                out=pa[:], lhsT=msg[:, t2:t2 + 2, :],
                                rhs=bt[:, t2:t2 + 2, 0:128], start=(t2 == 0),
                                stop=(t2 + 2 == nt),
                                perf_mode=mybir.MatmulPerfMode.DoubleRow,
                                skip_group_check=True)
                        if nt % 2:
                            nc.tensor.matmul(out=pa[:], lhsT=msg[:, nt - 1, :],
                                             rhs=bt[:, nt - 1, 0:128],
                                             start=(nt == 1), stop=True,
                                             skip_group_check=True)
                    else:
                        for t in range(nt):
                            nc.tensor.matmul(out=pa[:], lhsT=msg[:, t, :],
                                             rhs=bt[:, t, 0:128], start=(t == 0),
                                             stop=(t == nt - 1))
                    nc.vector.tensor_tensor(
                        out=hs[:, g * 128:(g + 1) * 128],
                        in0=hs[:, g * 128:(g + 1) * 128], in1=pa[:], op=OP.add)

            # ---- layer-0 table from x: replicated input, so each rank builds
            # the FULL table locally (no collective, no exposed startup AG).
            for c, (g0, g1, tab, cbase) in enumerate(
                    ((0, C0G, qv_tl[0][0], C0N), (C0G, NGRP, qv_tl[0][1], C1N))):
                for rb in range(R):
                    g = g0
                    while g < g1:
                        ns = min(8, g1 - g)
                        lh = stg.tile([128, 1024], FP8, tag="lh")
                        nc.sync.dma_start(
                            lh[:, 0:ns * 128],
                            t_xrb[rb * 128:(rb + 1) * 128,
                                  g * 128:(g + ns) * 128])
                        st = stg.tile([128, 8, 2 * H], TDT, tag="st")
                        for s in range(0, ns, 2):
                            n2 = min(2, ns - s)
                            ps = pnode.tile([128, 512], F32, tag="pn")
                            for u in range(n2):
                                nc.tensor.matmul(
                                    out=ps[:, u * 256:u * 256 + 2 * H],
                                    lhsT=lh[:, (s + u) * 128:(s + u + 1) * 128],
                                    rhs=Wqvk_t[0][:, 0:2 * H], start=True,
                                    stop=not meta["has_bv"],
                                    skip_group_check=True)
                                if meta["has_bv"]:
                                    nc.tensor.matmul(
                                        out=ps[:, u * 256:u * 256 + 2 * H],
                                        lhsT=ones_t[:],
                                        rhs=bv_t[0][0:1, 0:2 * H],
                                        start=False, stop=True,
                                        skip_group_check=True)
                            if (s // 2) % 2 == 0:
                                nc.scalar.activation(st[:, s:s + n2, :],
                                                     ps[:, 0:n2 * 256], AF.Copy)
                            else:
                                nc.vector.tensor_copy(st[:, s:s + n2, :],
                                                      ps[:, 0:n2 * 256])
                        r0 = rb * cbase + (g - g0) * 128
                        nc.sync.dma_start(
                            tab[r0:r0 + ns * 128, :].rearrange(
                                "(s p) f -> p s f", p=128),
                            st[:, 0:ns, :])
                        g += ns
            # local k table + s-part for layer 0 from the local x slice
            for g in range(NGRP):
                psk = pnode.tile([128, 512], F32, tag="pn")
                nc.tensor.matmul(out=psk[:, 0:H],
                                 lhsT=xT_t[:, g * 128:(g + 1) * 128],
                                 rhs=Wqvk_t[0][:, 2 * H:3 * H],
                                 start=True, stop=True, skip_group_check=True)
                nc.scalar.activation(k_pp[0][:, g, :], psk[:, 0:H], AF.Copy)
            emit_spart(0, xT_t)

            pp = ppool.tile([G, H], F32)
            indc = consts.tile([128, NGRP, G], DT)
            nc.sync.dma_start(
                indc[:], t_IndT[:].rearrange("(c p) g -> p c g", p=128))

            def emit_pool_part(c0g, c1g):
                for c in range(c0g, c1g):
                    trp = pedge.tile([128, 128], DT, tag="pa")
                    nc.tensor.transpose(out=trp[:],
                                        in_=h3f[:, c * 128:(c + 1) * 128],
                                        identity=ident[:])
                    hnode = tail.tile([128, 128], DT, tag="hnode")
                    nc.vector.tensor_copy(hnode[:], trp[:])
                    nc.tensor.matmul(out=pp[:],
                                     lhsT=indc[:, c, :],
                                     rhs=hnode[:],
                                     start=(c == 0), stop=(c == NGRP - 1))

            for l in range(NLAYER):
                hsrc_next = h3_pp[l % 2]
                if parts >= 2:
                    # full epoch-0 sweep first (its table is ready), then the
                    # first ep1 half; this keeps ep0 work ahead of any ep1
                    # stall on the second table chunk's AllGather.
                    emit_edges(0, l, 0, NGRP)
                    emit_edges(1, l, 0, C0G)
                # h3 chunk-0 final: start next layer's table chunk 0 (or pool)
                emit_leaky(l, 0)
                if l < NLAYER - 1:
                    build_chunk(l + 1, 0, hsrc_next)
                    emit_ag(l + 1, 0)
                elif parts >= 4:
                    emit_pool_part(0, C0G)
                if parts >= 2:
                    emit_edges(1, l, C0G, NGRP)
                if parts < 3:
                    break
                emit_leaky(l, 1)
                if l < NLAYER - 1:
                    build_chunk(l + 1, 1, hsrc_next)
                    emit_ag(l + 1, 1)
                    emit_spart(l + 1, hsrc_next)
                elif parts >= 4:
                    emit_pool_part(C0G, NGRP)

            if parts < 4:
                z_dbg = tail.tile([G, NCLS], F32, tag="zsb")
                nc.vector.tensor_copy(z_dbg[:], hs_pp[0][0:G, 0:NCLS])
                nc.sync.dma_start(t_out[:], z_dbg[:])
            else:
                pool_sb = tail.tile([G, H], F32, tag="poolsb")
                nc.vector.tensor_copy(pool_sb[:], pp[:])
                nc.sync.dma_start(pool_in[:], pool_sb[:])
                nc.gpsimd.collective_compute(
                    "AllGather", OP.bypass, replica_groups=[list(range(R))],
                    ins=[pool_in[:]], outs=[pool_out[:]])
                pr = tail.tile([G, R, H], F32, tag="pr")
                nc.sync.dma_start(pr[:], pool_out[:].rearrange("(r g) h -> g r h", r=R))
                pooled = tail.tile([G, H], F32, tag="pooled")
                nc.vector.tensor_tensor(out=pooled[:], in0=pr[:, 0, :], in1=pr[:, 1, :],
                                        op=OP.add)
                for r in range(2, R):
                    nc.vector.tensor_tensor(out=pooled[:], in0=pooled[:],
                                            in1=pr[:, r, :], op=OP.add)
                ptp = pedge.tile([H, G], F32, tag="pa")
                nc.tensor.transpose(out=ptp[:], in_=pooled[:], identity=identf[0:G, 0:G])
                pooledT = tail.tile([H, G], F32, tag="pooledT")
                nc.vector.tensor_copy(pooledT[:], ptp[:])
                zp = pedge.tile([G, NCLS], F32, tag="pa")
                nc.tensor.matmul(out=zp[:], lhsT=pooledT[:], rhs=Wch_t[:],
                                 start=True, stop=False)
                nc.tensor.matmul(out=zp[:], lhsT=clin_t[:], rhs=Wcc_t[:],
                                 start=False, stop=True)
                z_sb = tail.tile([G, NCLS], F32, tag="zsb")
                nc.vector.tensor_tensor(out=z_sb[:], in0=zp[:], in1=bc_t[:],
                                        op=OP.add)
                nc.sync.dma_start(t_out[:], z_sb[:])

    nc.compile()
    return nc


# ---------------------------------------------------------------------------

_CACHE = {}


def kernel(**inputs):
    in_maps, meta = prep(inputs)
    key = tuple(sorted((k, v) for k, v in meta.items()))
    if key not in _CACHE:
        _CACHE[key] = build(meta)
    nc = _CACHE[key]
    res = run_bass_kernel_spmd(nc, in_maps, list(range(R)))
    return np.asarray(res.results[0]["out"], np.float32)


def kernel_profiled(**inputs):
    """Like kernel() but also returns (exec_time_ns, trace_path)."""
    in_maps, meta = prep(inputs)
    key = tuple(sorted((k, v) for k, v in meta.items()))
    if key not in _CACHE:
        _CACHE[key] = build(meta)
    nc = _CACHE[key]
    res = run_bass_kernel_spmd(nc, in_maps, list(range(R)), trace=True)
    out = np.asarray(res.results[0]["out"], np.float32)
    trace_path = None
    if res.instructions_and_trace is not None:
        trace_path = res.instructions_and_trace[1]
    return out, res.exec_time_ns, trace_path


if __name__ == "__main__":
    pass


# revision 45
# speedup vs baseline: 1.6363x; 1.0185x over previous
"""Trainium2 Bass kernel for a 3-layer ResGatedGraphConv GNN (ClinicalGatedGCN).

Strategy (8 NeuronCores, SPMD), v3:
  - Nodes partitioned into 8 contiguous ranges of 6250 (padded to 6272 = 49
    groups of 128). Edges assigned to the rank owning their dst node, sorted
    by (src-chunk epoch, dst-group, dst) on the host.
  - The [q|v] node table is built DISTRIBUTED: each rank computes q,v (and k)
    only for its OWN nodes from its SBUF-resident h (one fused matmul per
    128-node group against [Wq|Wv|Wk]), stores the [q|v] rows to local DRAM,
    and AllGathers the table in two chunks (groups 0:24 -> qv_t0, 24:49 ->
    qv_t1; both tables stay under the int16 gather-index limit). This removes
    the per-layer h AllGather, all h re-reads, and ~7/8 of the table matmuls
    and PSUM->SBUF copies of the replicated design.
  - One dma_gather per (epoch, dst-group) fetches the src rows of [q|v].
    k[dst] is expanded on the PE via a host-shipped 0/1 selector ST; the
    segment-sum over dst is a PE matmul against selector S. S and ST are
    packed in ONE fp8 blob per (epoch, group) (0/1 is exact in fp8; matmul
    operands may mix fp8 with bf16), halving selector DMA traffic.
  - The edge-embedding rank-1 term (attr x We + gate bias) is batched: one
    K=5 matmul per 4-tile PSUM chunk against a block-diagonal [5, 4H]
    constant, with per-chunk attr columns shipped once as an fp8 constant.
  - BatchNorm folding: A = gamma/sqrt(var+eps) is folded into the NEXT
    layer's weights (and the classifier); B is folded into effective biases.
    The on-device h update is a single fused leaky-ReLU
    (scalar_tensor_tensor max(x, slope*x)).
  - Mean-pool per graph is a matmul against a host-built indicator with
    1/cnt folded in; partial pools are AllGather'd and summed; the tiny
    classifier runs on every core.
"""

import numpy as np
import ml_dtypes

import concourse.bacc as bacc
import concourse.bass as bass
import concourse.mybir as mybir
import concourse.tile as tile
from concourse.bass_utils import run_bass_kernel_spmd
from concourse.masks import make_identity

F32 = mybir.dt.float32
BF16 = mybir.dt.bfloat16
FP8 = mybir.dt.float8e4
I16 = mybir.dt.int16
AF = mybir.ActivationFunctionType
OP = mybir.AluOpType

# ---------------- problem constants (hardcoded per spec) ----------------
N, E, H, G, NCLIN, NCLS = 50000, 800000, 128, 64, 16, 2
NLAYER = 3
EPS = 1e-5
SLOPE = 0.01
R = 8                      # ranks / NeuronCores

NPR = (N + R - 1) // R     # real nodes per rank (6250)
NGRP = (NPR + 127) // 128  # 128-node groups per rank (49)
NPAD = NGRP * 128          # padded nodes per rank (6272)
C0G = 24                   # groups in AG chunk 0
C1G = NGRP - C0G           # groups in AG chunk 1 (25)
C0N = C0G * 128            # 3072 rows
C1N = C1G * 128            # 3200 rows
KQC = 4                    # psum chunk size in edge tiles (1 PSUM bank)


def wrap_idxs_block(idx):
    """Wrap one gather call's indices: idx j -> [j%16, j//16], tiled to 128 parts."""
    n = len(idx)
    assert n % 16 == 0
    w = np.asarray(idx, np.int16).reshape(n // 16, 16).T
    return np.tile(w, (8, 1))


# ---------------------------------------------------------------------------
# host-side preprocessing
# ---------------------------------------------------------------------------

def prep(inputs):
    x = np.asarray(inputs["x"], np.float32)
    edge_index = np.asarray(inputs["edge_index"])
    edge_attr = np.asarray(inputs["edge_attr"], np.float32)[:, 0]
    batch = np.asarray(inputs["batch"]).astype(np.int64)
    clinical = np.asarray(inputs["clinical"], np.float32)
    Wk, bk = np.asarray(inputs["Wk"], np.float32), np.asarray(inputs["bk"], np.float32)
    Wq, bq = np.asarray(inputs["Wq"], np.float32), np.asarray(inputs["bq"], np.float32)
    Wv, bv = np.asarray(inputs["Wv"], np.float32), np.asarray(inputs["bv"], np.float32)
    Ws, bs = np.asarray(inputs["Ws"], np.float32), np.asarray(inputs["bs"], np.float32)
    We, be = np.asarray(inputs["We"], np.float32), np.asarray(inputs["be"], np.float32)
    gamma = np.asarray(inputs["gamma"], np.float32)
    beta = np.asarray(inputs["beta"], np.float32)
    rmean = np.asarray(inputs["rmean"], np.float32)
    rvar = np.asarray(inputs["rvar"], np.float32)
    Wc, bc = np.asarray(inputs["Wc"], np.float32), np.asarray(inputs["bc"], np.float32)

    src = edge_index[0].astype(np.int64)
    dst = edge_index[1].astype(np.int64)

    # BN folded: true h_out = A*leaky(pre) + B; device h3 = leaky(pre).
    A = gamma / np.sqrt(rvar + EPS)          # [3, H]
    B = beta - rmean * A                     # [3, H]

    # effective weights: fold diag(A[l-1]) into layer-l input maps, and the
    # B[l-1] offset into layer-l biases.
    Wq_e = np.stack([Wq[l] * (A[l - 1][:, None] if l else 1.0) for l in range(NLAYER)])
    Wk_e = np.stack([Wk[l] * (A[l - 1][:, None] if l else 1.0) for l in range(NLAYER)])
    Wv_e = np.stack([Wv[l] * (A[l - 1][:, None] if l else 1.0) for l in range(NLAYER)])
    Ws_e = np.stack([Ws[l] * (A[l - 1][:, None] if l else 1.0) for l in range(NLAYER)])
    bgate = np.stack([bk[l] + bq[l] + be[l]
                      + (B[l - 1] @ (Wk[l] + Wq[l]) if l else 0.0)
                      for l in range(NLAYER)])
    bv_e = np.stack([bv[l] + (B[l - 1] @ Wv[l] if l else 0.0) for l in range(NLAYER)])
    bs_e = np.stack([bs[l] + (B[l - 1] @ Ws[l] if l else 0.0) for l in range(NLAYER)])
    Wc_h = Wc[0:H] * A[2][:, None]
    bc_e = bc + B[2] @ Wc[0:H]

    Wqvk = np.concatenate([Wq_e, Wv_e, Wk_e], axis=2)     # [3, H, 3H]

    # block-diagonal edge-term constant: rows 0..3 carry We on diag block,
    # row 4 carries the gate bias (broadcast to each tile block).
    BD = np.zeros((NLAYER, 5, KQC * H), np.float32)
    for l in range(NLAYER):
        for t in range(KQC):
            BD[l, t, t * H:(t + 1) * H] = We[l, 0, :]
            BD[l, 4, t * H:(t + 1) * H] = bgate[l]

    has_bv = bool(np.any(bv_e != 0))
    has_bs = bool(np.any(bs_e != 0))

    # ---- edge structure ----
    e_rank = dst // NPR
    dst_local = dst - e_rank * NPR
    grp = dst_local // 128
    drel = dst_local % 128
    rs = src // NPR
    lr = src - rs * NPR
    ep = (lr >= C0N).astype(np.int64)
    trow = np.where(ep == 0, rs * C0N + lr, rs * C1N + (lr - C0N))

    counts = np.zeros((2, R, NGRP), np.int64)
    np.add.at(counts, (ep, e_rank, grp), 1)
    nt_l = [np.ceil(counts[e].max(axis=0) / 128).astype(int) for e in (0, 1)]
    off_l = [np.concatenate([[0], np.cumsum(nt)]).astype(int) for nt in nt_l]
    nch_l = [np.ceil(nt / KQC).astype(int) for nt in nt_l]
    choff_l = [np.concatenate([[0], np.cumsum(nc)]).astype(int) for nc in nch_l]

    cntg = np.bincount(batch, minlength=G).astype(np.float32)
    inv_cnt = 1.0 / np.maximum(cntg, 1.0)

    order = np.lexsort((dst, grp, ep, e_rank))
    trow_s, drel_s, attr_s = trow[order], drel[order], edge_attr[order]
    key = (e_rank[order] * 2 + ep[order]) * NGRP + grp[order]
    starts = np.searchsorted(key, np.arange(R * 2 * NGRP + 1))

    in_maps = []
    for r in range(R):
        ep_arrs = {}
        for e in (0, 1):
            nt = nt_l[e]
            off = off_l[e]
            nch = nch_l[e]
            choff = choff_l[e]
            tot = int(off[-1])
            chtot = int(choff[-1])
            gidx = np.zeros((128, tot * 8), np.int16)
            blob = np.zeros((128, tot * 256), ml_dtypes.float8_e4m3)
            attr4 = np.zeros((5, chtot * 128), ml_dtypes.float8_e4m3)
            attr4[4, :] = 1.0
            for g in range(NGRP):
                ntg = int(nt[g])
                if ntg == 0:
                    continue
                k = (r * 2 + e) * NGRP + g
                s0 = int(starts[k])
                n = int(counts[e, r, g])
                o = int(off[g])
                co = int(choff[g])
                idx = np.zeros((ntg * 128,), np.int64)
                idx[:n] = trow_s[s0:s0 + n]
                gidx[:, o * 8:(o + ntg) * 8] = wrap_idxs_block(idx)
                j = np.arange(n)
                t = j // 128
                p = j % 128
                dr = drel_s[s0:s0 + n].astype(np.int64)
                # S: [p_edge, d] at cols (o+t)*256 + d
                blob[p, (o + t) * 256 + dr] = 1
                # ST: [drel, p_edge] at cols (o+t)*256 + 128 + p
                blob[dr, (o + t) * 256 + 128 + p] = 1
                attr4[t % KQC, (co + t // KQC) * 128 + p] = \
                    attr_s[s0:s0 + n].astype(ml_dtypes.float8_e4m3)
            ep_arrs[e] = (gidx, blob, attr4)
        IndT = np.zeros((NPAD, G), np.float32)
        lo, hi = r * NPR, min((r + 1) * NPR, N)
        IndT[np.arange(hi - lo), batch[lo:hi]] = inv_cnt[batch[lo:hi]]
        xT_loc = np.zeros((128, NPAD), np.float32)
        xT_loc[:, 0:hi - lo] = x[lo:hi].T
        x_rb = np.zeros((R * 128, NPAD), np.float32)
        for rr in range(R):
            rlo, rhi = rr * NPR, min((rr + 1) * NPR, N)
            x_rb[rr * 128:(rr + 1) * 128, 0:rhi - rlo] = x[rlo:rhi].T
        im = {
            "xT_loc": xT_loc.astype(ml_dtypes.bfloat16),
            "x_rb": x_rb.astype(ml_dtypes.float8_e4m3),
            "Wqvk": Wqvk.astype(ml_dtypes.bfloat16),
            "Ws": Ws_e.astype(ml_dtypes.bfloat16),
            "BD": BD.astype(ml_dtypes.bfloat16),
            "bv_row": np.concatenate(
                [np.zeros((NLAYER, 1, H), np.float32),
                 bv_e.reshape(NLAYER, 1, H),
                 np.zeros((NLAYER, 1, H), np.float32)], axis=2
            ).astype(ml_dtypes.bfloat16),
            "bs_col": bs_e.reshape(NLAYER, H, 1),
            "gidx0": ep_arrs[0][0], "blob0": ep_arrs[0][1], "attr0": ep_arrs[0][2],
            "gidx1": ep_arrs[1][0], "blob1": ep_arrs[1][1], "attr1": ep_arrs[1][2],
            "IndT": IndT.astype(ml_dtypes.bfloat16),
            "clinT": clinical.T.copy(),
            "Wc_h": Wc_h, "Wc_c": Wc[H:H + NCLIN],
            "bc_rep": np.tile(bc_e, (G, 1)),
        }
        in_maps.append(im)
    meta = dict(NT0=tuple(int(v) for v in nt_l[0]),
                NT1=tuple(int(v) for v in nt_l[1]),
                has_bv=has_bv, has_bs=has_bs, tab_fp8=True, dr_seg=True)
    return in_maps, meta


# ---------------------------------------------------------------------------
# device program
# ---------------------------------------------------------------------------

def build(meta):
    parts = meta.get("parts", 4)
    DT = BF16
    TDT = FP8 if meta.get("tab_fp8", True) else BF16
    NT_L = [list(meta["NT0"]), list(meta["NT1"])]
    OFF_L = [np.concatenate([[0], np.cumsum(nt)]).astype(int) for nt in NT_L]
    NCH_L = [np.ceil(np.array(nt) / KQC).astype(int) for nt in NT_L]
    CHOFF_L = [np.concatenate([[0], np.cumsum(nc)]).astype(int) for nc in NCH_L]
    TOT = [int(o[-1]) for o in OFF_L]
    CHTOT = [int(c[-1]) for c in CHOFF_L]
    TMAX = max(max(NT_L[0]), max(NT_L[1]))

    nc = bacc.Bacc("TRN2", target_bir_lowering=False, debug=False, num_devices=R)

    def din(name, shape, dt):
        return nc.dram_tensor(name, shape, dt, kind="ExternalInput").ap()

    t_xT = din("xT_loc", [128, NPAD], DT)
    t_xrb = din("x_rb", [R * 128, NPAD], FP8)
    t_Wqvk = din("Wqvk", [NLAYER, H, 3 * H], DT)
    t_Ws = din("Ws", [NLAYER, H, H], DT)
    t_BD = din("BD", [NLAYER, 5, KQC * H], DT)
    t_bv = din("bv_row", [NLAYER, 1, 3 * H], DT)
    t_bs = din("bs_col", [NLAYER, H, 1], F32)
    t_gidx = [din("gidx0", [128, TOT[0] * 8], I16),
              din("gidx1", [128, TOT[1] * 8], I16)]
    t_blob = [din("blob0", [128, TOT[0] * 256], FP8),
              din("blob1", [128, TOT[1] * 256], FP8)]
    t_attr = [din("attr0", [5, CHTOT[0] * 128], FP8),
              din("attr1", [5, CHTOT[1] * 128], FP8)]
    t_IndT = din("IndT", [NPAD, G], DT)
    t_clinT = din("clinT", [NCLIN, G], F32)
    t_Wc_h = din("Wc_h", [H, NCLS], F32)
    t_Wc_c = din("Wc_c", [NCLIN, NCLS], F32)
    t_bc = din("bc_rep", [G, NCLS], F32)

    t_out = nc.dram_tensor("out", [G, NCLS], F32, kind="ExternalOutput").ap()

    qv_loc = nc.dram_tensor("qv_loc", [NPAD, 2 * H], TDT).ap()
    # per-layer AG'd table pairs (separate per layer so the next layer's
    # AllGather never overwrites a table the current layer still gathers from)
    qv_tl = [None,
             [nc.dram_tensor("qv1_t0", [R * C0N, 2 * H], TDT, addr_space="Shared").ap(),
              nc.dram_tensor("qv1_t1", [R * C1N, 2 * H], TDT, addr_space="Shared").ap()],
             [nc.dram_tensor("qv2_t0", [R * C0N, 2 * H], TDT, addr_space="Shared").ap(),
              nc.dram_tensor("qv2_t1", [R * C1N, 2 * H], TDT, addr_space="Shared").ap()]]
    # layer-0 tables are built locally (x is replicated), no collective
    qv_tl[0] = [nc.dram_tensor("qv0_t0", [R * C0N, 2 * H], TDT).ap(),
                nc.dram_tensor("qv0_t1", [R * C1N, 2 * H], TDT).ap()]
    pool_in = nc.dram_tensor("pool_in", [G, H], F32).ap()
    pool_out = nc.dram_tensor("pool_out", [R * G, H], F32, addr_space="Shared").ap()

    with tile.TileContext(nc) as tc:
        import contextlib
        with contextlib.ExitStack() as ctx:
            consts = ctx.enter_context(tc.tile_pool(name="consts", bufs=1))
            hsb = ctx.enter_context(tc.tile_pool(name="hsb", bufs=1))
            h3p = ctx.enter_context(tc.tile_pool(name="h3p", bufs=1))
            ksb = ctx.enter_context(tc.tile_pool(name="ksb", bufs=1))
            stg = ctx.enter_context(tc.tile_pool(name="stg", bufs=4))
            tail = ctx.enter_context(tc.tile_pool(name="tail", bufs=2))
            edg = ctx.enter_context(tc.tile_pool(name="edg", bufs=5))
            sel = ctx.enter_context(tc.tile_pool(name="sel", bufs=6))
            edm = ctx.enter_context(tc.tile_pool(name="edm", bufs=3))
            pnode = ctx.enter_context(tc.tile_pool(name="pnode", bufs=2, space="PSUM"))
            pkq = ctx.enter_context(tc.tile_pool(name="pkq", bufs=2, space="PSUM"))
            pedge = ctx.enter_context(tc.tile_pool(name="pedge", bufs=2, space="PSUM"))
            ppool = ctx.enter_context(tc.tile_pool(name="ppool", bufs=1, space="PSUM"))

            _cid = [0]

            def load_const(src_ap, shape, dt):
                _cid[0] += 1
                t = consts.tile(shape, dt, tag=f"c{_cid[0]}_{src_ap.tensor.name}")
                nc.sync.dma_start(t[:], src_ap)
                return t

            Wqvk_t = [load_const(t_Wqvk[l], [H, 3 * H], DT) for l in range(NLAYER)]
            Ws_t = [load_const(t_Ws[l], [H, H], DT) for l in range(NLAYER)]
            BD_t = [load_const(t_BD[l], [5, KQC * H], DT) for l in range(NLAYER)]
            bv_t = ([load_const(t_bv[l], [1, 3 * H], DT) for l in range(NLAYER)]
                    if meta["has_bv"] else None)
            bs_t = ([load_const(t_bs[l], [H, 1], F32) for l in range(NLAYER)]
                    if meta["has_bs"] else None)
            ones_t = consts.tile([1, 128], DT)
            nc.vector.memset(ones_t[:], 1.0)
            gidx_t = [load_const(t_gidx[0], [128, TOT[0] * 8], I16),
                      load_const(t_gidx[1], [128, TOT[1] * 8], I16)]
            attr_t = [load_const(t_attr[0], [5, CHTOT[0] * 128], FP8),
                      load_const(t_attr[1], [5, CHTOT[1] * 128], FP8)]
            xT_t = load_const(t_xT, [128, NPAD], DT)
            ident = consts.tile([128, 128], DT)
            make_identity(nc, ident[:])
            identf = consts.tile([128, 128], F32)
            make_identity(nc, identf[:])
            clin_t = load_const(t_clinT, [NCLIN, G], F32)
            Wch_t = load_const(t_Wc_h, [H, NCLS], F32)
            Wcc_t = load_const(t_Wc_c, [NCLIN, NCLS], F32)
            bc_t = load_const(t_bc, [G, NCLS], F32)

            # ping-pong state by layer parity
            hs_pp = [hsb.tile([128, NPAD], DT, tag="hs0", name="hs0"),
                     hsb.tile([128, NPAD], DT, tag="hs1", name="hs1")]
            k_pp = [ksb.tile([128, NGRP, H], DT, tag="k0", name="k0"),
                    ksb.tile([128, NGRP, H], DT, tag="k1", name="k1")]
            h3_pp = [h3p.tile([128, NPAD], DT, tag="h3a", name="h3a"),
                     h3p.tile([128, NPAD], DT, tag="h3b", name="h3b")]
            h3f = h3_pp[0]                             # layer-2 output parity

            def build_chunk(l, c, hsrc):
                """Local q|v|k for groups of AG-chunk c of layer l."""
                g0, g1 = (0, C0G) if c == 0 else (C0G, NGRP)
                k_sb = k_pp[l % 2]
                g = g0
                while g < g1:
                    ns = min(8, g1 - g)
                    st = stg.tile([128, 8, 2 * H], TDT, tag="st")
                    for s in range(ns):
                        gg = g + s
                        ps = pnode.tile([128, 512], F32, tag="pn")
                        nc.tensor.matmul(
                            out=ps[:, 0:3 * H],
                            lhsT=hsrc[:, gg * 128:(gg + 1) * 128],
                            rhs=Wqvk_t[l][:], start=True,
                            stop=not meta["has_bv"],
                            skip_group_check=True)
                        if meta["has_bv"]:
                            nc.tensor.matmul(
                                out=ps[:, 0:3 * H], lhsT=ones_t[:],
                                rhs=bv_t[l][:], start=False, stop=True,
                                skip_group_check=True)
                        nc.scalar.activation(st[:, s, :], ps[:, 0:2 * H],
                                             AF.Copy)
                        nc.scalar.activation(k_sb[:, gg, :],
                                             ps[:, 2 * H:3 * H], AF.Copy)
                    nc.sync.dma_start(
                        qv_loc[g * 128:(g + ns) * 128, :].rearrange(
                            "(s p) f -> p s f", p=128),
                        st[:, 0:ns, :])
                    g += ns

            def emit_ag(l, c):
                ins = qv_loc[0:C0N, :] if c == 0 else qv_loc[C0N:NPAD, :]
                nc.gpsimd.collective_compute(
                    "AllGather", OP.bypass, replica_groups=[list(range(R))],
                    ins=[ins], outs=[qv_tl[l][c][:]])

            def emit_spart(l, hsrc):
                hs = hs_pp[l % 2]
                for c0 in range(0, NPAD, 512):
                    csz = min(512, NPAD - c0)
                    pss = pnode.tile([128, 512], F32, tag="pn")
                    nc.tensor.matmul(out=pss[:, 0:csz], lhsT=Ws_t[l][:],
                                     rhs=hsrc[:, c0:c0 + csz],
                                     start=True, stop=True)
                    if meta["has_bs"]:
                        nc.scalar.activation(hs[:, c0:c0 + csz], pss[:, 0:csz],
                                             AF.Identity, bias=bs_t[l][:],
                                             scale=1.0)
                    else:
                        nc.scalar.activation(hs[:, c0:c0 + csz], pss[:, 0:csz],
                                             AF.Copy)

            def emit_leaky(l, c):
                sl = slice(0, C0N) if c == 0 else slice(C0N, NPAD)
                nc.vector.scalar_tensor_tensor(
                    out=h3_pp[l % 2][:, sl], in0=hs_pp[l % 2][:, sl],
                    scalar=float(SLOPE), in1=hs_pp[l % 2][:, sl],
                    op0=OP.mult, op1=OP.max)

            def emit_edges(e, l, g0, g1):
                nt_list = NT_L[e]
                off = OFF_L[e]
                choff = CHOFF_L[e]
                hs = hs_pp[l % 2]
                k_sb = k_pp[l % 2]
                tab = qv_tl[l][e]
                for g in range(g0, g1):
                    nt = int(nt_list[g])
                    if nt == 0:
                        continue
                    ne = nt * 128
                    o = int(off[g])
                    co = int(choff[g])
                    bt = sel.tile([128, TMAX, 256], FP8, tag="blob")
                    nc.sync.dma_start(
                        bt[:, 0:nt, :],
                        t_blob[e][:, o * 256:(o + nt) * 256].rearrange(
                            "p (t d) -> p t d", t=nt))
                    gt = edg.tile([128, TMAX, 2 * H], TDT, tag="g")
                    nc.gpsimd.dma_gather(
                        gt[:, 0:nt, :], tab[:],
                        gidx_t[e][:, o * 8:(o + nt) * 8],
                        ne, ne, 2 * H, single_packet=(ne <= 512))
                    ktb = edm.tile([128, TMAX, 128], DT, tag="kt")
                    for ci in range((nt + KQC - 1) // KQC):
                        q0 = ci * KQC
                        qn = min(KQC, nt - q0)
                        pk = pkq.tile([128, KQC, 128], F32, tag="kq")
                        nc.tensor.matmul(
                            out=pk[:, 0:KQC, :],
                            lhsT=attr_t[e][0:5, (co + ci) * 128:(co + ci + 1) * 128],
                            rhs=BD_t[l][:], start=True, stop=False,
                            skip_group_check=True)
                        for t in range(q0, q0 + qn):
                            nc.tensor.matmul(
                                out=pk[:, t - q0, :],
                                lhsT=bt[:, t, 128:256],
                                rhs=k_sb[:, g, :], start=False, stop=False,
                                skip_group_check=True)
                        # q[src] folded into the same PSUM via identity matmul
                        nc.tensor.matmul(
                            out=pk[:, 0:qn, :], lhsT=ident[:],
                            rhs=gt[:, q0:q0 + qn, 0:H], start=False, stop=True,
                            skip_group_check=True)
                        nc.scalar.activation(ktb[:, q0:q0 + qn, :],
                                             pk[:, 0:qn, :], AF.Sigmoid)
                    dr = meta.get("dr_seg", False)
                    msg = edm.tile([128, TMAX, 128], FP8 if dr else DT, tag="msg")
                    nc.vector.tensor_tensor(out=msg[:, 0:nt, :],
                                            in0=ktb[:, 0:nt, :],
                                            in1=gt[:, 0:nt, H:2 * H], op=OP.mult)
                    pa = pedge.tile([128, 128], F32, tag="pa")
                    if dr:
                        npair = nt // 2
                        for t2 in range(0, npair * 2, 2):
                            nc.tensor.matmul(
                                out=pa[:], lhsT=msg[:, t2:t2 + 2, :],
                                rhs=bt[:, t2:t2 + 2, 0:128], start=(t2 == 0),
                                stop=(t2 + 2 == nt),
                                perf_mode=mybir.MatmulPerfMode.DoubleRow,
                                skip_group_check=True)
                        if nt % 2:
                            nc.tensor.matmul(out=pa[:], lhsT=msg[:, nt - 1, :],
                                             rhs=bt[:, nt - 1, 0:128],
                                             start=(nt == 1), stop=True,
                                             skip_group_check=True)
                    else:
                        for t in range(nt):
                            nc.tensor.matmul(out=pa[:], lhsT=msg[:, t, :],
                                             rhs=bt[:, t, 0:128], start=(t == 0),
                                             stop=(t == nt - 1))
                    nc.vector.tensor_tensor(
                        out=hs[:, g * 128:(g + 1) * 128],
                        in0=hs[:, g * 128:(g + 1) * 128], in1=pa[:], op=OP.add)

            # ---- layer-0 table from x: replicated input, so each rank builds
            # the FULL table locally (no collective, no exposed startup AG).
            def build_l0_chunk(c):
                g0, g1, tab, cbase = ((0, C0G, qv_tl[0][0], C0N) if c == 0
                                      else (C0G, NGRP, qv_tl[0][1], C1N))
                for rb in range(R):
                    g = g0
                    while g < g1:
                        ns = min(8, g1 - g)
                        lh = stg.tile([128, 1024], FP8, tag="lh")
                        nc.sync.dma_start(
                            lh[:, 0:ns * 128],
                            t_xrb[rb * 128:(rb + 1) * 128,
                                  g * 128:(g + ns) * 128])
                        st = stg.tile([128, 8, 2 * H], TDT, tag="st")
                        for s in range(0, ns, 2):
                            n2 = min(2, ns - s)
                            ps = pnode.tile([128, 512], F32, tag="pn")
                            for u in range(n2):
                                nc.tensor.matmul(
                                    out=ps[:, u * 256:u * 256 + 2 * H],
                                    lhsT=lh[:, (s + u) * 128:(s + u + 1) * 128],
                                    rhs=Wqvk_t[0][:, 0:2 * H], start=True,
                                    stop=not meta["has_bv"],
                                    skip_group_check=True)
                                if meta["has_bv"]:
                                    nc.tensor.matmul(
                                        out=ps[:, u * 256:u * 256 + 2 * H],
                                        lhsT=ones_t[:],
                                        rhs=bv_t[0][0:1, 0:2 * H],
                                        start=False, stop=True,
                                        skip_group_check=True)
                            if (s // 2) % 2 == 0:
                                nc.scalar.activation(st[:, s:s + n2, :],
                                                     ps[:, 0:n2 * 256], AF.Copy)
                            else:
                                nc.vector.tensor_copy(st[:, s:s + n2, :],
                                                      ps[:, 0:n2 * 256])
                        r0 = rb * cbase + (g - g0) * 128
                        nc.sync.dma_start(
                            tab[r0:r0 + ns * 128, :].rearrange(
                                "(s p) f -> p s f", p=128),
                            st[:, 0:ns, :])
                        g += ns
            build_l0_chunk(0)
            # local k table + s-part for layer 0 from the local x slice
            for g in range(NGRP):
                psk = pnode.tile([128, 512], F32, tag="pn")
                nc.tensor.matmul(out=psk[:, 0:H],
                                 lhsT=xT_t[:, g * 128:(g + 1) * 128],
                                 rhs=Wqvk_t[0][:, 2 * H:3 * H],
                                 start=True, stop=True, skip_group_check=True)
                nc.scalar.activation(k_pp[0][:, g, :], psk[:, 0:H], AF.Copy)
            emit_spart(0, xT_t)

            pp = ppool.tile([G, H], F32)
            indc = consts.tile([128, NGRP, G], DT)
            nc.sync.dma_start(
                indc[:], t_IndT[:].rearrange("(c p) g -> p c g", p=128))

            def emit_pool_part(c0g, c1g):
                for c in range(c0g, c1g):
                    trp = pedge.tile([128, 128], DT, tag="pa")
                    nc.tensor.transpose(out=trp[:],
                                        in_=h3f[:, c * 128:(c + 1) * 128],
                                        identity=ident[:])
                    hnode = tail.tile([128, 128], DT, tag="hnode")
                    nc.vector.tensor_copy(hnode[:], trp[:])
                    nc.tensor.matmul(out=pp[:],
                                     lhsT=indc[:, c, :],
                                     rhs=hnode[:],
                                     start=(c == 0), stop=(c == NGRP - 1))

            for l in range(NLAYER):
                hsrc_next = h3_pp[l % 2]
                if parts >= 2:
                    # full epoch-0 sweep first (its table is ready), then the
                    # first ep1 half; this keeps ep0 work ahead of any ep1
                    # stall on the second table chunk's AllGather.
                    if l == 0:
                        emit_edges(0, l, 0, 12)
                        build_l0_chunk(1)
                        emit_edges(0, l, 12, NGRP)
                    else:
                        emit_edges(0, l, 0, NGRP)
                    emit_edges(1, l, 0, C0G)
                # h3 chunk-0 final: start next layer's table chunk 0 (or pool)
                emit_leaky(l, 0)
                if l < NLAYER - 1:
                    build_chunk(l + 1, 0, hsrc_next)
                    emit_ag(l + 1, 0)
                elif parts >= 4:
                    emit_pool_part(0, C0G)
                if parts >= 2:
                    emit_edges(1, l, C0G, NGRP)
                if parts < 3:
                    break
                emit_leaky(l, 1)
                if l < NLAYER - 1:
                    build_chunk(l + 1, 1, hsrc_next)
                    emit_ag(l + 1, 1)
                    emit_spart(l + 1, hsrc_next)
                elif parts >= 4:
                    emit_pool_part(C0G, NGRP)

            if parts < 4:
                z_dbg = tail.tile([G, NCLS], F32, tag="zsb")
                nc.vector.tensor_copy(z_dbg[:], hs_pp[0][0:G, 0:NCLS])
                nc.sync.dma_start(t_out[:], z_dbg[:])
            else:
                pool_sb = tail.tile([G, H], F32, tag="poolsb")
                nc.vector.tensor_copy(pool_sb[:], pp[:])
                nc.sync.dma_start(pool_in[:], pool_sb[:])
                nc.gpsimd.collective_compute(
                    "AllGather", OP.bypass, replica_groups=[list(range(R))],
                    ins=[pool_in[:]], outs=[pool_out[:]])
                pr = tail.tile([G, R, H], F32, tag="pr")
                nc.sync.dma_start(pr[:], pool_out[:].rearrange("(r g) h -> g r h", r=R))
                pooled = tail.tile([G, H], F32, tag="pooled")
                nc.vector.tensor_tensor(out=pooled[:], in0=pr[:, 0, :], in1=pr[:, 1, :],
                                        op=OP.add)
                for r in range(2, R):
                    nc.vector.tensor_tensor(out=pooled[:], in0=pooled[:],
                                            in1=pr[:, r, :], op=OP.add)
                ptp = pedge.tile([H, G], F32, tag="pa")
                nc.tensor.transpose(out=ptp[:], in_=pooled[:], identity=identf[0:G, 0:G])
                pooledT = tail.tile([H, G], F32, tag="pooledT")
                nc.vector.tensor_copy(pooledT[:], ptp[:])
                zp = pedge.tile([G, NCLS], F32, tag="pa")
                nc.tensor.matmul(out=zp[:], lhsT=pooledT[:], rhs=Wch_t[:],
                                 start=True, stop=False)
                nc.tensor.matmul(out=zp[:], lhsT=clin_t[:], rhs=Wcc_t[:],
                                 start=False, stop=True)
                z_sb = tail.tile([G, NCLS], F32, tag="zsb")
                nc.vector.tensor_tensor(out=z_sb[:], in0=zp[:], in1=bc_t[:],
                                        op=OP.add)
                nc.sync.dma_start(t_out[:], z_sb[:])

    nc.compile()
    return nc


# ---------------------------------------------------------------------------

_CACHE = {}


def kernel(**inputs):
    in_maps, meta = prep(inputs)
    key = tuple(sorted((k, v) for k, v in meta.items()))
    if key not in _CACHE:
        _CACHE[key] = build(meta)
    nc = _CACHE[key]
    res = run_bass_kernel_spmd(nc, in_maps, list(range(R)))
    return np.asarray(res.results[0]["out"], np.float32)


def kernel_profiled(**inputs):
    """Like kernel() but also returns (exec_time_ns, trace_path)."""
    in_maps, meta = prep(inputs)
    key = tuple(sorted((k, v) for k, v in meta.items()))
    if key not in _CACHE:
        _CACHE[key] = build(meta)
    nc = _CACHE[key]
    res = run_bass_kernel_spmd(nc, in_maps, list(range(R)), trace=True)
    out = np.asarray(res.results[0]["out"], np.float32)
    trace_path = None
    if res.instructions_and_trace is not None:
        trace_path = res.instructions_and_trace[1]
    return out, res.exec_time_ns, trace_path


if __name__ == "__main__":
    pass


# revision 47
# speedup vs baseline: 1.6386x; 1.0014x over previous
"""Trainium2 Bass kernel for a 3-layer ResGatedGraphConv GNN (ClinicalGatedGCN).

Strategy (8 NeuronCores, SPMD), v3:
  - Nodes partitioned into 8 contiguous ranges of 6250 (padded to 6272 = 49
    groups of 128). Edges assigned to the rank owning their dst node, sorted
    by (src-chunk epoch, dst-group, dst) on the host.
  - The [q|v] node table is built DISTRIBUTED: each rank computes q,v (and k)
    only for its OWN nodes from its SBUF-resident h (one fused matmul per
    128-node group against [Wq|Wv|Wk]), stores the [q|v] rows to local DRAM,
    and AllGathers the table in two chunks (groups 0:24 -> qv_t0, 24:49 ->
    qv_t1; both tables stay under the int16 gather-index limit). This removes
    the per-layer h AllGather, all h re-reads, and ~7/8 of the table matmuls
    and PSUM->SBUF copies of the replicated design.
  - One dma_gather per (epoch, dst-group) fetches the src rows of [q|v].
    k[dst] is expanded on the PE via a host-shipped 0/1 selector ST; the
    segment-sum over dst is a PE matmul against selector S. S and ST are
    packed in ONE fp8 blob per (epoch, group) (0/1 is exact in fp8; matmul
    operands may mix fp8 with bf16), halving selector DMA traffic.
  - The edge-embedding rank-1 term (attr x We + gate bias) is batched: one
    K=5 matmul per 4-tile PSUM chunk against a block-diagonal [5, 4H]
    constant, with per-chunk attr columns shipped once as an fp8 constant.
  - BatchNorm folding: A = gamma/sqrt(var+eps) is folded into the NEXT
    layer's weights (and the classifier); B is folded into effective biases.
    The on-device h update is a single fused leaky-ReLU
    (scalar_tensor_tensor max(x, slope*x)).
  - Mean-pool per graph is a matmul against a host-built indicator with
    1/cnt folded in; partial pools are AllGather'd and summed; the tiny
    classifier runs on every core.
"""

import numpy as np
import ml_dtypes

import concourse.bacc as bacc
import concourse.bass as bass
import concourse.mybir as mybir
import concourse.tile as tile
from concourse.bass_utils import run_bass_kernel_spmd
from concourse.masks import make_identity

F32 = mybir.dt.float32
BF16 = mybir.dt.bfloat16
FP8 = mybir.dt.float8e4
I16 = mybir.dt.int16
AF = mybir.ActivationFunctionType
OP = mybir.AluOpType

# ---------------- problem constants (hardcoded per spec) ----------------
N, E, H, G, NCLIN, NCLS = 50000, 800000, 128, 64, 16, 2
NLAYER = 3
EPS = 1e-5
SLOPE = 0.01
R = 8                      # ranks / NeuronCores

NPR = (N + R - 1) // R     # real nodes per rank (6250)
NGRP = (NPR + 127) // 128  # 128-node groups per rank (49)
NPAD = NGRP * 128          # padded nodes per rank (6272)
C0G = 24                   # groups in AG chunk 0
C1G = NGRP - C0G           # groups in AG chunk 1 (25)
C0N = C0G * 128            # 3072 rows
C1N = C1G * 128            # 3200 rows
KQC = 4                    # psum chunk size in edge tiles (1 PSUM bank)


def wrap_idxs_block(idx):
    """Wrap one gather call's indices: idx j -> [j%16, j//16], tiled to 128 parts."""
    n = len(idx)
    assert n % 16 == 0
    w = np.asarray(idx, np.int16).reshape(n // 16, 16).T
    return np.tile(w, (8, 1))


# ---------------------------------------------------------------------------
# host-side preprocessing
# ---------------------------------------------------------------------------

def prep(inputs):
    x = np.asarray(inputs["x"], np.float32)
    edge_index = np.asarray(inputs["edge_index"])
    edge_attr = np.asarray(inputs["edge_attr"], np.float32)[:, 0]
    batch = np.asarray(inputs["batch"]).astype(np.int64)
    clinical = np.asarray(inputs["clinical"], np.float32)
    Wk, bk = np.asarray(inputs["Wk"], np.float32), np.asarray(inputs["bk"], np.float32)
    Wq, bq = np.asarray(inputs["Wq"], np.float32), np.asarray(inputs["bq"], np.float32)
    Wv, bv = np.asarray(inputs["Wv"], np.float32), np.asarray(inputs["bv"], np.float32)
    Ws, bs = np.asarray(inputs["Ws"], np.float32), np.asarray(inputs["bs"], np.float32)
    We, be = np.asarray(inputs["We"], np.float32), np.asarray(inputs["be"], np.float32)
    gamma = np.asarray(inputs["gamma"], np.float32)
    beta = np.asarray(inputs["beta"], np.float32)
    rmean = np.asarray(inputs["rmean"], np.float32)
    rvar = np.asarray(inputs["rvar"], np.float32)
    Wc, bc = np.asarray(inputs["Wc"], np.float32), np.asarray(inputs["bc"], np.float32)

    src = edge_index[0].astype(np.int64)
    dst = edge_index[1].astype(np.int64)

    # BN folded: true h_out = A*leaky(pre) + B; device h3 = leaky(pre).
    A = gamma / np.sqrt(rvar + EPS)          # [3, H]
    B = beta - rmean * A                     # [3, H]

    # effective weights: fold diag(A[l-1]) into layer-l input maps, and the
    # B[l-1] offset into layer-l biases.
    Wq_e = np.stack([Wq[l] * (A[l - 1][:, None] if l else 1.0) for l in range(NLAYER)])
    Wk_e = np.stack([Wk[l] * (A[l - 1][:, None] if l else 1.0) for l in range(NLAYER)])
    Wv_e = np.stack([Wv[l] * (A[l - 1][:, None] if l else 1.0) for l in range(NLAYER)])
    Ws_e = np.stack([Ws[l] * (A[l - 1][:, None] if l else 1.0) for l in range(NLAYER)])
    bgate = np.stack([bk[l] + bq[l] + be[l]
                      + (B[l - 1] @ (Wk[l] + Wq[l]) if l else 0.0)
                      for l in range(NLAYER)])
    bv_e = np.stack([bv[l] + (B[l - 1] @ Wv[l] if l else 0.0) for l in range(NLAYER)])
    bs_e = np.stack([bs[l] + (B[l - 1] @ Ws[l] if l else 0.0) for l in range(NLAYER)])
    Wc_h = Wc[0:H] * A[2][:, None]
    bc_e = bc + B[2] @ Wc[0:H]

    Wqvk = np.concatenate([Wq_e, Wv_e, Wk_e], axis=2)     # [3, H, 3H]

    # block-diagonal edge-term constant: rows 0..3 carry We on diag block,
    # row 4 carries the gate bias (broadcast to each tile block).
    BD = np.zeros((NLAYER, 5, KQC * H), np.float32)
    for l in range(NLAYER):
        for t in range(KQC):
            BD[l, t, t * H:(t + 1) * H] = We[l, 0, :]
            BD[l, 4, t * H:(t + 1) * H] = bgate[l]

    has_bv = bool(np.any(bv_e != 0))
    has_bs = bool(np.any(bs_e != 0))

    # ---- edge structure ----
    e_rank = dst // NPR
    dst_local = dst - e_rank * NPR
    grp = dst_local // 128
    drel = dst_local % 128
    rs = src // NPR
    lr = src - rs * NPR
    ep = (lr >= C0N).astype(np.int64)
    trow = np.where(ep == 0, rs * C0N + lr, rs * C1N + (lr - C0N))

    counts = np.zeros((2, R, NGRP), np.int64)
    np.add.at(counts, (ep, e_rank, grp), 1)
    nt_l = [np.ceil(counts[e].max(axis=0) / 128).astype(int) for e in (0, 1)]
    off_l = [np.concatenate([[0], np.cumsum(nt)]).astype(int) for nt in nt_l]
    nch_l = [np.ceil(nt / KQC).astype(int) for nt in nt_l]
    choff_l = [np.concatenate([[0], np.cumsum(nc)]).astype(int) for nc in nch_l]

    cntg = np.bincount(batch, minlength=G).astype(np.float32)
    inv_cnt = 1.0 / np.maximum(cntg, 1.0)

    order = np.lexsort((dst, grp, ep, e_rank))
    trow_s, drel_s, attr_s = trow[order], drel[order], edge_attr[order]
    key = (e_rank[order] * 2 + ep[order]) * NGRP + grp[order]
    starts = np.searchsorted(key, np.arange(R * 2 * NGRP + 1))

    in_maps = []
    for r in range(R):
        ep_arrs = {}
        for e in (0, 1):
            nt = nt_l[e]
            off = off_l[e]
            nch = nch_l[e]
            choff = choff_l[e]
            tot = int(off[-1])
            chtot = int(choff[-1])
            gidx = np.zeros((128, tot * 8), np.int16)
            blob = np.zeros((128, tot * 256), ml_dtypes.float8_e4m3)
            attr4 = np.zeros((5, chtot * 128), ml_dtypes.float8_e4m3)
            attr4[4, :] = 1.0
            for g in range(NGRP):
                ntg = int(nt[g])
                if ntg == 0:
                    continue
                k = (r * 2 + e) * NGRP + g
                s0 = int(starts[k])
                n = int(counts[e, r, g])
                o = int(off[g])
                co = int(choff[g])
                idx = np.full((ntg * 128,), -1, np.int64)
                idx[:n] = trow_s[s0:s0 + n]
                if n == 0:
                    idx[0] = 0
                gidx[:, o * 8:(o + ntg) * 8] = wrap_idxs_block(idx)
                j = np.arange(n)
                t = j // 128
                p = j % 128
                dr = drel_s[s0:s0 + n].astype(np.int64)
                # S: [p_edge, d] at cols (o+t)*256 + d
                blob[p, (o + t) * 256 + dr] = 1
                # ST: [drel, p_edge] at cols (o+t)*256 + 128 + p
                blob[dr, (o + t) * 256 + 128 + p] = 1
                attr4[t % KQC, (co + t // KQC) * 128 + p] = \
                    attr_s[s0:s0 + n].astype(ml_dtypes.float8_e4m3)
            ep_arrs[e] = (gidx, blob, attr4)
        IndT = np.zeros((NPAD, G), np.float32)
        lo, hi = r * NPR, min((r + 1) * NPR, N)
        IndT[np.arange(hi - lo), batch[lo:hi]] = inv_cnt[batch[lo:hi]]
        xT_loc = np.zeros((128, NPAD), np.float32)
        xT_loc[:, 0:hi - lo] = x[lo:hi].T
        x_rb = np.zeros((R * 128, NPAD), np.float32)
        for rr in range(R):
            rlo, rhi = rr * NPR, min((rr + 1) * NPR, N)
            x_rb[rr * 128:(rr + 1) * 128, 0:rhi - rlo] = x[rlo:rhi].T
        gcnt = np.maximum(counts[:, r, :], 1).astype(np.int32).reshape(1, 2 * NGRP)
        im = {
            "gcnt": gcnt,
            "xT_loc": xT_loc.astype(ml_dtypes.float8_e4m3),
            "x_rb": x_rb.astype(ml_dtypes.float8_e4m3),
            "Wqvk": Wqvk.astype(ml_dtypes.bfloat16),
            "Ws": Ws_e.astype(ml_dtypes.bfloat16),
            "BD": BD.astype(ml_dtypes.bfloat16),
            "bv_row": np.concatenate(
                [np.zeros((NLAYER, 1, H), np.float32),
                 bv_e.reshape(NLAYER, 1, H),
                 np.zeros((NLAYER, 1, H), np.float32)], axis=2
            ).astype(ml_dtypes.bfloat16),
            "bs_col": bs_e.reshape(NLAYER, H, 1),
            "gidx0": ep_arrs[0][0], "blob0": ep_arrs[0][1], "attr0": ep_arrs[0][2],
            "gidx1": ep_arrs[1][0], "blob1": ep_arrs[1][1], "attr1": ep_arrs[1][2],
            "IndT": IndT.astype(ml_dtypes.bfloat16),
            "clinT": clinical.T.copy(),
            "Wc_h": Wc_h, "Wc_c": Wc[H:H + NCLIN],
            "bc_rep": np.tile(bc_e, (G, 1)),
        }
        in_maps.append(im)
    meta = dict(NT0=tuple(int(v) for v in nt_l[0]),
                NT1=tuple(int(v) for v in nt_l[1]),
                has_bv=has_bv, has_bs=has_bs, tab_fp8=True, dr_seg=True)
    return in_maps, meta


# ---------------------------------------------------------------------------
# device program
# ---------------------------------------------------------------------------

def build(meta):
    parts = meta.get("parts", 4)
    DT = BF16
    TDT = FP8 if meta.get("tab_fp8", True) else BF16
    NT_L = [list(meta["NT0"]), list(meta["NT1"])]
    OFF_L = [np.concatenate([[0], np.cumsum(nt)]).astype(int) for nt in NT_L]
    NCH_L = [np.ceil(np.array(nt) / KQC).astype(int) for nt in NT_L]
    CHOFF_L = [np.concatenate([[0], np.cumsum(nc)]).astype(int) for nc in NCH_L]
    TOT = [int(o[-1]) for o in OFF_L]
    CHTOT = [int(c[-1]) for c in CHOFF_L]
    TMAX = max(max(NT_L[0]), max(NT_L[1]))

    nc = bacc.Bacc("TRN2", target_bir_lowering=False, debug=False, num_devices=R)

    def din(name, shape, dt):
        return nc.dram_tensor(name, shape, dt, kind="ExternalInput").ap()

    t_xT = din("xT_loc", [128, NPAD], FP8)
    t_cnt = din("gcnt", [1, 2 * NGRP], mybir.dt.int32)
    t_xrb = din("x_rb", [R * 128, NPAD], FP8)
    t_Wqvk = din("Wqvk", [NLAYER, H, 3 * H], DT)
    t_Ws = din("Ws", [NLAYER, H, H], DT)
    t_BD = din("BD", [NLAYER, 5, KQC * H], DT)
    t_bv = din("bv_row", [NLAYER, 1, 3 * H], DT)
    t_bs = din("bs_col", [NLAYER, H, 1], F32)
    t_gidx = [din("gidx0", [128, TOT[0] * 8], I16),
              din("gidx1", [128, TOT[1] * 8], I16)]
    t_blob = [din("blob0", [128, TOT[0] * 256], FP8),
              din("blob1", [128, TOT[1] * 256], FP8)]
    t_attr = [din("attr0", [5, CHTOT[0] * 128], FP8),
              din("attr1", [5, CHTOT[1] * 128], FP8)]
    t_IndT = din("IndT", [NPAD, G], DT)
    t_clinT = din("clinT", [NCLIN, G], F32)
    t_Wc_h = din("Wc_h", [H, NCLS], F32)
    t_Wc_c = din("Wc_c", [NCLIN, NCLS], F32)
    t_bc = din("bc_rep", [G, NCLS], F32)

    t_out = nc.dram_tensor("out", [G, NCLS], F32, kind="ExternalOutput").ap()

    qv_loc = nc.dram_tensor("qv_loc", [NPAD, 2 * H], TDT).ap()
    # per-layer AG'd table pairs (separate per layer so the next layer's
    # AllGather never overwrites a table the current layer still gathers from)
    qv_tl = [None,
             [nc.dram_tensor("qv1_t0", [R * C0N, 2 * H], TDT, addr_space="Shared").ap(),
              nc.dram_tensor("qv1_t1", [R * C1N, 2 * H], TDT, addr_space="Shared").ap()],
             [nc.dram_tensor("qv2_t0", [R * C0N, 2 * H], TDT, addr_space="Shared").ap(),
              nc.dram_tensor("qv2_t1", [R * C1N, 2 * H], TDT, addr_space="Shared").ap()]]
    # layer-0 tables are built locally (x is replicated), no collective
    qv_tl[0] = [nc.dram_tensor("qv0_t0", [R * C0N, 2 * H], TDT).ap(),
                nc.dram_tensor("qv0_t1", [R * C1N, 2 * H], TDT).ap()]
    pool_in = nc.dram_tensor("pool_in", [G, H], F32).ap()
    pool_out = nc.dram_tensor("pool_out", [R * G, H], F32, addr_space="Shared").ap()

    with tile.TileContext(nc) as tc:
        import contextlib
        with contextlib.ExitStack() as ctx:
            consts = ctx.enter_context(tc.tile_pool(name="consts", bufs=1))
            hsb = ctx.enter_context(tc.tile_pool(name="hsb", bufs=1))
            h3p = ctx.enter_context(tc.tile_pool(name="h3p", bufs=1))
            ksb = ctx.enter_context(tc.tile_pool(name="ksb", bufs=1))
            stg = ctx.enter_context(tc.tile_pool(name="stg", bufs=4))
            tail = ctx.enter_context(tc.tile_pool(name="tail", bufs=2))
            edg = ctx.enter_context(tc.tile_pool(name="edg", bufs=5))
            sel = ctx.enter_context(tc.tile_pool(name="sel", bufs=8))
            edm = ctx.enter_context(tc.tile_pool(name="edm", bufs=3))
            pnode = ctx.enter_context(tc.tile_pool(name="pnode", bufs=2, space="PSUM"))
            pkq = ctx.enter_context(tc.tile_pool(name="pkq", bufs=2, space="PSUM"))
            pedge = ctx.enter_context(tc.tile_pool(name="pedge", bufs=2, space="PSUM"))
            ppool = ctx.enter_context(tc.tile_pool(name="ppool", bufs=1, space="PSUM"))

            _cid = [0]

            def load_const(src_ap, shape, dt):
                _cid[0] += 1
                t = consts.tile(shape, dt, tag=f"c{_cid[0]}_{src_ap.tensor.name}")
                nc.sync.dma_start(t[:], src_ap)
                return t

            Wqvk_t = [load_const(t_Wqvk[l], [H, 3 * H], DT) for l in range(NLAYER)]
            Ws_t = [load_const(t_Ws[l], [H, H], DT) for l in range(NLAYER)]
            BD_t = [load_const(t_BD[l], [5, KQC * H], DT) for l in range(NLAYER)]
            bv_t = ([load_const(t_bv[l], [1, 3 * H], DT) for l in range(NLAYER)]
                    if meta["has_bv"] else None)
            bs_t = ([load_const(t_bs[l], [H, 1], F32) for l in range(NLAYER)]
                    if meta["has_bs"] else None)
            ones_t = consts.tile([1, 128], DT)
            nc.vector.memset(ones_t[:], 1.0)
            gidx_t = [load_const(t_gidx[0], [128, TOT[0] * 8], I16),
                      load_const(t_gidx[1], [128, TOT[1] * 8], I16)]
            attr_t = [load_const(t_attr[0], [5, CHTOT[0] * 128], FP8),
                      load_const(t_attr[1], [5, CHTOT[1] * 128], FP8)]
            xT_t = load_const(t_xT, [128, NPAD], FP8)
            cnt_t = load_const(t_cnt, [1, 2 * NGRP], mybir.dt.int32)
            ident = consts.tile([128, 128], DT)
            make_identity(nc, ident[:])
            identf = consts.tile([128, 128], F32)
            make_identity(nc, identf[:])
            clin_t = load_const(t_clinT, [NCLIN, G], F32)
            Wch_t = load_const(t_Wc_h, [H, NCLS], F32)
            Wcc_t = load_const(t_Wc_c, [NCLIN, NCLS], F32)
            bc_t = load_const(t_bc, [G, NCLS], F32)

            # ping-pong state by layer parity
            hs_pp = [hsb.tile([128, NPAD], DT, tag="hs0", name="hs0"),
                     hsb.tile([128, NPAD], DT, tag="hs1", name="hs1")]
            k_pp = [ksb.tile([128, NGRP, H], DT, tag="k0", name="k0"),
                    ksb.tile([128, NGRP, H], DT, tag="k1", name="k1")]
            h3_pp = [h3p.tile([128, NPAD], DT, tag="h3a", name="h3a"),
                     h3p.tile([128, NPAD], DT, tag="h3b", name="h3b")]
            h3f = h3_pp[0]                             # layer-2 output parity

            def build_chunk(l, c, hsrc):
                """Local q|v|k for groups of AG-chunk c of layer l."""
                g0, g1 = (0, C0G) if c == 0 else (C0G, NGRP)
                k_sb = k_pp[l % 2]
                g = g0
                while g < g1:
                    ns = min(8, g1 - g)
                    st = stg.tile([128, 8, 2 * H], TDT, tag="st")
                    for s in range(ns):
                        gg = g + s
                        ps = pnode.tile([128, 512], F32, tag="pn")
                        nc.tensor.matmul(
                            out=ps[:, 0:3 * H],
                            lhsT=hsrc[:, gg * 128:(gg + 1) * 128],
                            rhs=Wqvk_t[l][:], start=True,
                            stop=not meta["has_bv"],
                            skip_group_check=True)
                        if meta["has_bv"]:
                            nc.tensor.matmul(
                                out=ps[:, 0:3 * H], lhsT=ones_t[:],
                                rhs=bv_t[l][:], start=False, stop=True,
                                skip_group_check=True)
                        nc.scalar.activation(st[:, s, :], ps[:, 0:2 * H],
                                             AF.Copy)
                        nc.scalar.activation(k_sb[:, gg, :],
                                             ps[:, 2 * H:3 * H], AF.Copy)
                    nc.sync.dma_start(
                        qv_loc[g * 128:(g + ns) * 128, :].rearrange(
                            "(s p) f -> p s f", p=128),
                        st[:, 0:ns, :])
                    g += ns

            def emit_ag(l, c):
                ins = qv_loc[0:C0N, :] if c == 0 else qv_loc[C0N:NPAD, :]
                nc.gpsimd.collective_compute(
                    "AllGather", OP.bypass, replica_groups=[list(range(R))],
                    ins=[ins], outs=[qv_tl[l][c][:]])

            def emit_spart(l, hsrc):
                hs = hs_pp[l % 2]
                for c0 in range(0, NPAD, 512):
                    csz = min(512, NPAD - c0)
                    pss = pnode.tile([128, 512], F32, tag="pn")
                    nc.tensor.matmul(out=pss[:, 0:csz], lhsT=Ws_t[l][:],
                                     rhs=hsrc[:, c0:c0 + csz],
                                     start=True, stop=True)
                    if meta["has_bs"]:
                        nc.scalar.activation(hs[:, c0:c0 + csz], pss[:, 0:csz],
                                             AF.Identity, bias=bs_t[l][:],
                                             scale=1.0)
                    else:
                        nc.scalar.activation(hs[:, c0:c0 + csz], pss[:, 0:csz],
                                             AF.Copy)

            def emit_leaky(l, c):
                sl = slice(0, C0N) if c == 0 else slice(C0N, NPAD)
                nc.vector.scalar_tensor_tensor(
                    out=h3_pp[l % 2][:, sl], in0=hs_pp[l % 2][:, sl],
                    scalar=float(SLOPE), in1=hs_pp[l % 2][:, sl],
                    op0=OP.mult, op1=OP.max)

            def emit_edges(e, l, g0, g1):
                nt_list = NT_L[e]
                off = OFF_L[e]
                choff = CHOFF_L[e]
                hs = hs_pp[l % 2]
                k_sb = k_pp[l % 2]
                tab = qv_tl[l][e]
                for g in range(g0, g1):
                    nt = int(nt_list[g])
                    if nt == 0:
                        continue
                    ne = nt * 128
                    o = int(off[g])
                    co = int(choff[g])
                    bt = sel.tile([128, TMAX, 256], FP8, tag="blob")
                    nc.sync.dma_start(
                        bt[:, 0:nt, :],
                        t_blob[e][:, o * 256:(o + nt) * 256].rearrange(
                            "p (t d) -> p t d", t=nt))
                    gt = edg.tile([128, TMAX, 2 * H], TDT, tag="g")
                    nreg = nc.gpsimd.value_load(
                        cnt_t[0:1, e * NGRP + g:e * NGRP + g + 1])
                    nc.gpsimd.dma_gather(
                        gt[:, 0:nt, :], tab[:],
                        gidx_t[e][:, o * 8:(o + nt) * 8],
                        ne, nreg, 2 * H, single_packet=(ne <= 512))
                    ktb = edm.tile([128, TMAX, 128], DT, tag="kt")
                    for ci in range((nt + KQC - 1) // KQC):
                        q0 = ci * KQC
                        qn = min(KQC, nt - q0)
                        pk = pkq.tile([128, KQC, 128], F32, tag="kq")
                        nc.tensor.matmul(
                            out=pk[:, 0:KQC, :],
                            lhsT=attr_t[e][0:5, (co + ci) * 128:(co + ci + 1) * 128],
                            rhs=BD_t[l][:], start=True, stop=False,
                            skip_group_check=True)
                        for t in range(q0, q0 + qn):
                            nc.tensor.matmul(
                                out=pk[:, t - q0, :],
                                lhsT=bt[:, t, 128:256],
                                rhs=k_sb[:, g, :], start=False, stop=False,
                                skip_group_check=True)
                        # q[src] folded into the same PSUM via identity matmul
                        nc.tensor.matmul(
                            out=pk[:, 0:qn, :], lhsT=ident[:],
                            rhs=gt[:, q0:q0 + qn, 0:H], start=False, stop=True,
                            skip_group_check=True)
                        nc.scalar.activation(ktb[:, q0:q0 + qn, :],
                                             pk[:, 0:qn, :], AF.Sigmoid)
                    dr = meta.get("dr_seg", False)
                    msg = edm.tile([128, TMAX, 128], FP8 if dr else DT, tag="msg")
                    nc.vector.tensor_tensor(out=msg[:, 0:nt, :],
                                            in0=ktb[:, 0:nt, :],
                                            in1=gt[:, 0:nt, H:2 * H], op=OP.mult)
                    pa = pedge.tile([128, 128], F32, tag="pa")
                    if dr:
                        npair = nt // 2
                        for t2 in range(0, npair * 2, 2):
                            nc.tensor.matmul(
                                out=pa[:], lhsT=msg[:, t2:t2 + 2, :],
                                rhs=bt[:, t2:t2 + 2, 0:128], start=(t2 == 0),
                                stop=(t2 + 2 == nt),
                                perf_mode=mybir.MatmulPerfMode.DoubleRow,
                                skip_group_check=True)
                        if nt % 2:
                            nc.tensor.matmul(out=pa[:], lhsT=msg[:, nt - 1, :],
                                             rhs=bt[:, nt - 1, 0:128],
                                             start=(nt == 1), stop=True,
                                             skip_group_check=True)
                    else:
                        for t in range(nt):
                            nc.tensor.matmul(out=pa[:], lhsT=msg[:, t, :],
                                             rhs=bt[:, t, 0:128], start=(t == 0),
                                             stop=(t == nt - 1))
                    nc.vector.tensor_tensor(
                        out=hs[:, g * 128:(g + 1) * 128],
                        in0=hs[:, g * 128:(g + 1) * 128], in1=pa[:], op=OP.add)

            # ---- layer-0 table from x: replicated input, so each rank builds
            # the FULL table locally (no collective, no exposed startup AG).
            def build_l0_chunk(c):
                g0, g1, tab, cbase = ((0, C0G, qv_tl[0][0], C0N) if c == 0
                                      else (C0G, NGRP, qv_tl[0][1], C1N))
                for rb in range(R):
                    g = g0
                    while g < g1:
                        ns = min(8, g1 - g)
                        lh = stg.tile([128, 1024], FP8, tag="lh")
                        nc.sync.dma_start(
                            lh[:, 0:ns * 128],
                            t_xrb[rb * 128:(rb + 1) * 128,
                                  g * 128:(g + ns) * 128])
                        st = stg.tile([128, 8, 2 * H], TDT, tag="st")
                        for s in range(0, ns, 2):
                            n2 = min(2, ns - s)
                            ps = pnode.tile([128, 512], F32, tag="pn")
                            for u in range(n2):
                                nc.tensor.matmul(
                                    out=ps[:, u * 256:u * 256 + 2 * H],
                                    lhsT=lh[:, (s + u) * 128:(s + u + 1) * 128],
                                    rhs=Wqvk_t[0][:, 0:2 * H], start=True,
                                    stop=not meta["has_bv"],
                                    skip_group_check=True)
                                if meta["has_bv"]:
                                    nc.tensor.matmul(
                                        out=ps[:, u * 256:u * 256 + 2 * H],
                                        lhsT=ones_t[:],
                                        rhs=bv_t[0][0:1, 0:2 * H],
                                        start=False, stop=True,
                                        skip_group_check=True)
                            if (s // 2) % 2 == 0:
                                nc.scalar.activation(st[:, s:s + n2, :],
                                                     ps[:, 0:n2 * 256], AF.Copy)
                            else:
                                nc.vector.tensor_copy(st[:, s:s + n2, :],
                                                      ps[:, 0:n2 * 256])
                        r0 = rb * cbase + (g - g0) * 128
                        nc.sync.dma_start(
                            tab[r0:r0 + ns * 128, :].rearrange(
                                "(s p) f -> p s f", p=128),
                            st[:, 0:ns, :])
                        g += ns
            build_l0_chunk(0)
            # local k table + s-part for layer 0 from the local x slice
            for g in range(NGRP):
                psk = pnode.tile([128, 512], F32, tag="pn")
                nc.tensor.matmul(out=psk[:, 0:H],
                                 lhsT=xT_t[:, g * 128:(g + 1) * 128],
                                 rhs=Wqvk_t[0][:, 2 * H:3 * H],
                                 start=True, stop=True, skip_group_check=True)
                nc.scalar.activation(k_pp[0][:, g, :], psk[:, 0:H], AF.Copy)
            emit_spart(0, xT_t)

            for _ in range(5):
                gz = edg.tile([128, TMAX, 2 * H], TDT, tag="g")
                nc.vector.memset(gz[:], 0.0)

            pp = ppool.tile([G, H], F32)
            indc = consts.tile([128, NGRP, G], DT)
            nc.sync.dma_start(
                indc[:], t_IndT[:].rearrange("(c p) g -> p c g", p=128))

            def emit_pool_part(c0g, c1g):
                for c in range(c0g, c1g):
                    trp = pedge.tile([128, 128], DT, tag="pa")
                    nc.tensor.transpose(out=trp[:],
                                        in_=h3f[:, c * 128:(c + 1) * 128],
                                        identity=ident[:])
                    hnode = tail.tile([128, 128], DT, tag="hnode")
                    nc.vector.tensor_copy(hnode[:], trp[:])
                    nc.tensor.matmul(out=pp[:],
                                     lhsT=indc[:, c, :],
                                     rhs=hnode[:],
                                     start=(c == 0), stop=(c == NGRP - 1))

            for l in range(NLAYER):
                hsrc_next = h3_pp[l % 2]
                if parts >= 2:
                    # full epoch-0 sweep first (its table is ready), then the
                    # first ep1 half; this keeps ep0 work ahead of any ep1
                    # stall on the second table chunk's AllGather.
                    if l == 0:
                        emit_edges(0, l, 0, 12)
                        build_l0_chunk(1)
                        emit_edges(0, l, 12, NGRP)
                    else:
                        emit_edges(0, l, 0, NGRP)
                    emit_edges(1, l, 0, C0G)
                # h3 chunk-0 final: start next layer's table chunk 0 (or pool)
                emit_leaky(l, 0)
                if l < NLAYER - 1:
                    build_chunk(l + 1, 0, hsrc_next)
                    emit_ag(l + 1, 0)
                elif parts >= 4:
                    emit_pool_part(0, C0G)
                if parts >= 2:
                    emit_edges(1, l, C0G, NGRP)
                if parts < 3:
                    break
                emit_leaky(l, 1)
                if l < NLAYER - 1:
                    build_chunk(l + 1, 1, hsrc_next)
                    emit_ag(l + 1, 1)
                    emit_spart(l + 1, hsrc_next)
                elif parts >= 4:
                    emit_pool_part(C0G, NGRP)

            if parts < 4:
                z_dbg = tail.tile([G, NCLS], F32, tag="zsb")
                nc.vector.tensor_copy(z_dbg[:], hs_pp[0][0:G, 0:NCLS])
                nc.sync.dma_start(t_out[:], z_dbg[:])
            else:
                pool_sb = tail.tile([G, H], F32, tag="poolsb")
                nc.vector.tensor_copy(pool_sb[:], pp[:])
                nc.sync.dma_start(pool_in[:], pool_sb[:])
                nc.gpsimd.collective_compute(
                    "AllGather", OP.bypass, replica_groups=[list(range(R))],
                    ins=[pool_in[:]], outs=[pool_out[:]])
                pr = tail.tile([G, R, H], F32, tag="pr")
                nc.sync.dma_start(pr[:], pool_out[:].rearrange("(r g) h -> g r h", r=R))
                pooled = tail.tile([G, H], F32, tag="pooled")
                nc.vector.tensor_tensor(out=pooled[:], in0=pr[:, 0, :], in1=pr[:, 1, :],
                                        op=OP.add)
                for r in range(2, R):
                    nc.vector.tensor_tensor(out=pooled[:], in0=pooled[:],
                                            in1=pr[:, r, :], op=OP.add)
                ptp = pedge.tile([H, G], F32, tag="pa")
                nc.tensor.transpose(out=ptp[:], in_=pooled[:], identity=identf[0:G, 0:G])
                pooledT = tail.tile([H, G], F32, tag="pooledT")
                nc.vector.tensor_copy(pooledT[:], ptp[:])
                zp = pedge.tile([G, NCLS], F32, tag="pa")
                nc.tensor.matmul(out=zp[:], lhsT=pooledT[:], rhs=Wch_t[:],
                                 start=True, stop=False)
                nc.tensor.matmul(out=zp[:], lhsT=clin_t[:], rhs=Wcc_t[:],
                                 start=False, stop=True)
                z_sb = tail.tile([G, NCLS], F32, tag="zsb")
                nc.vector.tensor_tensor(out=z_sb[:], in0=zp[:], in1=bc_t[:],
                                        op=OP.add)
                nc.sync.dma_start(t_out[:], z_sb[:])

    nc.compile()
    return nc


# ---------------------------------------------------------------------------

_CACHE = {}


def kernel(**inputs):
    in_maps, meta = prep(inputs)
    key = tuple(sorted((k, v) for k, v in meta.items()))
    if key not in _CACHE:
        _CACHE[key] = build(meta)
    nc = _CACHE[key]
    res = run_bass_kernel_spmd(nc, in_maps, list(range(R)))
    return np.asarray(res.results[0]["out"], np.float32)


def kernel_profiled(**inputs):
    """Like kernel() but also returns (exec_time_ns, trace_path)."""
    in_maps, meta = prep(inputs)
    key = tuple(sorted((k, v) for k, v in meta.items()))
    if key not in _CACHE:
        _CACHE[key] = build(meta)
    nc = _CACHE[key]
    res = run_bass_kernel_spmd(nc, in_maps, list(range(R)), trace=True)
    out = np.asarray(res.results[0]["out"], np.float32)
    trace_path = None
    if res.instructions_and_trace is not None:
        trace_path = res.instructions_and_trace[1]
    return out, res.exec_time_ns, trace_path


if __name__ == "__main__":
    pass


# revision 52
# speedup vs baseline: 1.7061x; 1.0412x over previous
"""Trainium2 Bass kernel for a 3-layer ResGatedGraphConv GNN (ClinicalGatedGCN).

Strategy (8 NeuronCores, SPMD), v3:
  - Nodes partitioned into 8 contiguous ranges of 6250 (padded to 6272 = 49
    groups of 128). Edges assigned to the rank owning their dst node, sorted
    by (src-chunk epoch, dst-group, dst) on the host.
  - The [q|v] node table is built DISTRIBUTED: each rank computes q,v (and k)
    only for its OWN nodes from its SBUF-resident h (one fused matmul per
    128-node group against [Wq|Wv|Wk]), stores the [q|v] rows to local DRAM,
    and AllGathers the table in two chunks (groups 0:24 -> qv_t0, 24:49 ->
    qv_t1; both tables stay under the int16 gather-index limit). This removes
    the per-layer h AllGather, all h re-reads, and ~7/8 of the table matmuls
    and PSUM->SBUF copies of the replicated design.
  - One dma_gather per (epoch, dst-group) fetches the src rows of [q|v].
    k[dst] is expanded on the PE via a host-shipped 0/1 selector ST; the
    segment-sum over dst is a PE matmul against selector S. S and ST are
    packed in ONE fp8 blob per (epoch, group) (0/1 is exact in fp8; matmul
    operands may mix fp8 with bf16), halving selector DMA traffic.
  - The edge-embedding rank-1 term (attr x We + gate bias) is batched: one
    K=5 matmul per 4-tile PSUM chunk against a block-diagonal [5, 4H]
    constant, with per-chunk attr columns shipped once as an fp8 constant.
  - BatchNorm folding: A = gamma/sqrt(var+eps) is folded into the NEXT
    layer's weights (and the classifier); B is folded into effective biases.
    The on-device h update is a single fused leaky-ReLU
    (scalar_tensor_tensor max(x, slope*x)).
  - Mean-pool per graph is a matmul against a host-built indicator with
    1/cnt folded in; partial pools are AllGather'd and summed; the tiny
    classifier runs on every core.
"""

import numpy as np
import ml_dtypes

import concourse.bacc as bacc
import concourse.bass as bass
import concourse.mybir as mybir
import concourse.tile as tile
from concourse.bass_utils import run_bass_kernel_spmd
from concourse.masks import make_identity

F32 = mybir.dt.float32
BF16 = mybir.dt.bfloat16
FP8 = mybir.dt.float8e4
I16 = mybir.dt.int16
AF = mybir.ActivationFunctionType
OP = mybir.AluOpType

# ---------------- problem constants (hardcoded per spec) ----------------
N, E, H, G, NCLIN, NCLS = 50000, 800000, 128, 64, 16, 2
NLAYER = 3
EPS = 1e-5
SLOPE = 0.01
R = 8                      # ranks / NeuronCores

NPR = (N + R - 1) // R     # real nodes per rank (6250)
NGRP = (NPR + 127) // 128  # 128-node groups per rank (49)
NPAD = NGRP * 128          # padded nodes per rank (6272)
C0G = 24                   # groups in AG chunk 0
C1G = NGRP - C0G           # groups in AG chunk 1 (25)
C0N = C0G * 128            # 3072 rows
C1N = C1G * 128            # 3200 rows
KQC = 4                    # psum chunk size in edge tiles (1 PSUM bank)


def wrap_idxs_block(idx):
    """Wrap one gather call's indices: idx j -> [j%16, j//16], tiled to 128 parts."""
    n = len(idx)
    assert n % 16 == 0
    w = np.asarray(idx, np.int16).reshape(n // 16, 16).T
    return np.tile(w, (8, 1))


# ---------------------------------------------------------------------------
# host-side preprocessing
# ---------------------------------------------------------------------------

def prep(inputs):
    x = np.asarray(inputs["x"], np.float32)
    edge_index = np.asarray(inputs["edge_index"])
    edge_attr = np.asarray(inputs["edge_attr"], np.float32)[:, 0]
    batch = np.asarray(inputs["batch"]).astype(np.int64)
    clinical = np.asarray(inputs["clinical"], np.float32)
    Wk, bk = np.asarray(inputs["Wk"], np.float32), np.asarray(inputs["bk"], np.float32)
    Wq, bq = np.asarray(inputs["Wq"], np.float32), np.asarray(inputs["bq"], np.float32)
    Wv, bv = np.asarray(inputs["Wv"], np.float32), np.asarray(inputs["bv"], np.float32)
    Ws, bs = np.asarray(inputs["Ws"], np.float32), np.asarray(inputs["bs"], np.float32)
    We, be = np.asarray(inputs["We"], np.float32), np.asarray(inputs["be"], np.float32)
    gamma = np.asarray(inputs["gamma"], np.float32)
    beta = np.asarray(inputs["beta"], np.float32)
    rmean = np.asarray(inputs["rmean"], np.float32)
    rvar = np.asarray(inputs["rvar"], np.float32)
    Wc, bc = np.asarray(inputs["Wc"], np.float32), np.asarray(inputs["bc"], np.float32)

    src = edge_index[0].astype(np.int64)
    dst = edge_index[1].astype(np.int64)

    # BN folded: true h_out = A*leaky(pre) + B; device h3 = leaky(pre).
    A = gamma / np.sqrt(rvar + EPS)          # [3, H]
    B = beta - rmean * A                     # [3, H]

    # effective weights: fold diag(A[l-1]) into layer-l input maps, and the
    # B[l-1] offset into layer-l biases.
    Wq_e = np.stack([Wq[l] * (A[l - 1][:, None] if l else 1.0) for l in range(NLAYER)])
    Wk_e = np.stack([Wk[l] * (A[l - 1][:, None] if l else 1.0) for l in range(NLAYER)])
    Wv_e = np.stack([Wv[l] * (A[l - 1][:, None] if l else 1.0) for l in range(NLAYER)])
    Ws_e = np.stack([Ws[l] * (A[l - 1][:, None] if l else 1.0) for l in range(NLAYER)])
    bgate = np.stack([bk[l] + bq[l] + be[l]
                      + (B[l - 1] @ (Wk[l] + Wq[l]) if l else 0.0)
                      for l in range(NLAYER)])
    bv_e = np.stack([bv[l] + (B[l - 1] @ Wv[l] if l else 0.0) for l in range(NLAYER)])
    bs_e = np.stack([bs[l] + (B[l - 1] @ Ws[l] if l else 0.0) for l in range(NLAYER)])
    Wc_h = Wc[0:H] * A[2][:, None]
    bc_e = bc + B[2] @ Wc[0:H]

    Wqvk = np.concatenate([Wq_e, Wv_e, Wk_e], axis=2)     # [3, H, 3H]

    # block-diagonal edge-term constant: rows 0..3 carry We on diag block,
    # row 4 carries the gate bias (broadcast to each tile block).
    BD = np.zeros((NLAYER, 5, KQC * H), np.float32)
    for l in range(NLAYER):
        for t in range(KQC):
            BD[l, t, t * H:(t + 1) * H] = We[l, 0, :]
            BD[l, 4, t * H:(t + 1) * H] = bgate[l]

    has_bv = bool(np.any(bv_e != 0))
    has_bs = bool(np.any(bs_e != 0))

    # ---- edge structure ----
    e_rank = dst // NPR
    dst_local = dst - e_rank * NPR
    grp = dst_local // 128
    drel = dst_local % 128
    rs = src // NPR
    lr = src - rs * NPR
    ep = (lr >= C0N).astype(np.int64)
    trow = np.where(ep == 0, rs * C0N + lr, rs * C1N + (lr - C0N))

    counts = np.zeros((2, R, NGRP), np.int64)
    np.add.at(counts, (ep, e_rank, grp), 1)
    nt_l = [np.ceil(counts[e].max(axis=0) / 128).astype(int) for e in (0, 1)]
    off_l = [np.concatenate([[0], np.cumsum(nt)]).astype(int) for nt in nt_l]
    nch_l = [np.ceil(nt / KQC).astype(int) for nt in nt_l]
    choff_l = [np.concatenate([[0], np.cumsum(nc)]).astype(int) for nc in nch_l]

    cntg = np.bincount(batch, minlength=G).astype(np.float32)
    inv_cnt = 1.0 / np.maximum(cntg, 1.0)

    order = np.lexsort((dst, grp, ep, e_rank))
    trow_s, drel_s, attr_s = trow[order], drel[order], edge_attr[order]
    key = (e_rank[order] * 2 + ep[order]) * NGRP + grp[order]
    starts = np.searchsorted(key, np.arange(R * 2 * NGRP + 1))

    in_maps = []
    for r in range(R):
        ep_arrs = {}
        for e in (0, 1):
            nt = nt_l[e]
            off = off_l[e]
            nch = nch_l[e]
            choff = choff_l[e]
            tot = int(off[-1])
            chtot = int(choff[-1])
            gidx = np.zeros((128, tot * 8), np.int16)
            blob = np.zeros((128, tot * 256), ml_dtypes.float8_e4m3)
            attr4 = np.zeros((5, chtot * 128), ml_dtypes.float8_e4m3)
            attr4[4, :] = 1.0
            for g in range(NGRP):
                ntg = int(nt[g])
                if ntg == 0:
                    continue
                k = (r * 2 + e) * NGRP + g
                s0 = int(starts[k])
                n = int(counts[e, r, g])
                o = int(off[g])
                co = int(choff[g])
                idx = np.full((ntg * 128,), -1, np.int64)
                idx[:n] = trow_s[s0:s0 + n]
                if n == 0:
                    idx[0] = 0
                gidx[:, o * 8:(o + ntg) * 8] = wrap_idxs_block(idx)
                j = np.arange(n)
                t = j // 128
                p = j % 128
                dr = drel_s[s0:s0 + n].astype(np.int64)
                # S: [p_edge, d] at cols (o+t)*256 + d
                blob[p, (o + t) * 256 + dr] = 1
                # ST: [drel, p_edge] at cols (o+t)*256 + 128 + p
                blob[dr, (o + t) * 256 + 128 + p] = 1
                attr4[t % KQC, (co + t // KQC) * 128 + p] = \
                    attr_s[s0:s0 + n].astype(ml_dtypes.float8_e4m3)
            ep_arrs[e] = (gidx, blob, attr4)
        IndT = np.zeros((NPAD, G), np.float32)
        lo, hi = r * NPR, min((r + 1) * NPR, N)
        IndT[np.arange(hi - lo), batch[lo:hi]] = inv_cnt[batch[lo:hi]]
        xT_loc = np.zeros((128, NPAD), np.float32)
        xT_loc[:, 0:hi - lo] = x[lo:hi].T
        x_rb = np.zeros((R * 128, NPAD), np.float32)
        for rr in range(R):
            rlo, rhi = rr * NPR, min((rr + 1) * NPR, N)
            x_rb[rr * 128:(rr + 1) * 128, 0:rhi - rlo] = x[rlo:rhi].T
        gcnt = np.maximum(counts[:, r, :], 1).astype(np.int32).reshape(1, 2 * NGRP)
        im = {
            "gcnt": gcnt,
            "xT_loc": xT_loc.astype(ml_dtypes.float8_e4m3),
            "x_rb": x_rb.astype(ml_dtypes.float8_e4m3),
            "Wqvk": Wqvk.astype(ml_dtypes.bfloat16),
            "Ws": Ws_e.astype(ml_dtypes.bfloat16),
            "BD": BD.astype(ml_dtypes.bfloat16),
            "bv_row": np.concatenate(
                [np.zeros((NLAYER, 1, H), np.float32),
                 bv_e.reshape(NLAYER, 1, H),
                 np.zeros((NLAYER, 1, H), np.float32)], axis=2
            ).astype(ml_dtypes.bfloat16),
            "bs_col": bs_e.reshape(NLAYER, H, 1),
            "gidx0": ep_arrs[0][0], "blob0": ep_arrs[0][1], "attr0": ep_arrs[0][2],
            "gidx1": ep_arrs[1][0], "blob1": ep_arrs[1][1], "attr1": ep_arrs[1][2],
            "IndT": IndT.astype(ml_dtypes.bfloat16),
            "clinT": clinical.T.copy(),
            "Wc_h": Wc_h, "Wc_c": Wc[H:H + NCLIN],
            "bc_rep": np.tile(bc_e, (G, 1)),
        }
        in_maps.append(im)
    meta = dict(NT0=tuple(int(v) for v in nt_l[0]),
                NT1=tuple(int(v) for v in nt_l[1]),
                has_bv=has_bv, has_bs=has_bs, tab_fp8=True, dr_seg=True)
    return in_maps, meta


# ---------------------------------------------------------------------------
# device program
# ---------------------------------------------------------------------------

def build(meta):
    parts = meta.get("parts", 4)
    DT = BF16
    TDT = FP8 if meta.get("tab_fp8", True) else BF16
    NT_L = [list(meta["NT0"]), list(meta["NT1"])]
    OFF_L = [np.concatenate([[0], np.cumsum(nt)]).astype(int) for nt in NT_L]
    NCH_L = [np.ceil(np.array(nt) / KQC).astype(int) for nt in NT_L]
    CHOFF_L = [np.concatenate([[0], np.cumsum(nc)]).astype(int) for nc in NCH_L]
    TOT = [int(o[-1]) for o in OFF_L]
    CHTOT = [int(c[-1]) for c in CHOFF_L]
    TMAX = max(max(NT_L[0]), max(NT_L[1]))

    nc = bacc.Bacc("TRN2", target_bir_lowering=False, debug=False, num_devices=R)

    def din(name, shape, dt):
        return nc.dram_tensor(name, shape, dt, kind="ExternalInput").ap()

    t_xT = din("xT_loc", [128, NPAD], FP8)
    t_cnt = din("gcnt", [1, 2 * NGRP], mybir.dt.int32)
    t_xrb = din("x_rb", [R * 128, NPAD], FP8)
    t_Wqvk = din("Wqvk", [NLAYER, H, 3 * H], DT)
    t_Ws = din("Ws", [NLAYER, H, H], DT)
    t_BD = din("BD", [NLAYER, 5, KQC * H], DT)
    t_bv = din("bv_row", [NLAYER, 1, 3 * H], DT)
    t_bs = din("bs_col", [NLAYER, H, 1], F32)
    t_gidx = [din("gidx0", [128, TOT[0] * 8], I16),
              din("gidx1", [128, TOT[1] * 8], I16)]
    t_blob = [din("blob0", [128, TOT[0] * 256], FP8),
              din("blob1", [128, TOT[1] * 256], FP8)]
    t_attr = [din("attr0", [5, CHTOT[0] * 128], FP8),
              din("attr1", [5, CHTOT[1] * 128], FP8)]
    t_IndT = din("IndT", [NPAD, G], DT)
    t_clinT = din("clinT", [NCLIN, G], F32)
    t_Wc_h = din("Wc_h", [H, NCLS], F32)
    t_Wc_c = din("Wc_c", [NCLIN, NCLS], F32)
    t_bc = din("bc_rep", [G, NCLS], F32)

    t_out = nc.dram_tensor("out", [G, NCLS], F32, kind="ExternalOutput").ap()

    qv_loc = nc.dram_tensor("qv_loc", [NPAD, 2 * H], TDT).ap()
    # per-layer AG'd table pairs (separate per layer so the next layer's
    # AllGather never overwrites a table the current layer still gathers from)
    qv_tl = [None,
             [nc.dram_tensor("qv1_t0", [R * C0N, 2 * H], TDT, addr_space="Shared").ap(),
              nc.dram_tensor("qv1_t1", [R * C1N, 2 * H], TDT, addr_space="Shared").ap()],
             [nc.dram_tensor("qv2_t0", [R * C0N, 2 * H], TDT, addr_space="Shared").ap(),
              nc.dram_tensor("qv2_t1", [R * C1N, 2 * H], TDT, addr_space="Shared").ap()]]
    # layer-0 tables are built locally (x is replicated), no collective
    qv_tl[0] = [nc.dram_tensor("qv0_t0", [R * C0N, 2 * H], TDT).ap(),
                nc.dram_tensor("qv0_t1", [R * C1N, 2 * H], TDT).ap()]
    pool_in = nc.dram_tensor("pool_in", [G, H], F32).ap()
    pool_out = nc.dram_tensor("pool_out", [R * G, H], F32, addr_space="Shared").ap()

    with tile.TileContext(nc) as tc:
        import contextlib
        with contextlib.ExitStack() as ctx:
            consts = ctx.enter_context(tc.tile_pool(name="consts", bufs=1))
            hsb = ctx.enter_context(tc.tile_pool(name="hsb", bufs=1))
            h3p = ctx.enter_context(tc.tile_pool(name="h3p", bufs=1))
            ksb = ctx.enter_context(tc.tile_pool(name="ksb", bufs=1))
            stg = ctx.enter_context(tc.tile_pool(name="stg", bufs=4))
            tail = ctx.enter_context(tc.tile_pool(name="tail", bufs=2))
            edg = ctx.enter_context(tc.tile_pool(name="edg", bufs=5))
            sel = ctx.enter_context(tc.tile_pool(name="sel", bufs=8))
            edm = ctx.enter_context(tc.tile_pool(name="edm", bufs=3))
            pnode = ctx.enter_context(tc.tile_pool(name="pnode", bufs=2, space="PSUM"))
            pkq = ctx.enter_context(tc.tile_pool(name="pkq", bufs=2, space="PSUM"))
            pedge = ctx.enter_context(tc.tile_pool(name="pedge", bufs=2, space="PSUM"))
            ppool = ctx.enter_context(tc.tile_pool(name="ppool", bufs=1, space="PSUM"))

            _cid = [0]

            def load_const(src_ap, shape, dt):
                _cid[0] += 1
                t = consts.tile(shape, dt, tag=f"c{_cid[0]}_{src_ap.tensor.name}")
                nc.sync.dma_start(t[:], src_ap)
                return t

            Wqvk_t = [load_const(t_Wqvk[l], [H, 3 * H], DT) for l in range(NLAYER)]
            Ws_t = [load_const(t_Ws[l], [H, H], DT) for l in range(NLAYER)]
            BD_t = [load_const(t_BD[l], [5, KQC * H], DT) for l in range(NLAYER)]
            bv_t = ([load_const(t_bv[l], [1, 3 * H], DT) for l in range(NLAYER)]
                    if meta["has_bv"] else None)
            bs_t = ([load_const(t_bs[l], [H, 1], F32) for l in range(NLAYER)]
                    if meta["has_bs"] else None)
            ones_t = consts.tile([1, 128], DT)
            nc.vector.memset(ones_t[:], 1.0)
            gidx_t = [load_const(t_gidx[0], [128, TOT[0] * 8], I16),
                      load_const(t_gidx[1], [128, TOT[1] * 8], I16)]
            attr_t = [load_const(t_attr[0], [5, CHTOT[0] * 128], FP8),
                      load_const(t_attr[1], [5, CHTOT[1] * 128], FP8)]
            xT_t = load_const(t_xT, [128, NPAD], FP8)
            cnt_t = load_const(t_cnt, [1, 2 * NGRP], mybir.dt.int32)
            ident = consts.tile([128, 128], DT)
            make_identity(nc, ident[:])
            identf = consts.tile([128, 128], F32)
            make_identity(nc, identf[:])
            clin_t = load_const(t_clinT, [NCLIN, G], F32)
            Wch_t = load_const(t_Wc_h, [H, NCLS], F32)
            Wcc_t = load_const(t_Wc_c, [NCLIN, NCLS], F32)
            bc_t = load_const(t_bc, [G, NCLS], F32)

            # ping-pong state by layer parity
            hs_pp = [hsb.tile([128, NPAD], DT, tag="hs0", name="hs0"),
                     hsb.tile([128, NPAD], DT, tag="hs1", name="hs1")]
            k_pp = [ksb.tile([128, NGRP, H], DT, tag="k0", name="k0"),
                    ksb.tile([128, NGRP, H], DT, tag="k1", name="k1")]
            h3_pp = [h3p.tile([128, NPAD], DT, tag="h3a", name="h3a"),
                     h3p.tile([128, NPAD], DT, tag="h3b", name="h3b")]
            h3f = h3_pp[0]                             # layer-2 output parity

            def build_chunk(l, c, hsrc):
                """Local q|v|k for groups of AG-chunk c of layer l."""
                g0, g1 = (0, C0G) if c == 0 else (C0G, NGRP)
                k_sb = k_pp[l % 2]
                g = g0
                while g < g1:
                    ns = min(8, g1 - g)
                    st = stg.tile([128, 8, 2 * H], TDT, tag="st")
                    for s in range(ns):
                        gg = g + s
                        ps = pnode.tile([128, 512], F32, tag="pn")
                        nc.tensor.matmul(
                            out=ps[:, 0:3 * H],
                            lhsT=hsrc[:, gg * 128:(gg + 1) * 128],
                            rhs=Wqvk_t[l][:], start=True,
                            stop=not meta["has_bv"],
                            skip_group_check=True)
                        if meta["has_bv"]:
                            nc.tensor.matmul(
                                out=ps[:, 0:3 * H], lhsT=ones_t[:],
                                rhs=bv_t[l][:], start=False, stop=True,
                                skip_group_check=True)
                        nc.scalar.activation(st[:, s, :], ps[:, 0:2 * H],
                                             AF.Copy)
                        nc.scalar.activation(k_sb[:, gg, :],
                                             ps[:, 2 * H:3 * H], AF.Copy)
                    nc.sync.dma_start(
                        qv_loc[g * 128:(g + ns) * 128, :].rearrange(
                            "(s p) f -> p s f", p=128),
                        st[:, 0:ns, :])
                    g += ns

            def emit_ag(l, c):
                ins = qv_loc[0:C0N, :] if c == 0 else qv_loc[C0N:NPAD, :]
                nc.gpsimd.collective_compute(
                    "AllGather", OP.bypass, replica_groups=[list(range(R))],
                    ins=[ins], outs=[qv_tl[l][c][:]])

            def emit_spart(l, hsrc):
                hs = hs_pp[l % 2]
                for c0 in range(0, NPAD, 512):
                    csz = min(512, NPAD - c0)
                    pss = pnode.tile([128, 512], F32, tag="pn")
                    nc.tensor.matmul(out=pss[:, 0:csz], lhsT=Ws_t[l][:],
                                     rhs=hsrc[:, c0:c0 + csz],
                                     start=True, stop=True)
                    if meta["has_bs"]:
                        nc.scalar.activation(hs[:, c0:c0 + csz], pss[:, 0:csz],
                                             AF.Identity, bias=bs_t[l][:],
                                             scale=1.0)
                    else:
                        nc.scalar.activation(hs[:, c0:c0 + csz], pss[:, 0:csz],
                                             AF.Copy)

            def emit_leaky(l, c):
                sl = slice(0, C0N) if c == 0 else slice(C0N, NPAD)
                nc.vector.scalar_tensor_tensor(
                    out=h3_pp[l % 2][:, sl], in0=hs_pp[l % 2][:, sl],
                    scalar=float(SLOPE), in1=hs_pp[l % 2][:, sl],
                    op0=OP.mult, op1=OP.max)

            def emit_edges(e, l, g0, g1):
                nt_list = NT_L[e]
                off = OFF_L[e]
                choff = CHOFF_L[e]
                hs = hs_pp[l % 2]
                k_sb = k_pp[l % 2]
                tab = qv_tl[l][e]
                for g in range(g0, g1):
                    nt = int(nt_list[g])
                    if nt == 0:
                        continue
                    ne = nt * 128
                    o = int(off[g])
                    co = int(choff[g])
                    bt = sel.tile([128, TMAX, 256], FP8, tag="blob")
                    nc.sync.dma_start(
                        bt[:, 0:nt, :],
                        t_blob[e][:, o * 256:(o + nt) * 256].rearrange(
                            "p (t d) -> p t d", t=nt))
                    gt = edg.tile([128, TMAX, 2 * H], TDT, tag="g")
                    nreg = nc.gpsimd.value_load(
                        cnt_t[0:1, e * NGRP + g:e * NGRP + g + 1])
                    nc.gpsimd.dma_gather(
                        gt[:, 0:nt, :], tab[:],
                        gidx_t[e][:, o * 8:(o + nt) * 8],
                        ne, nreg, 2 * H, single_packet=(ne <= 512))
                    ktb = edm.tile([128, TMAX, 128], DT, tag="kt")
                    for ci in range((nt + KQC - 1) // KQC):
                        q0 = ci * KQC
                        qn = min(KQC, nt - q0)
                        pk = pkq.tile([128, KQC, 128], F32, tag="kq")
                        nc.tensor.matmul(
                            out=pk[:, 0:KQC, :],
                            lhsT=attr_t[e][0:5, (co + ci) * 128:(co + ci + 1) * 128],
                            rhs=BD_t[l][:], start=True, stop=False,
                            skip_group_check=True)
                        for t in range(q0, q0 + qn):
                            nc.tensor.matmul(
                                out=pk[:, t - q0, :],
                                lhsT=bt[:, t, 128:256],
                                rhs=k_sb[:, g, :], start=False, stop=False,
                                skip_group_check=True)
                        # q[src] folded into the same PSUM via identity matmul
                        nc.tensor.matmul(
                            out=pk[:, 0:qn, :], lhsT=ident[:],
                            rhs=gt[:, q0:q0 + qn, 0:H], start=False, stop=True,
                            skip_group_check=True)
                        nc.scalar.activation(ktb[:, q0:q0 + qn, :],
                                             pk[:, 0:qn, :], AF.Sigmoid)
                    dr = meta.get("dr_seg", False)
                    msg = edm.tile([128, TMAX, 128], FP8 if dr else DT, tag="msg")
                    nc.vector.tensor_tensor(out=msg[:, 0:nt, :],
                                            in0=ktb[:, 0:nt, :],
                                            in1=gt[:, 0:nt, H:2 * H], op=OP.mult)
                    pa = pedge.tile([128, 128], F32, tag="pa")
                    if dr:
                        npair = nt // 2
                        for t2 in range(0, npair * 2, 2):
                            nc.tensor.matmul(
                                out=pa[:], lhsT=msg[:, t2:t2 + 2, :],
                                rhs=bt[:, t2:t2 + 2, 0:128], start=(t2 == 0),
                                stop=(t2 + 2 == nt),
                                perf_mode=mybir.MatmulPerfMode.DoubleRow,
                                skip_group_check=True)
                        if nt % 2:
                            nc.tensor.matmul(out=pa[:], lhsT=msg[:, nt - 1, :],
                                             rhs=bt[:, nt - 1, 0:128],
                                             start=(nt == 1), stop=True,
                                             skip_group_check=True)
                    else:
                        for t in range(nt):
                            nc.tensor.matmul(out=pa[:], lhsT=msg[:, t, :],
                                             rhs=bt[:, t, 0:128], start=(t == 0),
                                             stop=(t == nt - 1))
                    nc.vector.tensor_tensor(
                        out=hs[:, g * 128:(g + 1) * 128],
                        in0=hs[:, g * 128:(g + 1) * 128], in1=pa[:], op=OP.add)

            # ---- layer-0 table from x: replicated input, so each rank builds
            # the FULL table locally (no collective, no exposed startup AG).
            def build_l0_chunk(c):
                g0, g1, tab, cbase = ((0, C0G, qv_tl[0][0], C0N) if c == 0
                                      else (C0G, NGRP, qv_tl[0][1], C1N))
                for rb in range(R):
                    g = g0
                    while g < g1:
                        ns = min(8, g1 - g)
                        lh = stg.tile([128, 1024], FP8, tag="lh")
                        nc.sync.dma_start(
                            lh[:, 0:ns * 128],
                            t_xrb[rb * 128:(rb + 1) * 128,
                                  g * 128:(g + ns) * 128])
                        st = stg.tile([128, 8, 2 * H], TDT, tag="st")
                        for s in range(0, ns, 2):
                            n2 = min(2, ns - s)
                            ps = pnode.tile([128, 512], F32, tag="pn")
                            for u in range(n2):
                                nc.tensor.matmul(
                                    out=ps[:, u * 256:u * 256 + 2 * H],
                                    lhsT=lh[:, (s + u) * 128:(s + u + 1) * 128],
                                    rhs=Wqvk_t[0][:, 0:2 * H], start=True,
                                    stop=not meta["has_bv"],
                                    skip_group_check=True)
                                if meta["has_bv"]:
                                    nc.tensor.matmul(
                                        out=ps[:, u * 256:u * 256 + 2 * H],
                                        lhsT=ones_t[:],
                                        rhs=bv_t[0][0:1, 0:2 * H],
                                        start=False, stop=True,
                                        skip_group_check=True)
                            if (s // 2) % 2 == 0:
                                nc.scalar.activation(st[:, s:s + n2, :],
                                                     ps[:, 0:n2 * 256], AF.Copy)
                            else:
                                nc.vector.tensor_copy(st[:, s:s + n2, :],
                                                      ps[:, 0:n2 * 256])
                        r0 = rb * cbase + (g - g0) * 128
                        nc.sync.dma_start(
                            tab[r0:r0 + ns * 128, :].rearrange(
                                "(s p) f -> p s f", p=128),
                            st[:, 0:ns, :])
                        g += ns
            build_l0_chunk(0)
            # local k table + s-part for layer 0 from the local x slice
            for g in range(NGRP):
                psk = pnode.tile([128, 512], F32, tag="pn")
                nc.tensor.matmul(out=psk[:, 0:H],
                                 lhsT=xT_t[:, g * 128:(g + 1) * 128],
                                 rhs=Wqvk_t[0][:, 2 * H:3 * H],
                                 start=True, stop=True, skip_group_check=True)
                nc.scalar.activation(k_pp[0][:, g, :], psk[:, 0:H], AF.Copy)
            emit_spart(0, xT_t)

            for _ in range(5):
                gz = edg.tile([128, TMAX, 2 * H], TDT, tag="g")
                nc.vector.memset(gz[:], 0.0)

            pp = ppool.tile([G, H], F32)
            indc = consts.tile([128, NGRP, G], DT)
            nc.sync.dma_start(
                indc[:], t_IndT[:].rearrange("(c p) g -> p c g", p=128))

            def emit_pool_part(c0g, c1g):
                for c in range(c0g, c1g):
                    trp = pedge.tile([128, 128], DT, tag="pa")
                    nc.tensor.transpose(out=trp[:],
                                        in_=h3f[:, c * 128:(c + 1) * 128],
                                        identity=ident[:])
                    hnode = tail.tile([128, 128], DT, tag="hnode")
                    nc.vector.tensor_copy(hnode[:], trp[:])
                    nc.tensor.matmul(out=pp[:],
                                     lhsT=indc[:, c, :],
                                     rhs=hnode[:],
                                     start=(c == 0), stop=(c == NGRP - 1))

            for l in range(NLAYER):
                hsrc_next = h3_pp[l % 2]
                if parts >= 2:
                    if l == 0:
                        # layer 0 consumes no AllGather (local tables), so run
                        # both epochs' first halves up front and fire the hook
                        # at ~50% -- the next layer's AG0 starts much earlier.
                        emit_edges(0, l, 0, 12)
                        build_l0_chunk(1)
                        emit_edges(0, l, 12, C0G)
                        emit_edges(1, l, 0, C0G)
                    else:
                        # full epoch-0 sweep first (its table is ready), then
                        # the first ep1 half; this keeps ep0 work ahead of any
                        # ep1 stall on the second table chunk's AllGather.
                        emit_edges(0, l, 0, NGRP)
                        emit_edges(1, l, 0, C0G)
                # h3 chunk-0 final: start next layer's table chunk 0 (or pool)
                emit_leaky(l, 0)
                if l < NLAYER - 1:
                    build_chunk(l + 1, 0, hsrc_next)
                    emit_ag(l + 1, 0)
                elif parts >= 4:
                    emit_pool_part(0, C0G)
                if parts >= 2:
                    if l == 0:
                        emit_edges(0, l, C0G, NGRP)
                    emit_edges(1, l, C0G, NGRP)
                if parts < 3:
                    break
                emit_leaky(l, 1)
                if l < NLAYER - 1:
                    build_chunk(l + 1, 1, hsrc_next)
                    emit_ag(l + 1, 1)
                    emit_spart(l + 1, hsrc_next)
                elif parts >= 4:
                    emit_pool_part(C0G, NGRP)
            if parts < 4:
                z_dbg = tail.tile([G, NCLS], F32, tag="zsb")
                nc.vector.tensor_copy(z_dbg[:], hs_pp[0][0:G, 0:NCLS])
                nc.sync.dma_start(t_out[:], z_dbg[:])
            else:
                pool_sb = tail.tile([G, H], F32, tag="poolsb")
                nc.vector.tensor_copy(pool_sb[:], pp[:])
                nc.sync.dma_start(pool_in[:], pool_sb[:])
                nc.gpsimd.collective_compute(
                    "AllGather", OP.bypass, replica_groups=[list(range(R))],
                    ins=[pool_in[:]], outs=[pool_out[:]])
                pr = tail.tile([G, R, H], F32, tag="pr")
                nc.sync.dma_start(pr[:], pool_out[:].rearrange("(r g) h -> g r h", r=R))
                pooled = tail.tile([G, H], F32, tag="pooled")
                nc.vector.tensor_tensor(out=pooled[:], in0=pr[:, 0, :], in1=pr[:, 1, :],
                                        op=OP.add)
                for r in range(2, R):
                    nc.vector.tensor_tensor(out=pooled[:], in0=pooled[:],
                                            in1=pr[:, r, :], op=OP.add)
                ptp = pedge.tile([H, G], F32, tag="pa")
                nc.tensor.transpose(out=ptp[:], in_=pooled[:], identity=identf[0:G, 0:G])
                pooledT = tail.tile([H, G], F32, tag="pooledT")
                nc.vector.tensor_copy(pooledT[:], ptp[:])
                zp = pedge.tile([G, NCLS], F32, tag="pa")
                nc.tensor.matmul(out=zp[:], lhsT=pooledT[:], rhs=Wch_t[:],
                                 start=True, stop=False)
                nc.tensor.matmul(out=zp[:], lhsT=clin_t[:], rhs=Wcc_t[:],
                                 start=False, stop=True)
                z_sb = tail.tile([G, NCLS], F32, tag="zsb")
                nc.vector.tensor_tensor(out=z_sb[:], in0=zp[:], in1=bc_t[:],
                                        op=OP.add)
                nc.sync.dma_start(t_out[:], z_sb[:])

    nc.compile()
    return nc


# ---------------------------------------------------------------------------

_CACHE = {}


def kernel(**inputs):
    in_maps, meta = prep(inputs)
    key = tuple(sorted((k, v) for k, v in meta.items()))
    if key not in _CACHE:
        _CACHE[key] = build(meta)
    nc = _CACHE[key]
    res = run_bass_kernel_spmd(nc, in_maps, list(range(R)))
    return np.asarray(res.results[0]["out"], np.float32)


def kernel_profiled(**inputs):
    """Like kernel() but also returns (exec_time_ns, trace_path)."""
    in_maps, meta = prep(inputs)
    key = tuple(sorted((k, v) for k, v in meta.items()))
    if key not in _CACHE:
        _CACHE[key] = build(meta)
    nc = _CACHE[key]
    res = run_bass_kernel_spmd(nc, in_maps, list(range(R)), trace=True)
    out = np.asarray(res.results[0]["out"], np.float32)
    trace_path = None
    if res.instructions_and_trace is not None:
        trace_path = res.instructions_and_trace[1]
    return out, res.exec_time_ns, trace_path


if __name__ == "__main__":
    pass


# revision 53
# speedup vs baseline: 1.8168x; 1.0649x over previous
"""Trainium2 Bass kernel for a 3-layer ResGatedGraphConv GNN (ClinicalGatedGCN).

Strategy (8 NeuronCores, SPMD), v3:
  - Nodes partitioned into 8 contiguous ranges of 6250 (padded to 6272 = 49
    groups of 128). Edges assigned to the rank owning their dst node, sorted
    by (src-chunk epoch, dst-group, dst) on the host.
  - The [q|v] node table is built DISTRIBUTED: each rank computes q,v (and k)
    only for its OWN nodes from its SBUF-resident h (one fused matmul per
    128-node group against [Wq|Wv|Wk]), stores the [q|v] rows to local DRAM,
    and AllGathers the table in two chunks (groups 0:24 -> qv_t0, 24:49 ->
    qv_t1; both tables stay under the int16 gather-index limit). This removes
    the per-layer h AllGather, all h re-reads, and ~7/8 of the table matmuls
    and PSUM->SBUF copies of the replicated design.
  - One dma_gather per (epoch, dst-group) fetches the src rows of [q|v].
    k[dst] is expanded on the PE via a host-shipped 0/1 selector ST; the
    segment-sum over dst is a PE matmul against selector S. S and ST are
    packed in ONE fp8 blob per (epoch, group) (0/1 is exact in fp8; matmul
    operands may mix fp8 with bf16), halving selector DMA traffic.
  - The edge-embedding rank-1 term (attr x We + gate bias) is batched: one
    K=5 matmul per 4-tile PSUM chunk against a block-diagonal [5, 4H]
    constant, with per-chunk attr columns shipped once as an fp8 constant.
  - BatchNorm folding: A = gamma/sqrt(var+eps) is folded into the NEXT
    layer's weights (and the classifier); B is folded into effective biases.
    The on-device h update is a single fused leaky-ReLU
    (scalar_tensor_tensor max(x, slope*x)).
  - Mean-pool per graph is a matmul against a host-built indicator with
    1/cnt folded in; partial pools are AllGather'd and summed; the tiny
    classifier runs on every core.
"""

import numpy as np
import ml_dtypes

import concourse.bacc as bacc
import concourse.bass as bass
import concourse.mybir as mybir
import concourse.tile as tile
from concourse.bass_utils import run_bass_kernel_spmd
from concourse.masks import make_identity

F32 = mybir.dt.float32
BF16 = mybir.dt.bfloat16
FP8 = mybir.dt.float8e4
I16 = mybir.dt.int16
AF = mybir.ActivationFunctionType
OP = mybir.AluOpType

# ---------------- problem constants (hardcoded per spec) ----------------
N, E, H, G, NCLIN, NCLS = 50000, 800000, 128, 64, 16, 2
NLAYER = 3
EPS = 1e-5
SLOPE = 0.01
R = 8                      # ranks / NeuronCores

NPR = (N + R - 1) // R     # real nodes per rank (6250)
NGRP = (NPR + 127) // 128  # 128-node groups per rank (49)
NPAD = NGRP * 128          # padded nodes per rank (6272)
C0G = 24                   # groups in AG chunk 0
C1G = NGRP - C0G           # groups in AG chunk 1 (25)
C0N = C0G * 128            # 3072 rows
C1N = C1G * 128            # 3200 rows
KQC = 4                    # psum chunk size in edge tiles (1 PSUM bank)


def wrap_idxs_block(idx):
    """Wrap one gather call's indices: idx j -> [j%16, j//16], tiled to 128 parts."""
    n = len(idx)
    assert n % 16 == 0
    w = np.asarray(idx, np.int16).reshape(n // 16, 16).T
    return np.tile(w, (8, 1))


# ---------------------------------------------------------------------------
# host-side preprocessing
# ---------------------------------------------------------------------------

def prep(inputs):
    x = np.asarray(inputs["x"], np.float32)
    edge_index = np.asarray(inputs["edge_index"])
    edge_attr = np.asarray(inputs["edge_attr"], np.float32)[:, 0]
    batch = np.asarray(inputs["batch"]).astype(np.int64)
    clinical = np.asarray(inputs["clinical"], np.float32)
    Wk, bk = np.asarray(inputs["Wk"], np.float32), np.asarray(inputs["bk"], np.float32)
    Wq, bq = np.asarray(inputs["Wq"], np.float32), np.asarray(inputs["bq"], np.float32)
    Wv, bv = np.asarray(inputs["Wv"], np.float32), np.asarray(inputs["bv"], np.float32)
    Ws, bs = np.asarray(inputs["Ws"], np.float32), np.asarray(inputs["bs"], np.float32)
    We, be = np.asarray(inputs["We"], np.float32), np.asarray(inputs["be"], np.float32)
    gamma = np.asarray(inputs["gamma"], np.float32)
    beta = np.asarray(inputs["beta"], np.float32)
    rmean = np.asarray(inputs["rmean"], np.float32)
    rvar = np.asarray(inputs["rvar"], np.float32)
    Wc, bc = np.asarray(inputs["Wc"], np.float32), np.asarray(inputs["bc"], np.float32)

    src = edge_index[0].astype(np.int64)
    dst = edge_index[1].astype(np.int64)

    # BN folded: true h_out = A*leaky(pre) + B; device h3 = leaky(pre).
    A = gamma / np.sqrt(rvar + EPS)          # [3, H]
    B = beta - rmean * A                     # [3, H]

    # effective weights: fold diag(A[l-1]) into layer-l input maps, and the
    # B[l-1] offset into layer-l biases.
    Wq_e = np.stack([Wq[l] * (A[l - 1][:, None] if l else 1.0) for l in range(NLAYER)])
    Wk_e = np.stack([Wk[l] * (A[l - 1][:, None] if l else 1.0) for l in range(NLAYER)])
    Wv_e = np.stack([Wv[l] * (A[l - 1][:, None] if l else 1.0) for l in range(NLAYER)])
    Ws_e = np.stack([Ws[l] * (A[l - 1][:, None] if l else 1.0) for l in range(NLAYER)])
    bgate = np.stack([bk[l] + bq[l] + be[l]
                      + (B[l - 1] @ (Wk[l] + Wq[l]) if l else 0.0)
                      for l in range(NLAYER)])
    bv_e = np.stack([bv[l] + (B[l - 1] @ Wv[l] if l else 0.0) for l in range(NLAYER)])
    bs_e = np.stack([bs[l] + (B[l - 1] @ Ws[l] if l else 0.0) for l in range(NLAYER)])
    Wc_h = Wc[0:H] * A[2][:, None]
    bc_e = bc + B[2] @ Wc[0:H]

    Wqvk = np.concatenate([Wq_e, Wv_e, Wk_e], axis=2)     # [3, H, 3H]

    # block-diagonal edge-term constant: rows 0..3 carry We on diag block,
    # row 4 carries the gate bias (broadcast to each tile block).
    BD = np.zeros((NLAYER, 5, KQC * H), np.float32)
    for l in range(NLAYER):
        for t in range(KQC):
            BD[l, t, t * H:(t + 1) * H] = We[l, 0, :]
            BD[l, 4, t * H:(t + 1) * H] = bgate[l]

    has_bv = bool(np.any(bv_e != 0))
    has_bs = bool(np.any(bs_e != 0))

    # ---- edge structure ----
    e_rank = dst // NPR
    dst_local = dst - e_rank * NPR
    grp = dst_local // 128
    drel = dst_local % 128
    rs = src // NPR
    lr = src - rs * NPR
    ep = (lr >= C0N).astype(np.int64)
    trow = np.where(ep == 0, rs * C0N + lr, rs * C1N + (lr - C0N))

    counts = np.zeros((2, R, NGRP), np.int64)
    np.add.at(counts, (ep, e_rank, grp), 1)
    nt_l = [np.ceil(counts[e].max(axis=0) / 128).astype(int) for e in (0, 1)]
    off_l = [np.concatenate([[0], np.cumsum(nt)]).astype(int) for nt in nt_l]
    nch_l = [np.ceil(nt / KQC).astype(int) for nt in nt_l]
    choff_l = [np.concatenate([[0], np.cumsum(nc)]).astype(int) for nc in nch_l]

    cntg = np.bincount(batch, minlength=G).astype(np.float32)
    inv_cnt = 1.0 / np.maximum(cntg, 1.0)

    order = np.lexsort((dst, grp, ep, e_rank))
    trow_s, drel_s, attr_s = trow[order], drel[order], edge_attr[order]
    key = (e_rank[order] * 2 + ep[order]) * NGRP + grp[order]
    starts = np.searchsorted(key, np.arange(R * 2 * NGRP + 1))

    in_maps = []
    for r in range(R):
        ep_arrs = {}
        for e in (0, 1):
            nt = nt_l[e]
            off = off_l[e]
            nch = nch_l[e]
            choff = choff_l[e]
            tot = int(off[-1])
            chtot = int(choff[-1])
            gidx = np.zeros((128, tot * 8), np.int16)
            blob = np.zeros((128, tot * 256), ml_dtypes.float8_e4m3)
            attr4 = np.zeros((5, chtot * 128), ml_dtypes.float8_e4m3)
            attr4[4, :] = 1.0
            for g in range(NGRP):
                ntg = int(nt[g])
                if ntg == 0:
                    continue
                k = (r * 2 + e) * NGRP + g
                s0 = int(starts[k])
                n = int(counts[e, r, g])
                o = int(off[g])
                co = int(choff[g])
                idx = np.full((ntg * 128,), -1, np.int64)
                idx[:n] = trow_s[s0:s0 + n]
                if n == 0:
                    idx[0] = 0
                gidx[:, o * 8:(o + ntg) * 8] = wrap_idxs_block(idx)
                j = np.arange(n)
                t = j // 128
                p = j % 128
                dr = drel_s[s0:s0 + n].astype(np.int64)
                # S: [p_edge, d] at cols (o+t)*256 + d
                blob[p, (o + t) * 256 + dr] = 1
                # ST: [drel, p_edge] at cols (o+t)*256 + 128 + p
                blob[dr, (o + t) * 256 + 128 + p] = 1
                attr4[t % KQC, (co + t // KQC) * 128 + p] = \
                    attr_s[s0:s0 + n].astype(ml_dtypes.float8_e4m3)
            ep_arrs[e] = (gidx, blob, attr4)
        IndT = np.zeros((NPAD, G), np.float32)
        lo, hi = r * NPR, min((r + 1) * NPR, N)
        IndT[np.arange(hi - lo), batch[lo:hi]] = inv_cnt[batch[lo:hi]]
        xT_loc = np.zeros((128, NPAD), np.float32)
        xT_loc[:, 0:hi - lo] = x[lo:hi].T
        x_rb = np.zeros((R * 128, NPAD), np.float32)
        for rr in range(R):
            rlo, rhi = rr * NPR, min((rr + 1) * NPR, N)
            x_rb[rr * 128:(rr + 1) * 128, 0:rhi - rlo] = x[rlo:rhi].T
        gcnt = np.maximum(counts[:, r, :], 1).astype(np.int32).reshape(1, 2 * NGRP)
        im = {
            "gcnt": gcnt,
            "xT_loc": xT_loc.astype(ml_dtypes.float8_e4m3),
            "x_rb": x_rb.astype(ml_dtypes.float8_e4m3),
            "Wqvk": Wqvk.astype(ml_dtypes.bfloat16),
            "Ws": Ws_e.astype(ml_dtypes.bfloat16),
            "BD": BD.astype(ml_dtypes.bfloat16),
            "bv_row": np.concatenate(
                [np.zeros((NLAYER, 1, H), np.float32),
                 bv_e.reshape(NLAYER, 1, H),
                 np.zeros((NLAYER, 1, H), np.float32)], axis=2
            ).astype(ml_dtypes.bfloat16),
            "bs_col": bs_e.reshape(NLAYER, H, 1),
            "gidx0": ep_arrs[0][0], "blob0": ep_arrs[0][1], "attr0": ep_arrs[0][2],
            "gidx1": ep_arrs[1][0], "blob1": ep_arrs[1][1], "attr1": ep_arrs[1][2],
            "IndT": IndT.astype(ml_dtypes.bfloat16),
            "clinT": clinical.T.copy(),
            "Wc_h": Wc_h, "Wc_c": Wc[H:H + NCLIN],
            "bc_rep": np.tile(bc_e, (G, 1)),
        }
        in_maps.append(im)
    meta = dict(NT0=tuple(int(v) for v in nt_l[0]),
                NT1=tuple(int(v) for v in nt_l[1]),
                has_bv=has_bv, has_bs=has_bs, tab_fp8=True, dr_seg=True)
    return in_maps, meta


# ---------------------------------------------------------------------------
# device program
# ---------------------------------------------------------------------------

def build(meta):
    parts = meta.get("parts", 4)
    DT = BF16
    TDT = FP8 if meta.get("tab_fp8", True) else BF16
    NT_L = [list(meta["NT0"]), list(meta["NT1"])]
    OFF_L = [np.concatenate([[0], np.cumsum(nt)]).astype(int) for nt in NT_L]
    NCH_L = [np.ceil(np.array(nt) / KQC).astype(int) for nt in NT_L]
    CHOFF_L = [np.concatenate([[0], np.cumsum(nc)]).astype(int) for nc in NCH_L]
    TOT = [int(o[-1]) for o in OFF_L]
    CHTOT = [int(c[-1]) for c in CHOFF_L]
    TMAX = max(max(NT_L[0]), max(NT_L[1]))

    nc = bacc.Bacc("TRN2", target_bir_lowering=False, debug=False, num_devices=R)

    def din(name, shape, dt):
        return nc.dram_tensor(name, shape, dt, kind="ExternalInput").ap()

    t_xT = din("xT_loc", [128, NPAD], FP8)
    t_cnt = din("gcnt", [1, 2 * NGRP], mybir.dt.int32)
    t_xrb = din("x_rb", [R * 128, NPAD], FP8)
    t_Wqvk = din("Wqvk", [NLAYER, H, 3 * H], DT)
    t_Ws = din("Ws", [NLAYER, H, H], DT)
    t_BD = din("BD", [NLAYER, 5, KQC * H], DT)
    t_bv = din("bv_row", [NLAYER, 1, 3 * H], DT)
    t_bs = din("bs_col", [NLAYER, H, 1], F32)
    t_gidx = [din("gidx0", [128, TOT[0] * 8], I16),
              din("gidx1", [128, TOT[1] * 8], I16)]
    t_blob = [din("blob0", [128, TOT[0] * 256], FP8),
              din("blob1", [128, TOT[1] * 256], FP8)]
    t_attr = [din("attr0", [5, CHTOT[0] * 128], FP8),
              din("attr1", [5, CHTOT[1] * 128], FP8)]
    t_IndT = din("IndT", [NPAD, G], DT)
    t_clinT = din("clinT", [NCLIN, G], F32)
    t_Wc_h = din("Wc_h", [H, NCLS], F32)
    t_Wc_c = din("Wc_c", [NCLIN, NCLS], F32)
    t_bc = din("bc_rep", [G, NCLS], F32)

    t_out = nc.dram_tensor("out", [G, NCLS], F32, kind="ExternalOutput").ap()

    qv_loc = nc.dram_tensor("qv_loc", [NPAD, 2 * H], TDT).ap()
    # per-layer AG'd table pairs (separate per layer so the next layer's
    # AllGather never overwrites a table the current layer still gathers from)
    qv_tl = [None,
             [nc.dram_tensor("qv1_t0", [R * C0N, 2 * H], TDT, addr_space="Shared").ap(),
              nc.dram_tensor("qv1_t1", [R * C1N, 2 * H], TDT, addr_space="Shared").ap()],
             [nc.dram_tensor("qv2_t0", [R * C0N, 2 * H], TDT, addr_space="Shared").ap(),
              nc.dram_tensor("qv2_t1", [R * C1N, 2 * H], TDT, addr_space="Shared").ap()]]
    # layer-0 tables are built locally (x is replicated), no collective
    qv_tl[0] = [nc.dram_tensor("qv0_t0", [R * C0N, 2 * H], TDT).ap(),
                nc.dram_tensor("qv0_t1", [R * C1N, 2 * H], TDT).ap()]
    pool_in = nc.dram_tensor("pool_in", [G, H], F32).ap()
    pool_out = nc.dram_tensor("pool_out", [R * G, H], F32, addr_space="Shared").ap()

    with tile.TileContext(nc) as tc:
        import contextlib
        with contextlib.ExitStack() as ctx:
            consts = ctx.enter_context(tc.tile_pool(name="consts", bufs=1))
            hsb = ctx.enter_context(tc.tile_pool(name="hsb", bufs=1))
            h3p = ctx.enter_context(tc.tile_pool(name="h3p", bufs=1))
            ksb = ctx.enter_context(tc.tile_pool(name="ksb", bufs=1))
            stg = ctx.enter_context(tc.tile_pool(name="stg", bufs=4))
            tail = ctx.enter_context(tc.tile_pool(name="tail", bufs=2))
            edg = ctx.enter_context(tc.tile_pool(name="edg", bufs=5))
            sel = ctx.enter_context(tc.tile_pool(name="sel", bufs=8))
            edm = ctx.enter_context(tc.tile_pool(name="edm", bufs=3))
            pnode = ctx.enter_context(tc.tile_pool(name="pnode", bufs=2, space="PSUM"))
            pkq = ctx.enter_context(tc.tile_pool(name="pkq", bufs=2, space="PSUM"))
            pedge = ctx.enter_context(tc.tile_pool(name="pedge", bufs=2, space="PSUM"))
            ppool = ctx.enter_context(tc.tile_pool(name="ppool", bufs=1, space="PSUM"))

            _cid = [0]

            def load_const(src_ap, shape, dt):
                _cid[0] += 1
                t = consts.tile(shape, dt, tag=f"c{_cid[0]}_{src_ap.tensor.name}")
                nc.sync.dma_start(t[:], src_ap)
                return t

            Wqvk_t = [load_const(t_Wqvk[l], [H, 3 * H], DT) for l in range(NLAYER)]
            Ws_t = [load_const(t_Ws[l], [H, H], DT) for l in range(NLAYER)]
            BD_t = [load_const(t_BD[l], [5, KQC * H], DT) for l in range(NLAYER)]
            bv_t = ([load_const(t_bv[l], [1, 3 * H], DT) for l in range(NLAYER)]
                    if meta["has_bv"] else None)
            bs_t = ([load_const(t_bs[l], [H, 1], F32) for l in range(NLAYER)]
                    if meta["has_bs"] else None)
            ones_t = consts.tile([1, 128], DT)
            nc.vector.memset(ones_t[:], 1.0)
            gidx_t = [load_const(t_gidx[0], [128, TOT[0] * 8], I16),
                      load_const(t_gidx[1], [128, TOT[1] * 8], I16)]
            attr_t = [load_const(t_attr[0], [5, CHTOT[0] * 128], FP8),
                      load_const(t_attr[1], [5, CHTOT[1] * 128], FP8)]
            xT_t = load_const(t_xT, [128, NPAD], FP8)
            cnt_t = load_const(t_cnt, [1, 2 * NGRP], mybir.dt.int32)
            ident = consts.tile([128, 128], DT)
            make_identity(nc, ident[:])
            identf = consts.tile([128, 128], F32)
            make_identity(nc, identf[:])
            clin_t = load_const(t_clinT, [NCLIN, G], F32)
            Wch_t = load_const(t_Wc_h, [H, NCLS], F32)
            Wcc_t = load_const(t_Wc_c, [NCLIN, NCLS], F32)
            bc_t = load_const(t_bc, [G, NCLS], F32)

            # ping-pong state by layer parity
            hs_pp = [hsb.tile([128, NPAD], DT, tag="hs0", name="hs0"),
                     hsb.tile([128, NPAD], DT, tag="hs1", name="hs1")]
            k_pp = [ksb.tile([128, NGRP, H], DT, tag="k0", name="k0"),
                    ksb.tile([128, NGRP, H], DT, tag="k1", name="k1")]
            h3_pp = [h3p.tile([128, NPAD], DT, tag="h3a", name="h3a"),
                     h3p.tile([128, NPAD], DT, tag="h3b", name="h3b")]
            h3f = h3_pp[0]                             # layer-2 output parity

            def build_chunk(l, c, hsrc):
                """Local q|v|k for groups of AG-chunk c of layer l."""
                g0, g1 = (0, C0G) if c == 0 else (C0G, NGRP)
                k_sb = k_pp[l % 2]
                g = g0
                while g < g1:
                    ns = min(8, g1 - g)
                    st = stg.tile([128, 8, 2 * H], TDT, tag="st")
                    for s in range(ns):
                        gg = g + s
                        ps = pnode.tile([128, 512], F32, tag="pn")
                        nc.tensor.matmul(
                            out=ps[:, 0:3 * H],
                            lhsT=hsrc[:, gg * 128:(gg + 1) * 128],
                            rhs=Wqvk_t[l][:], start=True,
                            stop=not meta["has_bv"],
                            skip_group_check=True)
                        if meta["has_bv"]:
                            nc.tensor.matmul(
                                out=ps[:, 0:3 * H], lhsT=ones_t[:],
                                rhs=bv_t[l][:], start=False, stop=True,
                                skip_group_check=True)
                        nc.scalar.activation(st[:, s, :], ps[:, 0:2 * H],
                                             AF.Copy)
                        nc.scalar.activation(k_sb[:, gg, :],
                                             ps[:, 2 * H:3 * H], AF.Copy)
                    nc.sync.dma_start(
                        qv_loc[g * 128:(g + ns) * 128, :].rearrange(
                            "(s p) f -> p s f", p=128),
                        st[:, 0:ns, :])
                    g += ns

            def emit_ag(l, c):
                ins = qv_loc[0:C0N, :] if c == 0 else qv_loc[C0N:NPAD, :]
                nc.gpsimd.collective_compute(
                    "AllGather", OP.bypass, replica_groups=[list(range(R))],
                    ins=[ins], outs=[qv_tl[l][c][:]])

            def emit_spart(l, hsrc):
                hs = hs_pp[l % 2]
                for c0 in range(0, NPAD, 512):
                    csz = min(512, NPAD - c0)
                    pss = pnode.tile([128, 512], F32, tag="pn")
                    nc.tensor.matmul(out=pss[:, 0:csz], lhsT=Ws_t[l][:],
                                     rhs=hsrc[:, c0:c0 + csz],
                                     start=True, stop=True)
                    if meta["has_bs"]:
                        nc.scalar.activation(hs[:, c0:c0 + csz], pss[:, 0:csz],
                                             AF.Identity, bias=bs_t[l][:],
                                             scale=1.0)
                    else:
                        nc.scalar.activation(hs[:, c0:c0 + csz], pss[:, 0:csz],
                                             AF.Copy)

            def make_tail_cb(l, c_start, c_end):
                # During layer l's ep1 sweep over [c_start, c_end): as each
                # dst-group finalizes, leaky just that slice and build the
                # next layer's table rows for it, storing every 8 groups so
                # the chunk AllGather can fire the moment the sweep ends.
                st_state = {}

                def cb(g, l=l):
                    sl = slice(g * 128, (g + 1) * 128)
                    nc.vector.scalar_tensor_tensor(
                        out=h3_pp[l % 2][:, sl], in0=hs_pp[l % 2][:, sl],
                        scalar=float(SLOPE), in1=hs_pp[l % 2][:, sl],
                        op0=OP.mult, op1=OP.max)
                    if l >= NLAYER - 1:
                        return
                    s = (g - c_start) % 8
                    if s == 0:
                        st_state["st"] = stg.tile([128, 8, 2 * H], TDT,
                                                  tag="st", name="st_cb")
                        st_state["g0"] = g
                    st = st_state["st"]
                    ps = pnode.tile([128, 512], F32, tag="pn", name="pn_cb")
                    nc.tensor.matmul(
                        out=ps[:, 0:3 * H],
                        lhsT=h3_pp[l % 2][:, sl],
                        rhs=Wqvk_t[l + 1][:], start=True,
                        stop=not meta["has_bv"], skip_group_check=True)
                    if meta["has_bv"]:
                        nc.tensor.matmul(
                            out=ps[:, 0:3 * H], lhsT=ones_t[:],
                            rhs=bv_t[l + 1][:], start=False, stop=True,
                            skip_group_check=True)
                    nc.scalar.activation(st[:, s, :], ps[:, 0:2 * H], AF.Copy)
                    nc.scalar.activation(k_pp[(l + 1) % 2][:, g, :],
                                         ps[:, 2 * H:3 * H], AF.Copy)
                    if s == 7 or g == c_end - 1:
                        g0b = st_state["g0"]
                        nc.sync.dma_start(
                            qv_loc[g0b * 128:(g + 1) * 128, :].rearrange(
                                "(s p) f -> p s f", p=128),
                            st[:, 0:g + 1 - g0b, :])
                return cb

            def emit_leaky(l, c):
                sl = slice(0, C0N) if c == 0 else slice(C0N, NPAD)
                nc.vector.scalar_tensor_tensor(
                    out=h3_pp[l % 2][:, sl], in0=hs_pp[l % 2][:, sl],
                    scalar=float(SLOPE), in1=hs_pp[l % 2][:, sl],
                    op0=OP.mult, op1=OP.max)

            def emit_edges(e, l, g0, g1, after_group=None):
                nt_list = NT_L[e]
                off = OFF_L[e]
                choff = CHOFF_L[e]
                hs = hs_pp[l % 2]
                k_sb = k_pp[l % 2]
                tab = qv_tl[l][e]
                for g in range(g0, g1):
                    nt = int(nt_list[g])
                    if nt == 0:
                        if after_group is not None:
                            after_group(g)
                        continue
                    ne = nt * 128
                    o = int(off[g])
                    co = int(choff[g])
                    bt = sel.tile([128, TMAX, 256], FP8, tag="blob")
                    nc.sync.dma_start(
                        bt[:, 0:nt, :],
                        t_blob[e][:, o * 256:(o + nt) * 256].rearrange(
                            "p (t d) -> p t d", t=nt))
                    gt = edg.tile([128, TMAX, 2 * H], TDT, tag="g")
                    nreg = nc.gpsimd.value_load(
                        cnt_t[0:1, e * NGRP + g:e * NGRP + g + 1])
                    nc.gpsimd.dma_gather(
                        gt[:, 0:nt, :], tab[:],
                        gidx_t[e][:, o * 8:(o + nt) * 8],
                        ne, nreg, 2 * H, single_packet=(ne <= 512))
                    ktb = edm.tile([128, TMAX, 128], DT, tag="kt")
                    for ci in range((nt + KQC - 1) // KQC):
                        q0 = ci * KQC
                        qn = min(KQC, nt - q0)
                        pk = pkq.tile([128, KQC, 128], F32, tag="kq")
                        nc.tensor.matmul(
                            out=pk[:, 0:KQC, :],
                            lhsT=attr_t[e][0:5, (co + ci) * 128:(co + ci + 1) * 128],
                            rhs=BD_t[l][:], start=True, stop=False,
                            skip_group_check=True)
                        for t in range(q0, q0 + qn):
                            nc.tensor.matmul(
                                out=pk[:, t - q0, :],
                                lhsT=bt[:, t, 128:256],
                                rhs=k_sb[:, g, :], start=False, stop=False,
                                skip_group_check=True)
                        # q[src] folded into the same PSUM via identity matmul
                        nc.tensor.matmul(
                            out=pk[:, 0:qn, :], lhsT=ident[:],
                            rhs=gt[:, q0:q0 + qn, 0:H], start=False, stop=True,
                            skip_group_check=True)
                        nc.scalar.activation(ktb[:, q0:q0 + qn, :],
                                             pk[:, 0:qn, :], AF.Sigmoid)
                    dr = meta.get("dr_seg", False)
                    msg = edm.tile([128, TMAX, 128], FP8 if dr else DT, tag="msg")
                    nc.vector.tensor_tensor(out=msg[:, 0:nt, :],
                                            in0=ktb[:, 0:nt, :],
                                            in1=gt[:, 0:nt, H:2 * H], op=OP.mult)
                    pa = pedge.tile([128, 128], F32, tag="pa")
                    if dr:
                        npair = nt // 2
                        for t2 in range(0, npair * 2, 2):
                            nc.tensor.matmul(
                                out=pa[:], lhsT=msg[:, t2:t2 + 2, :],
                                rhs=bt[:, t2:t2 + 2, 0:128], start=(t2 == 0),
                                stop=(t2 + 2 == nt),
                                perf_mode=mybir.MatmulPerfMode.DoubleRow,
                                skip_group_check=True)
                        if nt % 2:
                            nc.tensor.matmul(out=pa[:], lhsT=msg[:, nt - 1, :],
                                             rhs=bt[:, nt - 1, 0:128],
                                             start=(nt == 1), stop=True,
                                             skip_group_check=True)
                    else:
                        for t in range(nt):
                            nc.tensor.matmul(out=pa[:], lhsT=msg[:, t, :],
                                             rhs=bt[:, t, 0:128], start=(t == 0),
                                             stop=(t == nt - 1))
                    nc.vector.tensor_tensor(
                        out=hs[:, g * 128:(g + 1) * 128],
                        in0=hs[:, g * 128:(g + 1) * 128], in1=pa[:], op=OP.add)
                    if after_group is not None:
                        after_group(g)

            # ---- layer-0 table from x: replicated input, so each rank builds
            # the FULL table locally (no collective, no exposed startup AG).
            def build_l0_chunk(c):
                g0, g1, tab, cbase = ((0, C0G, qv_tl[0][0], C0N) if c == 0
                                      else (C0G, NGRP, qv_tl[0][1], C1N))
                for rb in range(R):
                    g = g0
                    while g < g1:
                        ns = min(8, g1 - g)
                        lh = stg.tile([128, 1024], FP8, tag="lh")
                        nc.sync.dma_start(
                            lh[:, 0:ns * 128],
                            t_xrb[rb * 128:(rb + 1) * 128,
                                  g * 128:(g + ns) * 128])
                        st = stg.tile([128, 8, 2 * H], TDT, tag="st")
                        for s in range(0, ns, 2):
                            n2 = min(2, ns - s)
                            ps = pnode.tile([128, 512], F32, tag="pn")
                            for u in range(n2):
                                nc.tensor.matmul(
                                    out=ps[:, u * 256:u * 256 + 2 * H],
                                    lhsT=lh[:, (s + u) * 128:(s + u + 1) * 128],
                                    rhs=Wqvk_t[0][:, 0:2 * H], start=True,
                                    stop=not meta["has_bv"],
                                    skip_group_check=True)
                                if meta["has_bv"]:
                                    nc.tensor.matmul(
                                        out=ps[:, u * 256:u * 256 + 2 * H],
                                        lhsT=ones_t[:],
                                        rhs=bv_t[0][0:1, 0:2 * H],
                                        start=False, stop=True,
                                        skip_group_check=True)
                            if (s // 2) % 2 == 0:
                                nc.scalar.activation(st[:, s:s + n2, :],
                                                     ps[:, 0:n2 * 256], AF.Copy)
                            else:
                                nc.vector.tensor_copy(st[:, s:s + n2, :],
                                                      ps[:, 0:n2 * 256])
                        r0 = rb * cbase + (g - g0) * 128
                        nc.sync.dma_start(
                            tab[r0:r0 + ns * 128, :].rearrange(
                                "(s p) f -> p s f", p=128),
                            st[:, 0:ns, :])
                        g += ns
            build_l0_chunk(0)
            # local k table + s-part for layer 0 from the local x slice
            for g in range(NGRP):
                psk = pnode.tile([128, 512], F32, tag="pn")
                nc.tensor.matmul(out=psk[:, 0:H],
                                 lhsT=xT_t[:, g * 128:(g + 1) * 128],
                                 rhs=Wqvk_t[0][:, 2 * H:3 * H],
                                 start=True, stop=True, skip_group_check=True)
                nc.scalar.activation(k_pp[0][:, g, :], psk[:, 0:H], AF.Copy)
            emit_spart(0, xT_t)

            for _ in range(5):
                gz = edg.tile([128, TMAX, 2 * H], TDT, tag="g")
                nc.vector.memset(gz[:], 0.0)

            pp = ppool.tile([G, H], F32)
            indc = consts.tile([128, NGRP, G], DT)
            nc.sync.dma_start(
                indc[:], t_IndT[:].rearrange("(c p) g -> p c g", p=128))

            def emit_pool_part(c0g, c1g):
                for c in range(c0g, c1g):
                    trp = pedge.tile([128, 128], DT, tag="pa")
                    nc.tensor.transpose(out=trp[:],
                                        in_=h3f[:, c * 128:(c + 1) * 128],
                                        identity=ident[:])
                    hnode = tail.tile([128, 128], DT, tag="hnode")
                    nc.vector.tensor_copy(hnode[:], trp[:])
                    nc.tensor.matmul(out=pp[:],
                                     lhsT=indc[:, c, :],
                                     rhs=hnode[:],
                                     start=(c == 0), stop=(c == NGRP - 1))

            for l in range(NLAYER):
                hsrc_next = h3_pp[l % 2]
                last = l >= NLAYER - 1
                cb0 = make_tail_cb(l, 0, C0G) if (parts >= 3 and not last) else None
                cb1 = make_tail_cb(l, C0G, NGRP) if (parts >= 3 and not last) else None
                if parts >= 2:
                    if l == 0:
                        # layer 0 consumes no AllGather (local tables), so run
                        # both epochs' first halves up front and fire the hook
                        # at ~50% -- the next layer's AG0 starts much earlier.
                        emit_edges(0, l, 0, 12)
                        build_l0_chunk(1)
                        emit_edges(0, l, 12, C0G)
                        emit_edges(1, l, 0, C0G, after_group=cb0)
                    else:
                        # full epoch-0 sweep first (its table is ready), then
                        # the first ep1 half; this keeps ep0 work ahead of any
                        # ep1 stall on the second table chunk's AllGather.
                        emit_edges(0, l, 0, NGRP)
                        emit_edges(1, l, 0, C0G, after_group=cb0)
                # h3 chunk-0 final: fire next layer's chunk-0 AllGather (the
                # per-group callback already built and stored the rows)
                if not last:
                    if parts < 2:
                        emit_leaky(l, 0)
                        build_chunk(l + 1, 0, hsrc_next)
                    emit_ag(l + 1, 0)
                else:
                    emit_leaky(l, 0)
                    if parts >= 4:
                        emit_pool_part(0, C0G)
                if parts >= 2:
                    if l == 0:
                        emit_edges(0, l, C0G, NGRP)
                    emit_edges(1, l, C0G, NGRP, after_group=cb1)
                if parts < 3:
                    break
                if not last:
                    emit_ag(l + 1, 1)
                    emit_spart(l + 1, hsrc_next)
                else:
                    emit_leaky(l, 1)
                    if parts >= 4:
                        emit_pool_part(C0G, NGRP)
            if parts < 4:
                z_dbg = tail.tile([G, NCLS], F32, tag="zsb")
                nc.vector.tensor_copy(z_dbg[:], hs_pp[0][0:G, 0:NCLS])
                nc.sync.dma_start(t_out[:], z_dbg[:])
            else:
                pool_sb = tail.tile([G, H], F32, tag="poolsb")
                nc.vector.tensor_copy(pool_sb[:], pp[:])
                nc.sync.dma_start(pool_in[:], pool_sb[:])
                nc.gpsimd.collective_compute(
                    "AllGather", OP.bypass, replica_groups=[list(range(R))],
                    ins=[pool_in[:]], outs=[pool_out[:]])
                pr = tail.tile([G, R, H], F32, tag="pr")
                nc.sync.dma_start(pr[:], pool_out[:].rearrange("(r g) h -> g r h", r=R))
                pooled = tail.tile([G, H], F32, tag="pooled")
                nc.vector.tensor_tensor(out=pooled[:], in0=pr[:, 0, :], in1=pr[:, 1, :],
                                        op=OP.add)
                for r in range(2, R):
                    nc.vector.tensor_tensor(out=pooled[:], in0=pooled[:],
                                            in1=pr[:, r, :], op=OP.add)
                ptp = pedge.tile([H, G], F32, tag="pa")
                nc.tensor.transpose(out=ptp[:], in_=pooled[:], identity=identf[0:G, 0:G])
                pooledT = tail.tile([H, G], F32, tag="pooledT")
                nc.vector.tensor_copy(pooledT[:], ptp[:])
                zp = pedge.tile([G, NCLS], F32, tag="pa")
                nc.tensor.matmul(out=zp[:], lhsT=pooledT[:], rhs=Wch_t[:],
                                 start=True, stop=False)
                nc.tensor.matmul(out=zp[:], lhsT=clin_t[:], rhs=Wcc_t[:],
                                 start=False, stop=True)
                z_sb = tail.tile([G, NCLS], F32, tag="zsb")
                nc.vector.tensor_tensor(out=z_sb[:], in0=zp[:], in1=bc_t[:],
                                        op=OP.add)
                nc.sync.dma_start(t_out[:], z_sb[:])

    nc.compile()
    return nc


# ---------------------------------------------------------------------------

_CACHE = {}


def kernel(**inputs):
    in_maps, meta = prep(inputs)
    key = tuple(sorted((k, v) for k, v in meta.items()))
    if key not in _CACHE:
        _CACHE[key] = build(meta)
    nc = _CACHE[key]
    res = run_bass_kernel_spmd(nc, in_maps, list(range(R)))
    return np.asarray(res.results[0]["out"], np.float32)


def kernel_profiled(**inputs):
    """Like kernel() but also returns (exec_time_ns, trace_path)."""
    in_maps, meta = prep(inputs)
    key = tuple(sorted((k, v) for k, v in meta.items()))
    if key not in _CACHE:
        _CACHE[key] = build(meta)
    nc = _CACHE[key]
    res = run_bass_kernel_spmd(nc, in_maps, list(range(R)), trace=True)
    out = np.asarray(res.results[0]["out"], np.float32)
    trace_path = None
    if res.instructions_and_trace is not None:
        trace_path = res.instructions_and_trace[1]
    return out, res.exec_time_ns, trace_path


if __name__ == "__main__":
    pass


# revision 54
# speedup vs baseline: 1.8398x; 1.0127x over previous
"""Trainium2 Bass kernel for a 3-layer ResGatedGraphConv GNN (ClinicalGatedGCN).

Strategy (8 NeuronCores, SPMD), v3:
  - Nodes partitioned into 8 contiguous ranges of 6250 (padded to 6272 = 49
    groups of 128). Edges assigned to the rank owning their dst node, sorted
    by (src-chunk epoch, dst-group, dst) on the host.
  - The [q|v] node table is built DISTRIBUTED: each rank computes q,v (and k)
    only for its OWN nodes from its SBUF-resident h (one fused matmul per
    128-node group against [Wq|Wv|Wk]), stores the [q|v] rows to local DRAM,
    and AllGathers the table in two chunks (groups 0:24 -> qv_t0, 24:49 ->
    qv_t1; both tables stay under the int16 gather-index limit). This removes
    the per-layer h AllGather, all h re-reads, and ~7/8 of the table matmuls
    and PSUM->SBUF copies of the replicated design.
  - One dma_gather per (epoch, dst-group) fetches the src rows of [q|v].
    k[dst] is expanded on the PE via a host-shipped 0/1 selector ST; the
    segment-sum over dst is a PE matmul against selector S. S and ST are
    packed in ONE fp8 blob per (epoch, group) (0/1 is exact in fp8; matmul
    operands may mix fp8 with bf16), halving selector DMA traffic.
  - The edge-embedding rank-1 term (attr x We + gate bias) is batched: one
    K=5 matmul per 4-tile PSUM chunk against a block-diagonal [5, 4H]
    constant, with per-chunk attr columns shipped once as an fp8 constant.
  - BatchNorm folding: A = gamma/sqrt(var+eps) is folded into the NEXT
    layer's weights (and the classifier); B is folded into effective biases.
    The on-device h update is a single fused leaky-ReLU
    (scalar_tensor_tensor max(x, slope*x)).
  - Mean-pool per graph is a matmul against a host-built indicator with
    1/cnt folded in; partial pools are AllGather'd and summed; the tiny
    classifier runs on every core.
"""

import numpy as np
import ml_dtypes

import concourse.bacc as bacc
import concourse.bass as bass
import concourse.mybir as mybir
import concourse.tile as tile
from concourse.bass_utils import run_bass_kernel_spmd
from concourse.masks import make_identity

F32 = mybir.dt.float32
BF16 = mybir.dt.bfloat16
FP8 = mybir.dt.float8e4
I16 = mybir.dt.int16
AF = mybir.ActivationFunctionType
OP = mybir.AluOpType

# ---------------- problem constants (hardcoded per spec) ----------------
N, E, H, G, NCLIN, NCLS = 50000, 800000, 128, 64, 16, 2
NLAYER = 3
EPS = 1e-5
SLOPE = 0.01
R = 8                      # ranks / NeuronCores

NPR = (N + R - 1) // R     # real nodes per rank (6250)
NGRP = (NPR + 127) // 128  # 128-node groups per rank (49)
NPAD = NGRP * 128          # padded nodes per rank (6272)
C0G = 24                   # groups in AG chunk 0
C1G = NGRP - C0G           # groups in AG chunk 1 (25)
C0N = C0G * 128            # 3072 rows
C1N = C1G * 128            # 3200 rows
KQC = 4                    # psum chunk size in edge tiles (1 PSUM bank)


def wrap_idxs_block(idx):
    """Wrap one gather call's indices: idx j -> [j%16, j//16], tiled to 128 parts."""
    n = len(idx)
    assert n % 16 == 0
    w = np.asarray(idx, np.int16).reshape(n // 16, 16).T
    return np.tile(w, (8, 1))


# ---------------------------------------------------------------------------
# host-side preprocessing
# ---------------------------------------------------------------------------

def prep(inputs):
    x = np.asarray(inputs["x"], np.float32)
    edge_index = np.asarray(inputs["edge_index"])
    edge_attr = np.asarray(inputs["edge_attr"], np.float32)[:, 0]
    batch = np.asarray(inputs["batch"]).astype(np.int64)
    clinical = np.asarray(inputs["clinical"], np.float32)
    Wk, bk = np.asarray(inputs["Wk"], np.float32), np.asarray(inputs["bk"], np.float32)
    Wq, bq = np.asarray(inputs["Wq"], np.float32), np.asarray(inputs["bq"], np.float32)
    Wv, bv = np.asarray(inputs["Wv"], np.float32), np.asarray(inputs["bv"], np.float32)
    Ws, bs = np.asarray(inputs["Ws"], np.float32), np.asarray(inputs["bs"], np.float32)
    We, be = np.asarray(inputs["We"], np.float32), np.asarray(inputs["be"], np.float32)
    gamma = np.asarray(inputs["gamma"], np.float32)
    beta = np.asarray(inputs["beta"], np.float32)
    rmean = np.asarray(inputs["rmean"], np.float32)
    rvar = np.asarray(inputs["rvar"], np.float32)
    Wc, bc = np.asarray(inputs["Wc"], np.float32), np.asarray(inputs["bc"], np.float32)

    src = edge_index[0].astype(np.int64)
    dst = edge_index[1].astype(np.int64)

    # BN folded: true h_out = A*leaky(pre) + B; device h3 = leaky(pre).
    A = gamma / np.sqrt(rvar + EPS)          # [3, H]
    B = beta - rmean * A                     # [3, H]

    # effective weights: fold diag(A[l-1]) into layer-l input maps, and the
    # B[l-1] offset into layer-l biases.
    Wq_e = np.stack([Wq[l] * (A[l - 1][:, None] if l else 1.0) for l in range(NLAYER)])
    Wk_e = np.stack([Wk[l] * (A[l - 1][:, None] if l else 1.0) for l in range(NLAYER)])
    Wv_e = np.stack([Wv[l] * (A[l - 1][:, None] if l else 1.0) for l in range(NLAYER)])
    Ws_e = np.stack([Ws[l] * (A[l - 1][:, None] if l else 1.0) for l in range(NLAYER)])
    bgate = np.stack([bk[l] + bq[l] + be[l]
                      + (B[l - 1] @ (Wk[l] + Wq[l]) if l else 0.0)
                      for l in range(NLAYER)])
    bv_e = np.stack([bv[l] + (B[l - 1] @ Wv[l] if l else 0.0) for l in range(NLAYER)])
    bs_e = np.stack([bs[l] + (B[l - 1] @ Ws[l] if l else 0.0) for l in range(NLAYER)])
    Wc_h = Wc[0:H] * A[2][:, None]
    bc_e = bc + B[2] @ Wc[0:H]

    Wqvk = np.concatenate([Wq_e, Wv_e, Wk_e], axis=2)     # [3, H, 3H]

    # block-diagonal edge-term constant: rows 0..3 carry We on diag block,
    # row 4 carries the gate bias (broadcast to each tile block).
    BD = np.zeros((NLAYER, 5, KQC * H), np.float32)
    for l in range(NLAYER):
        for t in range(KQC):
            BD[l, t, t * H:(t + 1) * H] = We[l, 0, :]
            BD[l, 4, t * H:(t + 1) * H] = bgate[l]

    has_bv = bool(np.any(bv_e != 0))
    has_bs = bool(np.any(bs_e != 0))

    # ---- edge structure ----
    e_rank = dst // NPR
    dst_local = dst - e_rank * NPR
    grp = dst_local // 128
    drel = dst_local % 128
    rs = src // NPR
    lr = src - rs * NPR
    ep = (lr >= C0N).astype(np.int64)
    trow = np.where(ep == 0, rs * C0N + lr, rs * C1N + (lr - C0N))

    counts = np.zeros((2, R, NGRP), np.int64)
    np.add.at(counts, (ep, e_rank, grp), 1)
    nt_l = [np.ceil(counts[e].max(axis=0) / 128).astype(int) for e in (0, 1)]
    off_l = [np.concatenate([[0], np.cumsum(nt)]).astype(int) for nt in nt_l]
    nch_l = [np.ceil(nt / KQC).astype(int) for nt in nt_l]
    choff_l = [np.concatenate([[0], np.cumsum(nc)]).astype(int) for nc in nch_l]

    cntg = np.bincount(batch, minlength=G).astype(np.float32)
    inv_cnt = 1.0 / np.maximum(cntg, 1.0)

    order = np.lexsort((dst, grp, ep, e_rank))
    trow_s, drel_s, attr_s = trow[order], drel[order], edge_attr[order]
    key = (e_rank[order] * 2 + ep[order]) * NGRP + grp[order]
    starts = np.searchsorted(key, np.arange(R * 2 * NGRP + 1))

    in_maps = []
    for r in range(R):
        ep_arrs = {}
        for e in (0, 1):
            nt = nt_l[e]
            off = off_l[e]
            nch = nch_l[e]
            choff = choff_l[e]
            tot = int(off[-1])
            chtot = int(choff[-1])
            gidx = np.zeros((128, tot * 8), np.int16)
            blob = np.zeros((128, tot * 256), ml_dtypes.float8_e4m3)
            attr4 = np.zeros((5, chtot * 128), ml_dtypes.float8_e4m3)
            attr4[4, :] = 1.0
            for g in range(NGRP):
                ntg = int(nt[g])
                if ntg == 0:
                    continue
                k = (r * 2 + e) * NGRP + g
                s0 = int(starts[k])
                n = int(counts[e, r, g])
                o = int(off[g])
                co = int(choff[g])
                idx = np.full((ntg * 128,), -1, np.int64)
                idx[:n] = trow_s[s0:s0 + n]
                if n == 0:
                    idx[0] = 0
                gidx[:, o * 8:(o + ntg) * 8] = wrap_idxs_block(idx)
                j = np.arange(n)
                t = j // 128
                p = j % 128
                dr = drel_s[s0:s0 + n].astype(np.int64)
                # S: [p_edge, d] at cols (o+t)*256 + d
                blob[p, (o + t) * 256 + dr] = 1
                # ST: [drel, p_edge] at cols (o+t)*256 + 128 + p
                blob[dr, (o + t) * 256 + 128 + p] = 1
                attr4[t % KQC, (co + t // KQC) * 128 + p] = \
                    attr_s[s0:s0 + n].astype(ml_dtypes.float8_e4m3)
            ep_arrs[e] = (gidx, blob, attr4)
        IndT = np.zeros((NPAD, G), np.float32)
        lo, hi = r * NPR, min((r + 1) * NPR, N)
        IndT[np.arange(hi - lo), batch[lo:hi]] = inv_cnt[batch[lo:hi]]
        xT_loc = np.zeros((128, NPAD), np.float32)
        xT_loc[:, 0:hi - lo] = x[lo:hi].T
        x_rb = np.zeros((R * 128, NPAD), np.float32)
        for rr in range(R):
            rlo, rhi = rr * NPR, min((rr + 1) * NPR, N)
            x_rb[rr * 128:(rr + 1) * 128, 0:rhi - rlo] = x[rlo:rhi].T
        gcnt = np.maximum(counts[:, r, :], 1).astype(np.int32).reshape(1, 2 * NGRP)
        im = {
            "gcnt": gcnt,
            "xT_loc": xT_loc.astype(ml_dtypes.float8_e4m3),
            "x_rb": x_rb.astype(ml_dtypes.float8_e4m3),
            "Wqvk": Wqvk.astype(ml_dtypes.bfloat16),
            "Ws": Ws_e.astype(ml_dtypes.bfloat16),
            "BD": BD.astype(ml_dtypes.bfloat16),
            "bv_row": np.concatenate(
                [np.zeros((NLAYER, 1, H), np.float32),
                 bv_e.reshape(NLAYER, 1, H),
                 np.zeros((NLAYER, 1, H), np.float32)], axis=2
            ).astype(ml_dtypes.bfloat16),
            "bs_col": bs_e.reshape(NLAYER, H, 1),
            "gidx0": ep_arrs[0][0], "blob0": ep_arrs[0][1], "attr0": ep_arrs[0][2],
            "gidx1": ep_arrs[1][0], "blob1": ep_arrs[1][1], "attr1": ep_arrs[1][2],
            "IndT": IndT.astype(ml_dtypes.bfloat16),
            "clinT": clinical.T.copy(),
            "Wc_h": Wc_h, "Wc_c": Wc[H:H + NCLIN],
            "bc_rep": np.tile(bc_e, (G, 1)),
        }
        in_maps.append(im)
    meta = dict(NT0=tuple(int(v) for v in nt_l[0]),
                NT1=tuple(int(v) for v in nt_l[1]),
                has_bv=has_bv, has_bs=has_bs, tab_fp8=True, dr_seg=True)
    return in_maps, meta


# ---------------------------------------------------------------------------
# device program
# ---------------------------------------------------------------------------

def build(meta):
    parts = meta.get("parts", 4)
    DT = BF16
    TDT = FP8 if meta.get("tab_fp8", True) else BF16
    NT_L = [list(meta["NT0"]), list(meta["NT1"])]
    OFF_L = [np.concatenate([[0], np.cumsum(nt)]).astype(int) for nt in NT_L]
    NCH_L = [np.ceil(np.array(nt) / KQC).astype(int) for nt in NT_L]
    CHOFF_L = [np.concatenate([[0], np.cumsum(nc)]).astype(int) for nc in NCH_L]
    TOT = [int(o[-1]) for o in OFF_L]
    CHTOT = [int(c[-1]) for c in CHOFF_L]
    TMAX = max(max(NT_L[0]), max(NT_L[1]))

    nc = bacc.Bacc("TRN2", target_bir_lowering=False, debug=False, num_devices=R)

    def din(name, shape, dt):
        return nc.dram_tensor(name, shape, dt, kind="ExternalInput").ap()

    t_xT = din("xT_loc", [128, NPAD], FP8)
    t_cnt = din("gcnt", [1, 2 * NGRP], mybir.dt.int32)
    t_xrb = din("x_rb", [R * 128, NPAD], FP8)
    t_Wqvk = din("Wqvk", [NLAYER, H, 3 * H], DT)
    t_Ws = din("Ws", [NLAYER, H, H], DT)
    t_BD = din("BD", [NLAYER, 5, KQC * H], DT)
    t_bv = din("bv_row", [NLAYER, 1, 3 * H], DT)
    t_bs = din("bs_col", [NLAYER, H, 1], F32)
    t_gidx = [din("gidx0", [128, TOT[0] * 8], I16),
              din("gidx1", [128, TOT[1] * 8], I16)]
    t_blob = [din("blob0", [128, TOT[0] * 256], FP8),
              din("blob1", [128, TOT[1] * 256], FP8)]
    t_attr = [din("attr0", [5, CHTOT[0] * 128], FP8),
              din("attr1", [5, CHTOT[1] * 128], FP8)]
    t_IndT = din("IndT", [NPAD, G], DT)
    t_clinT = din("clinT", [NCLIN, G], F32)
    t_Wc_h = din("Wc_h", [H, NCLS], F32)
    t_Wc_c = din("Wc_c", [NCLIN, NCLS], F32)
    t_bc = din("bc_rep", [G, NCLS], F32)

    t_out = nc.dram_tensor("out", [G, NCLS], F32, kind="ExternalOutput").ap()

    qv_loc = nc.dram_tensor("qv_loc", [NPAD, 2 * H], TDT).ap()
    # per-layer AG'd table pairs (separate per layer so the next layer's
    # AllGather never overwrites a table the current layer still gathers from)
    qv_tl = [None,
             [nc.dram_tensor("qv1_t0", [R * C0N, 2 * H], TDT, addr_space="Shared").ap(),
              nc.dram_tensor("qv1_t1", [R * C1N, 2 * H], TDT, addr_space="Shared").ap()],
             [nc.dram_tensor("qv2_t0", [R * C0N, 2 * H], TDT, addr_space="Shared").ap(),
              nc.dram_tensor("qv2_t1", [R * C1N, 2 * H], TDT, addr_space="Shared").ap()]]
    # layer-0 tables are built locally (x is replicated), no collective
    qv_tl[0] = [nc.dram_tensor("qv0_t0", [R * C0N, 2 * H], TDT).ap(),
                nc.dram_tensor("qv0_t1", [R * C1N, 2 * H], TDT).ap()]
    pool_in = nc.dram_tensor("pool_in", [G, H], F32).ap()
    pool_out = nc.dram_tensor("pool_out", [R * G, H], F32, addr_space="Shared").ap()

    with tile.TileContext(nc) as tc:
        import contextlib
        with contextlib.ExitStack() as ctx:
            consts = ctx.enter_context(tc.tile_pool(name="consts", bufs=1))
            hsb = ctx.enter_context(tc.tile_pool(name="hsb", bufs=1))
            h3p = ctx.enter_context(tc.tile_pool(name="h3p", bufs=1))
            ksb = ctx.enter_context(tc.tile_pool(name="ksb", bufs=1))
            stg = ctx.enter_context(tc.tile_pool(name="stg", bufs=4))
            tail = ctx.enter_context(tc.tile_pool(name="tail", bufs=2))
            edg = ctx.enter_context(tc.tile_pool(name="edg", bufs=5))
            sel = ctx.enter_context(tc.tile_pool(name="sel", bufs=8))
            edm = ctx.enter_context(tc.tile_pool(name="edm", bufs=3))
            pnode = ctx.enter_context(tc.tile_pool(name="pnode", bufs=2, space="PSUM"))
            pkq = ctx.enter_context(tc.tile_pool(name="pkq", bufs=2, space="PSUM"))
            pedge = ctx.enter_context(tc.tile_pool(name="pedge", bufs=2, space="PSUM"))
            ppool = ctx.enter_context(tc.tile_pool(name="ppool", bufs=1, space="PSUM"))

            _cid = [0]

            def load_const(src_ap, shape, dt):
                _cid[0] += 1
                t = consts.tile(shape, dt, tag=f"c{_cid[0]}_{src_ap.tensor.name}")
                nc.sync.dma_start(t[:], src_ap)
                return t

            Wqvk_t = [load_const(t_Wqvk[l], [H, 3 * H], DT) for l in range(NLAYER)]
            Ws_t = [load_const(t_Ws[l], [H, H], DT) for l in range(NLAYER)]
            BD_t = [load_const(t_BD[l], [5, KQC * H], DT) for l in range(NLAYER)]
            bv_t = ([load_const(t_bv[l], [1, 3 * H], DT) for l in range(NLAYER)]
                    if meta["has_bv"] else None)
            bs_t = ([load_const(t_bs[l], [H, 1], F32) for l in range(NLAYER)]
                    if meta["has_bs"] else None)
            ones_t = consts.tile([1, 128], DT)
            nc.vector.memset(ones_t[:], 1.0)
            gidx_t = [load_const(t_gidx[0], [128, TOT[0] * 8], I16),
                      load_const(t_gidx[1], [128, TOT[1] * 8], I16)]
            attr_t = [load_const(t_attr[0], [5, CHTOT[0] * 128], FP8),
                      load_const(t_attr[1], [5, CHTOT[1] * 128], FP8)]
            xT_t = load_const(t_xT, [128, NPAD], FP8)
            cnt_t = load_const(t_cnt, [1, 2 * NGRP], mybir.dt.int32)
            ident = consts.tile([128, 128], DT)
            make_identity(nc, ident[:])
            identf = consts.tile([128, 128], F32)
            make_identity(nc, identf[:])
            clin_t = load_const(t_clinT, [NCLIN, G], F32)
            Wch_t = load_const(t_Wc_h, [H, NCLS], F32)
            Wcc_t = load_const(t_Wc_c, [NCLIN, NCLS], F32)
            bc_t = load_const(t_bc, [G, NCLS], F32)

            # ping-pong state by layer parity
            hs_pp = [hsb.tile([128, NPAD], DT, tag="hs0", name="hs0"),
                     hsb.tile([128, NPAD], DT, tag="hs1", name="hs1")]
            k_pp = [ksb.tile([128, NGRP, H], DT, tag="k0", name="k0"),
                    ksb.tile([128, NGRP, H], DT, tag="k1", name="k1")]
            h3_pp = [h3p.tile([128, NPAD], DT, tag="h3a", name="h3a"),
                     h3p.tile([128, NPAD], DT, tag="h3b", name="h3b")]
            h3f = h3_pp[0]                             # layer-2 output parity

            def build_chunk(l, c, hsrc):
                """Local q|v|k for groups of AG-chunk c of layer l."""
                g0, g1 = (0, C0G) if c == 0 else (C0G, NGRP)
                k_sb = k_pp[l % 2]
                g = g0
                while g < g1:
                    ns = min(8, g1 - g)
                    st = stg.tile([128, 8, 2 * H], TDT, tag="st")
                    for s in range(ns):
                        gg = g + s
                        ps = pnode.tile([128, 512], F32, tag="pn")
                        nc.tensor.matmul(
                            out=ps[:, 0:3 * H],
                            lhsT=hsrc[:, gg * 128:(gg + 1) * 128],
                            rhs=Wqvk_t[l][:], start=True,
                            stop=not meta["has_bv"],
                            skip_group_check=True)
                        if meta["has_bv"]:
                            nc.tensor.matmul(
                                out=ps[:, 0:3 * H], lhsT=ones_t[:],
                                rhs=bv_t[l][:], start=False, stop=True,
                                skip_group_check=True)
                        nc.scalar.activation(st[:, s, :], ps[:, 0:2 * H],
                                             AF.Copy)
                        nc.scalar.activation(k_sb[:, gg, :],
                                             ps[:, 2 * H:3 * H], AF.Copy)
                    nc.sync.dma_start(
                        qv_loc[g * 128:(g + ns) * 128, :].rearrange(
                            "(s p) f -> p s f", p=128),
                        st[:, 0:ns, :])
                    g += ns

            def emit_ag(l, c):
                ins = qv_loc[0:C0N, :] if c == 0 else qv_loc[C0N:NPAD, :]
                nc.gpsimd.collective_compute(
                    "AllGather", OP.bypass, replica_groups=[list(range(R))],
                    ins=[ins], outs=[qv_tl[l][c][:]])

            def emit_spart(l, hsrc):
                hs = hs_pp[l % 2]
                for c0 in range(0, NPAD, 512):
                    csz = min(512, NPAD - c0)
                    pss = pnode.tile([128, 512], F32, tag="pn")
                    nc.tensor.matmul(out=pss[:, 0:csz], lhsT=Ws_t[l][:],
                                     rhs=hsrc[:, c0:c0 + csz],
                                     start=True, stop=True)
                    if meta["has_bs"]:
                        nc.scalar.activation(hs[:, c0:c0 + csz], pss[:, 0:csz],
                                             AF.Identity, bias=bs_t[l][:],
                                             scale=1.0)
                    else:
                        nc.scalar.activation(hs[:, c0:c0 + csz], pss[:, 0:csz],
                                             AF.Copy)

            def make_tail_cb(l, c_start, c_end):
                # During layer l's ep1 sweep over [c_start, c_end): as each
                # dst-group finalizes, leaky just that slice and build the
                # next layer's table rows for it, storing every 8 groups so
                # the chunk AllGather can fire the moment the sweep ends.
                st_state = {}

                def cb(g, l=l):
                    sl = slice(g * 128, (g + 1) * 128)
                    nc.vector.scalar_tensor_tensor(
                        out=h3_pp[l % 2][:, sl], in0=hs_pp[l % 2][:, sl],
                        scalar=float(SLOPE), in1=hs_pp[l % 2][:, sl],
                        op0=OP.mult, op1=OP.max)
                    if l >= NLAYER - 1:
                        return
                    s = (g - c_start) % 8
                    if s == 0:
                        st_state["st"] = stg.tile([128, 8, 2 * H], TDT,
                                                  tag="st", name="st_cb")
                        st_state["g0"] = g
                    st = st_state["st"]
                    ps = pnode.tile([128, 512], F32, tag="pn", name="pn_cb")
                    nc.tensor.matmul(
                        out=ps[:, 0:3 * H],
                        lhsT=h3_pp[l % 2][:, sl],
                        rhs=Wqvk_t[l + 1][:], start=True,
                        stop=not meta["has_bv"], skip_group_check=True)
                    if meta["has_bv"]:
                        nc.tensor.matmul(
                            out=ps[:, 0:3 * H], lhsT=ones_t[:],
                            rhs=bv_t[l + 1][:], start=False, stop=True,
                            skip_group_check=True)
                    nc.scalar.activation(st[:, s, :], ps[:, 0:2 * H], AF.Copy)
                    nc.scalar.activation(k_pp[(l + 1) % 2][:, g, :],
                                         ps[:, 2 * H:3 * H], AF.Copy)
                    if s == 7 or g == c_end - 1:
                        g0b = st_state["g0"]
                        nc.sync.dma_start(
                            qv_loc[g0b * 128:(g + 1) * 128, :].rearrange(
                                "(s p) f -> p s f", p=128),
                            st[:, 0:g + 1 - g0b, :])
                return cb

            def emit_leaky(l, c):
                sl = slice(0, C0N) if c == 0 else slice(C0N, NPAD)
                nc.vector.scalar_tensor_tensor(
                    out=h3_pp[l % 2][:, sl], in0=hs_pp[l % 2][:, sl],
                    scalar=float(SLOPE), in1=hs_pp[l % 2][:, sl],
                    op0=OP.mult, op1=OP.max)

            def emit_edges(e, l, g0, g1, after_group=None):
                nt_list = NT_L[e]
                off = OFF_L[e]
                choff = CHOFF_L[e]
                hs = hs_pp[l % 2]
                k_sb = k_pp[l % 2]
                tab = qv_tl[l][e]
                for g in range(g0, g1):
                    nt = int(nt_list[g])
                    if nt == 0:
                        if after_group is not None:
                            after_group(g)
                        continue
                    ne = nt * 128
                    o = int(off[g])
                    co = int(choff[g])
                    bt = sel.tile([128, TMAX, 256], FP8, tag="blob")
                    nc.sync.dma_start(
                        bt[:, 0:nt, :],
                        t_blob[e][:, o * 256:(o + nt) * 256].rearrange(
                            "p (t d) -> p t d", t=nt))
                    gt = edg.tile([128, TMAX, 2 * H], TDT, tag="g")
                    nreg = nc.gpsimd.value_load(
                        cnt_t[0:1, e * NGRP + g:e * NGRP + g + 1])
                    nc.gpsimd.dma_gather(
                        gt[:, 0:nt, :], tab[:],
                        gidx_t[e][:, o * 8:(o + nt) * 8],
                        ne, nreg, 2 * H, single_packet=(ne <= 512))
                    ktb = edm.tile([128, TMAX, 128], DT, tag="kt")
                    for ci in range((nt + KQC - 1) // KQC):
                        q0 = ci * KQC
                        qn = min(KQC, nt - q0)
                        pk = pkq.tile([128, KQC, 128], F32, tag="kq")
                        nc.tensor.matmul(
                            out=pk[:, 0:KQC, :],
                            lhsT=attr_t[e][0:5, (co + ci) * 128:(co + ci + 1) * 128],
                            rhs=BD_t[l][:], start=True, stop=False,
                            skip_group_check=True)
                        for t in range(q0, q0 + qn):
                            nc.tensor.matmul(
                                out=pk[:, t - q0, :],
                                lhsT=bt[:, t, 128:256],
                                rhs=k_sb[:, g, :], start=False, stop=False,
                                skip_group_check=True)
                        # q[src] folded into the same PSUM via identity matmul
                        nc.tensor.matmul(
                            out=pk[:, 0:qn, :], lhsT=ident[:],
                            rhs=gt[:, q0:q0 + qn, 0:H], start=False, stop=True,
                            skip_group_check=True)
                        nc.scalar.activation(ktb[:, q0:q0 + qn, :],
                                             pk[:, 0:qn, :], AF.Sigmoid)
                    dr = meta.get("dr_seg", False)
                    msg = edm.tile([128, TMAX, 128], FP8 if dr else DT, tag="msg")
                    nc.vector.tensor_tensor(out=msg[:, 0:nt, :],
                                            in0=ktb[:, 0:nt, :],
                                            in1=gt[:, 0:nt, H:2 * H], op=OP.mult)
                    pa = pedge.tile([128, 128], F32, tag="pa")
                    if dr:
                        npair = nt // 2
                        for t2 in range(0, npair * 2, 2):
                            nc.tensor.matmul(
                                out=pa[:], lhsT=msg[:, t2:t2 + 2, :],
                                rhs=bt[:, t2:t2 + 2, 0:128], start=(t2 == 0),
                                stop=(t2 + 2 == nt),
                                perf_mode=mybir.MatmulPerfMode.DoubleRow,
                                skip_group_check=True)
                        if nt % 2:
                            nc.tensor.matmul(out=pa[:], lhsT=msg[:, nt - 1, :],
                                             rhs=bt[:, nt - 1, 0:128],
                                             start=(nt == 1), stop=True,
                                             skip_group_check=True)
                    else:
                        for t in range(nt):
                            nc.tensor.matmul(out=pa[:], lhsT=msg[:, t, :],
                                             rhs=bt[:, t, 0:128], start=(t == 0),
                                             stop=(t == nt - 1))
                    nc.vector.tensor_tensor(
                        out=hs[:, g * 128:(g + 1) * 128],
                        in0=hs[:, g * 128:(g + 1) * 128], in1=pa[:], op=OP.add)
                    if after_group is not None:
                        after_group(g)

            # ---- layer-0 table from x: replicated input, so each rank builds
            # the FULL table locally (no collective, no exposed startup AG).
            def build_l0_chunk(c):
                g0, g1, tab, cbase = ((0, C0G, qv_tl[0][0], C0N) if c == 0
                                      else (C0G, NGRP, qv_tl[0][1], C1N))
                for rb in range(R):
                    g = g0
                    while g < g1:
                        ns = min(8, g1 - g)
                        lh = stg.tile([128, 1024], FP8, tag="lh")
                        nc.sync.dma_start(
                            lh[:, 0:ns * 128],
                            t_xrb[rb * 128:(rb + 1) * 128,
                                  g * 128:(g + ns) * 128])
                        st = stg.tile([128, 8, 2 * H], TDT, tag="st")
                        for s in range(0, ns, 2):
                            n2 = min(2, ns - s)
                            ps = pnode.tile([128, 512], F32, tag="pn")
                            for u in range(n2):
                                nc.tensor.matmul(
                                    out=ps[:, u * 256:u * 256 + 2 * H],
                                    lhsT=lh[:, (s + u) * 128:(s + u + 1) * 128],
                                    rhs=Wqvk_t[0][:, 0:2 * H], start=True,
                                    stop=not meta["has_bv"],
                                    skip_group_check=True)
                                if meta["has_bv"]:
                                    nc.tensor.matmul(
                                        out=ps[:, u * 256:u * 256 + 2 * H],
                                        lhsT=ones_t[:],
                                        rhs=bv_t[0][0:1, 0:2 * H],
                                        start=False, stop=True,
                                        skip_group_check=True)
                            if (s // 2) % 2 == 0:
                                nc.scalar.activation(st[:, s:s + n2, :],
                                                     ps[:, 0:n2 * 256], AF.Copy)
                            else:
                                nc.vector.tensor_copy(st[:, s:s + n2, :],
                                                      ps[:, 0:n2 * 256])
                        r0 = rb * cbase + (g - g0) * 128
                        nc.sync.dma_start(
                            tab[r0:r0 + ns * 128, :].rearrange(
                                "(s p) f -> p s f", p=128),
                            st[:, 0:ns, :])
                        g += ns
            build_l0_chunk(0)
            # local k table + s-part for layer 0 from the local x slice
            for g in range(NGRP):
                psk = pnode.tile([128, 512], F32, tag="pn")
                nc.tensor.matmul(out=psk[:, 0:H],
                                 lhsT=xT_t[:, g * 128:(g + 1) * 128],
                                 rhs=Wqvk_t[0][:, 2 * H:3 * H],
                                 start=True, stop=True, skip_group_check=True)
                nc.scalar.activation(k_pp[0][:, g, :], psk[:, 0:H], AF.Copy)
            emit_spart(0, xT_t)

            for _ in range(5):
                gz = edg.tile([128, TMAX, 2 * H], TDT, tag="g")
                nc.vector.memset(gz[:], 0.0)

            pp = ppool.tile([G, H], F32)
            indc = consts.tile([128, NGRP, G], DT)
            nc.sync.dma_start(
                indc[:], t_IndT[:].rearrange("(c p) g -> p c g", p=128))

            def make_pool_cb(l):
                def cb(g, l=l):
                    sl = slice(g * 128, (g + 1) * 128)
                    nc.vector.scalar_tensor_tensor(
                        out=h3_pp[l % 2][:, sl], in0=hs_pp[l % 2][:, sl],
                        scalar=float(SLOPE), in1=hs_pp[l % 2][:, sl],
                        op0=OP.mult, op1=OP.max)
                    trp = pedge.tile([128, 128], DT, tag="pa", name="trp_cb")
                    nc.tensor.transpose(out=trp[:], in_=h3f[:, sl],
                                        identity=ident[:])
                    hnode = tail.tile([128, 128], DT, tag="hnode",
                                      name="hnode_cb")
                    nc.vector.tensor_copy(hnode[:], trp[:])
                    nc.tensor.matmul(out=pp[:], lhsT=indc[:, g, :],
                                     rhs=hnode[:],
                                     start=(g == 0), stop=(g == NGRP - 1))
                return cb

            def emit_pool_part(c0g, c1g):
                for c in range(c0g, c1g):
                    trp = pedge.tile([128, 128], DT, tag="pa")
                    nc.tensor.transpose(out=trp[:],
                                        in_=h3f[:, c * 128:(c + 1) * 128],
                                        identity=ident[:])
                    hnode = tail.tile([128, 128], DT, tag="hnode")
                    nc.vector.tensor_copy(hnode[:], trp[:])
                    nc.tensor.matmul(out=pp[:],
                                     lhsT=indc[:, c, :],
                                     rhs=hnode[:],
                                     start=(c == 0), stop=(c == NGRP - 1))

            for l in range(NLAYER):
                hsrc_next = h3_pp[l % 2]
                last = l >= NLAYER - 1
                if parts >= 3 and not last:
                    cb0 = make_tail_cb(l, 0, C0G)
                    cb1 = make_tail_cb(l, C0G, NGRP)
                elif parts >= 4 and last:
                    cb0 = cb1 = make_pool_cb(l)
                else:
                    cb0 = cb1 = None
                if parts >= 2:
                    if l == 0:
                        # layer 0 consumes no AllGather (local tables), so run
                        # both epochs' first halves up front and fire the hook
                        # at ~50% -- the next layer's AG0 starts much earlier.
                        emit_edges(0, l, 0, 12)
                        build_l0_chunk(1)
                        emit_edges(0, l, 12, C0G)
                        emit_edges(1, l, 0, C0G, after_group=cb0)
                    else:
                        # full epoch-0 sweep first (its table is ready), then
                        # the first ep1 half; this keeps ep0 work ahead of any
                        # ep1 stall on the second table chunk's AllGather.
                        emit_edges(0, l, 0, NGRP)
                        emit_edges(1, l, 0, C0G, after_group=cb0)
                # h3 chunk-0 final: fire next layer's chunk-0 AllGather (the
                # per-group callback already built and stored the rows)
                if not last:
                    if parts < 2:
                        emit_leaky(l, 0)
                        build_chunk(l + 1, 0, hsrc_next)
                    emit_ag(l + 1, 0)
                elif cb0 is None:
                    emit_leaky(l, 0)
                if parts >= 2:
                    if l == 0:
                        emit_edges(0, l, C0G, NGRP)
                    emit_edges(1, l, C0G, NGRP, after_group=cb1)
                if parts < 3:
                    break
                if not last:
                    emit_ag(l + 1, 1)
                    emit_spart(l + 1, hsrc_next)
                elif cb1 is None:
                    emit_leaky(l, 1)
            if parts < 4:
                z_dbg = tail.tile([G, NCLS], F32, tag="zsb")
                nc.vector.tensor_copy(z_dbg[:], hs_pp[0][0:G, 0:NCLS])
                nc.sync.dma_start(t_out[:], z_dbg[:])
            else:
                pool_sb = tail.tile([G, H], F32, tag="poolsb")
                nc.vector.tensor_copy(pool_sb[:], pp[:])
                nc.sync.dma_start(pool_in[:], pool_sb[:])
                nc.gpsimd.collective_compute(
                    "AllGather", OP.bypass, replica_groups=[list(range(R))],
                    ins=[pool_in[:]], outs=[pool_out[:]])
                pr = tail.tile([G, R, H], F32, tag="pr")
                nc.sync.dma_start(pr[:], pool_out[:].rearrange("(r g) h -> g r h", r=R))
                pooled = tail.tile([G, H], F32, tag="pooled")
                nc.vector.tensor_tensor(out=pooled[:], in0=pr[:, 0, :], in1=pr[:, 1, :],
                                        op=OP.add)
                for r in range(2, R):
                    nc.vector.tensor_tensor(out=pooled[:], in0=pooled[:],
                                            in1=pr[:, r, :], op=OP.add)
                ptp = pedge.tile([H, G], F32, tag="pa")
                nc.tensor.transpose(out=ptp[:], in_=pooled[:], identity=identf[0:G, 0:G])
                pooledT = tail.tile([H, G], F32, tag="pooledT")
                nc.vector.tensor_copy(pooledT[:], ptp[:])
                zp = pedge.tile([G, NCLS], F32, tag="pa")
                nc.tensor.matmul(out=zp[:], lhsT=pooledT[:], rhs=Wch_t[:],
                                 start=True, stop=False)
                nc.tensor.matmul(out=zp[:], lhsT=clin_t[:], rhs=Wcc_t[:],
                                 start=False, stop=True)
                z_sb = tail.tile([G, NCLS], F32, tag="zsb")
                nc.vector.tensor_tensor(out=z_sb[:], in0=zp[:], in1=bc_t[:],
                                        op=OP.add)
                nc.sync.dma_start(t_out[:], z_sb[:])

    nc.compile()
    return nc


# ---------------------------------------------------------------------------

_CACHE = {}


def kernel(**inputs):
    in_maps, meta = prep(inputs)
    key = tuple(sorted((k, v) for k, v in meta.items()))
    if key not in _CACHE:
        _CACHE[key] = build(meta)
    nc = _CACHE[key]
    res = run_bass_kernel_spmd(nc, in_maps, list(range(R)))
    return np.asarray(res.results[0]["out"], np.float32)


def kernel_profiled(**inputs):
    """Like kernel() but also returns (exec_time_ns, trace_path)."""
    in_maps, meta = prep(inputs)
    key = tuple(sorted((k, v) for k, v in meta.items()))
    if key not in _CACHE:
        _CACHE[key] = build(meta)
    nc = _CACHE[key]
    res = run_bass_kernel_spmd(nc, in_maps, list(range(R)), trace=True)
    out = np.asarray(res.results[0]["out"], np.float32)
    trace_path = None
    if res.instructions_and_trace is not None:
        trace_path = res.instructions_and_trace[1]
    return out, res.exec_time_ns, trace_path


if __name__ == "__main__":
    pass
